# revision 1
# baseline (speedup 1.0000x reference)
"""Trainium2 Bass kernel for BillboardAllocatorGNN.

Sharding: 8 cores; core c handles sample c//2, node-half c%2 (data parallel
over batch, dst-parallel within each sample pair).

Edge phase (per layer): node-major edge slots with fixed per-node capacity
K=18 kill both the dst-side gather and the one-hot scatter for 96% of edges:
a single SWDGE gather fetches duplicated-bf16 hs1 rows (256B descriptors)
into [dst-node-partition, slot] layout, the dst-side hs2 contribution is a
free-dim broadcast add from SBUF, and segment-sum is a strided free-axis
tensor_reduce. Overflow edges (deg>K) go through a small one-hot matmul
side path whose PSUM accumulation group also hosts the main agg transpose.
Pad slots gather a -30000 sentinel row so relu zeroes them.

Pipelining: updates, next-layer table builds, and attention score/value
chunks are interleaved into the main gather stream. Layer 0 builds its
table locally from raw features (no collective); layer 1 exchanges hs1
in slices issued as updates complete; the final attention merges across
the pair via a 2KB unnormalized-softmax stats exchange (scores are O(1),
so no max subtraction is needed).
"""
import sys
import os

sys.path.insert(0, "/opt/trn_rl_repo")

import numpy as np
import ml_dtypes

# ---- problem dims (hardcoded per spec) ----
B, N, E = 4, 20000, 320000
F_NODE, F_AD = 16, 8
H, L, HEADS = 64, 2, 4
DH = H // HEADS

NCORES = 8
N_HALF = N // 2                 # 10000 real nodes per core
NBUCK = 79                      # 128-node pages per core
NPAD = NBUCK * 128              # 10112 padded nodes per core
K = 18                          # main slots per node
MCOLS = NBUCK * K               # 1422 main gather columns
PPG = 2                         # pages per main gather group
NGRP = (NBUCK + PPG - 1) // PPG  # 27
OVFCH = 2                       # overflow chunks per page (cap 256 edges)
OCOLS = NBUCK * OVFCH           # 158 overflow columns
OG = 12                         # overflow chunks per gather group
NTAB = 2 * NPAD                 # hs1 rows (both halves)
HS2OFF = NTAB                   # hs2 rows start
SENT = NTAB + NPAD              # sentinel row id
TROWS = SENT + 1
CH = 512                        # node-chunk for encoder/update/attention
NEG = np.float32(-1e9)

_CACHE = {}


def _ovf_groups():
    out = []
    c = 0
    while c < OCOLS:
        g = min(OG, OCOLS - c)
        out.append((c, g))
        c += g
    return out


def _main_groups():
    out = []
    b = 0
    while b < NBUCK:
        g = min(PPG, NBUCK - b)
        out.append((b, g))
        b += g
    return out


def _build():
    import concourse.mybir as mybir
    import concourse.tile as tile
    import concourse.bacc as bacc
    from concourse.tile import add_dep_helper
    from concourse.masks import make_identity

    f32 = mybir.dt.float32
    bf16 = mybir.dt.bfloat16
    i16 = mybir.dt.int16
    AF = mybir.ActivationFunctionType
    OP = mybir.AluOpType

    nc = bacc.Bacc("TRN2", target_bir_lowering=False, debug=False,
                   num_swdge_queues=2)

    # ---- I/O ----
    d_xT = nc.dram_tensor("xT", [F_NODE, NPAD], bf16, kind="ExternalInput")
    d_xTF = nc.dram_tensor("xTF", [F_NODE, NTAB], bf16, kind="ExternalInput")
    d_cmbM = nc.dram_tensor("cmbM", [128, MCOLS * 8], i16, kind="ExternalInput")
    d_cmbO = nc.dram_tensor("cmbO", [128, 2 * OCOLS * 8], i16,
                            kind="ExternalInput")
    d_dlocO = nc.dram_tensor("dlocO", [128, 2 * OCOLS], bf16,
                             kind="ExternalInput")
    d_iota = nc.dram_tensor("iota128", [128, 128], bf16, kind="ExternalInput")
    d_maskf = nc.dram_tensor("maskf", [128, NBUCK], f32, kind="ExternalInput")
    d_maskn = nc.dram_tensor("maskneg", [128, NBUCK], f32, kind="ExternalInput")
    d_wnode = nc.dram_tensor("w_node16", [F_NODE, H], bf16,
                             kind="ExternalInput")
    d_bnode = nc.dram_tensor("b_node_col", [H, 1], f32, kind="ExternalInput")
    d_msgw1 = nc.dram_tensor("msgw1", [H + 1, L, H], bf16,
                             kind="ExternalInput")
    d_msgw2 = nc.dram_tensor("msgw2", [H, L, H], bf16, kind="ExternalInput")
    d_brep = nc.dram_tensor("bias_rep", [128, L, H], f32, kind="ExternalInput")
    d_updw1 = nc.dram_tensor("updw1", [H, L, H], bf16, kind="ExternalInput")
    d_updw2 = nc.dram_tensor("updw2", [H, L, H], bf16, kind="ExternalInput")
    d_updb = nc.dram_tensor("upd_b_col", [H, L], f32, kind="ExternalInput")
    d_wad = nc.dram_tensor("w_ad", [F_AD, H], f32, kind="ExternalInput")
    d_bad = nc.dram_tensor("b_ad_row", [1, H], f32, kind="ExternalInput")
    d_ad = nc.dram_tensor("ad_col", [F_AD, 1], f32, kind="ExternalInput")
    d_wq = nc.dram_tensor("wq", [H, H], f32, kind="ExternalInput")
    d_bq = nc.dram_tensor("bq_row", [1, H], f32, kind="ExternalInput")
    d_wk16 = nc.dram_tensor("wk16", [H, H], bf16, kind="ExternalInput")
    d_wv16 = nc.dram_tensor("wv16", [H, H], bf16, kind="ExternalInput")
    d_bv = nc.dram_tensor("bv_col", [H, 1], f32, kind="ExternalInput")
    d_wo = nc.dram_tensor("wo", [H, H], f32, kind="ExternalInput")
    d_bo = nc.dram_tensor("bo_row", [1, H], f32, kind="ExternalInput")
    d_lng = nc.dram_tensor("ln_g_row", [1, H], f32, kind="ExternalInput")
    d_lnb = nc.dram_tensor("ln_b_row", [1, H], f32, kind="ExternalInput")
    d_qmask = nc.dram_tensor("qmask", [H, HEADS], f32, kind="ExternalInput")
    d_cmask = nc.dram_tensor("cmask", [H, HEADS], f32, kind="ExternalInput")
    d_pol = nc.dram_tensor("pol_row", [1, H], f32, kind="ExternalInput")
    d_out = nc.dram_tensor("logits", [128, NBUCK], f32, kind="ExternalOutput")

    PAIRS = [[0, 1], [2, 3], [4, 5], [6, 7]]
    n_ch = (NPAD + CH - 1) // CH
    TB = 4                      # node pages per table-export batch

    with tile.TileContext(nc) as tc:
        with (
            tc.tile_pool(name="persist", bufs=1) as pp,
            tc.tile_pool(name="mp", bufs=2) as mp,
            tc.tile_pool(name="g3", bufs=3) as g3,
            tc.tile_pool(name="single", bufs=1) as sp,
            tc.tile_pool(name="psum", bufs=2, space="PSUM") as ps,
            tc.tile_pool(name="dram", bufs=1, space="DRAM") as dp,
        ):
            # ---- persistent state / constants ----
            hT = pp.tile([H + 1, NPAD], bf16)      # node states + ones row
            aggT = pp.tile([H, NPAD], bf16)        # per-layer aggregate
            hs2 = pp.tile([128, NBUCK, H], bf16)   # dst table, node-major
            ovfmsg = pp.tile([128, OCOLS, H], bf16)
            cmbM = pp.tile([128, MCOLS * 8], i16)
            cmbO = pp.tile([128, 2 * OCOLS * 8], i16)
            dlocO = pp.tile([128, 2 * OCOLS], bf16)
            iota = pp.tile([128, 128], bf16)
            ident = pp.tile([128, 128], f32)
            wnode = pp.tile([F_NODE, H], bf16)
            bnode = pp.tile([H, 1], f32)
            msgw1 = pp.tile([H + 1, L, H], bf16)
            msgw2 = pp.tile([H, L, H], bf16)
            updw1 = pp.tile([H, L, H], bf16)
            updw2 = pp.tile([H, L, H], bf16)
            updb = pp.tile([H, L], f32)

            make_identity(nc, ident[:])
            nc.gpsimd.memset(hT[H:H + 1, :], 1.0)
            ident_bf4 = pp.tile([HEADS, HEADS], bf16)
            nc.vector.tensor_copy(out=ident_bf4[:], in_=ident[:HEADS, :HEADS])

            for dst_t, src_t in [
                (cmbM, d_cmbM), (cmbO, d_cmbO), (dlocO, d_dlocO),
                (iota, d_iota), (wnode, d_wnode), (bnode, d_bnode),
                (msgw1, d_msgw1), (msgw2, d_msgw2),
                (updw1, d_updw1), (updw2, d_updw2), (updb, d_updb),
            ]:
                nc.sync.dma_start(out=dst_t[:], in_=src_t[:])

            # ---- DRAM: gather table, hs1 exchange, attention payload ----
            tab = dp.tile([TROWS, 128], bf16)
            hs1own = dp.tile([NPAD, H], bf16)
            hs1pair = dp.tile([NTAB, H], bf16)
            pay_own = dp.tile([HEADS, H + 2], f32)
            pay_full = dp.tile([2, HEADS, H + 2], f32)

            # sentinel row (once; hs1/hs2 rewrites never touch it)
            sentc = sp.tile([1, 128], bf16, tag="sent")
            nc.gpsimd.memset(sentc[:], -30000.0)
            w_sent = nc.sync.dma_start(out=tab[SENT:SENT + 1, :], in_=sentc[:])

            # ---- node encoder: hT = relu(w_node^T @ xT + b) ----
            for i in range(n_ch):
                lo = i * CH
                sz = min(CH, NPAD - lo)
                xt = mp.tile([F_NODE, CH], bf16, tag="xtf")
                nc.sync.dma_start(out=xt[:, :sz], in_=d_xT[:, lo:lo + sz])
                h_ps = ps.tile([H, CH], f32, tag="big", space="PSUM")
                nc.tensor.matmul(h_ps[:, :sz], lhsT=wnode[:], rhs=xt[:, :sz],
                                 start=True, stop=True)
                nc.vector.tensor_scalar(out=hT[:H, lo:lo + sz],
                                        in0=h_ps[:, :sz], scalar1=bnode[:],
                                        scalar2=0.0, op0=OP.add, op1=OP.max)

            # ---- attention prelude: q from ad (independent of layers) ----
            wad = sp.tile([F_AD, H], f32, tag="w64")
            adc = sp.tile([F_AD, 1], f32, tag="col")
            nc.sync.dma_start(out=wad[:], in_=d_wad[:])
            nc.sync.dma_start(out=adc[:], in_=d_ad[:])
            a_ps = ps.tile([1, H], f32, tag="sm1", space="PSUM")
            nc.tensor.matmul(a_ps[:], lhsT=adc[:], rhs=wad[:], start=True,
                             stop=True)
            bad = sp.tile([1, H], f32, tag="row1")
            nc.sync.dma_start(out=bad[:], in_=d_bad[:])
            a_row = sp.tile([1, H], f32, tag="arow")
            nc.vector.tensor_tensor(out=a_row[:], in0=a_ps[:], in1=bad[:],
                                    op=OP.add)
            nc.vector.tensor_scalar(out=a_row[:], in0=a_row[:], scalar1=0.0,
                                    scalar2=None, op0=OP.max)
            aT_ps = ps.tile([H, 1], f32, tag="sm1", space="PSUM")
            nc.tensor.transpose(aT_ps[:], in_=a_row[:], identity=ident[:1, :1])
            a_col = sp.tile([H, 1], f32, tag="acol")
            nc.scalar.activation(out=a_col[:], in_=aT_ps[:], func=AF.Copy)
            wq_t = sp.tile([H, H], f32, tag="w64b")
            nc.sync.dma_start(out=wq_t[:], in_=d_wq[:])
            q_ps = ps.tile([1, H], f32, tag="sm1", space="PSUM")
            nc.tensor.matmul(q_ps[:], lhsT=a_col[:], rhs=wq_t[:], start=True,
                             stop=True)
            bqr = sp.tile([1, H], f32, tag="row2")
            nc.sync.dma_start(out=bqr[:], in_=d_bq[:])
            q_row = sp.tile([1, H], f32, tag="qrow")
            nc.vector.tensor_tensor(out=q_row[:], in0=q_ps[:], in1=bqr[:],
                                    op=OP.add)
            qT_ps = ps.tile([H, 1], f32, tag="sm1", space="PSUM")
            nc.tensor.transpose(qT_ps[:], in_=q_row[:], identity=ident[:1, :1])
            qmask = sp.tile([H, HEADS], f32, tag="qm")
            nc.sync.dma_start(out=qmask[:], in_=d_qmask[:])
            q_col = sp.tile([H, 1], f32, tag="qcol")
            nc.scalar.activation(out=q_col[:], in_=qT_ps[:], func=AF.Copy)
            qblk = sp.tile([H, HEADS], bf16, tag="qblk")
            nc.vector.tensor_tensor(out=qblk[:],
                                    in0=q_col[:].to_broadcast([H, HEADS]),
                                    in1=qmask[:], op=OP.mult)
            wk16 = sp.tile([H, H], bf16, tag="w64c")
            wv16 = sp.tile([H, H], bf16, tag="w64d")
            nc.sync.dma_start(out=wk16[:], in_=d_wk16[:])
            nc.sync.dma_start(out=wv16[:], in_=d_wv16[:])

            # attention state (filled during the last layer's main stream)
            scores = pp.tile([HEADS, NPAD], bf16)    # exp(raw), unnormalized
            vall = pp.tile([128, NBUCK, H], bf16)
            sm = sp.tile([HEADS, n_ch], f32, tag="sm")

            # ---- message-passing layers, software-pipelined ----
            SL = [(0, 48), (48, 72), (72, NBUCK)]    # cc slices (pages)
            hs1pairs = []
            for _si, (p0, p1) in enumerate(SL):
                hs1pair_s = dp.tile([2 * 128 * (p1 - p0), H], bf16,
                                    name=f"hs1pair_{_si}")
                hs1pairs.append(hs1pair_s)
            hs2t = []
            for _li in range(L):
                hs2_l = pp.tile([128, NBUCK, H], bf16, name=f"hs2_{_li}")
                hs2t.append(hs2_l)

            gathers = [[] for _ in range(L)]         # per-layer gather instrs
            hs2_writes = [[] for _ in range(L)]
            ccs = [[] for _ in range(L)]
            expands = [[] for _ in range(L)]

            def hs2_batch(l, t0):
                tn = min(TB, NBUCK - t0)
                p2 = ps.tile([128, TB * H], f32, tag="tab8", space="PSUM")
                for j in range(tn):
                    nc.tensor.matmul(
                        p2[:, j * H:(j + 1) * H],
                        lhsT=hT[:H, (t0 + j) * 128:(t0 + j + 1) * 128],
                        rhs=msgw2[:, l, :], start=True, stop=True)
                nc.scalar.activation(
                    out=hs2t[l][:, t0:t0 + tn, :],
                    in_=p2[:, :tn * H].rearrange("p (t f) -> p t f", f=H),
                    func=AF.Copy)
                dst0 = tab[HS2OFF + t0 * 128:HS2OFF + (t0 + tn) * 128,
                           0:H].rearrange("(t p) f -> p t f", p=128)
                w1 = nc.sync.dma_start(out=dst0, in_=hs2t[l][:, t0:t0 + tn, :])
                if l > 0:
                    # WAR: previous layer's overflow-dst gathers read this
                    nogrp = len(_ovf_groups())
                    for gi in gathers[l - 1][:nogrp]:
                        add_dep_helper(w1.ins, gi.ins,
                                       reason="WAR: hs2 rewrite after gathers")
                hs2_writes[l].append(w1)

            def hs1_batch(l, t0):
                tn = min(TB, NBUCK - t0)
                p1 = ps.tile([128, TB * H], f32, tag="tab8", space="PSUM")
                for j in range(tn):
                    nc.tensor.matmul(
                        p1[:, j * H:(j + 1) * H],
                        lhsT=hT[:, (t0 + j) * 128:(t0 + j + 1) * 128],
                        rhs=msgw1[:, l, :], start=True, stop=True)
                st = mp.tile([128, TB, H], bf16, tag="st")
                nc.scalar.activation(
                    out=st[:, :tn, :],
                    in_=p1[:, :tn * H].rearrange("p (t f) -> p t f", f=H),
                    func=AF.Copy)
                nc.sync.dma_start(
                    out=hs1own[t0 * 128:(t0 + tn) * 128, :].rearrange(
                        "(t p) f -> p t f", p=128),
                    in_=st[:, :tn, :])

            def issue_cc(l, s):
                p0, p1 = SL[s]
                cc0 = nc.gpsimd.collective_compute(
                    "AllGather", mybir.AluOpType.bypass, replica_groups=PAIRS,
                    ins=[hs1own[p0 * 128:p1 * 128, :]],
                    outs=[hs1pairs[s].opt()])
                if l > 0:
                    for e in expands[l - 1]:
                        add_dep_helper(cc0.ins, e.ins,
                                       reason="WAR: pair rewrite after expand")
                ccs[l].append(cc0)

            def expand_tab(l):
                # copy exchanged hs1 slices into tab hs1 region (cols 0:64)
                for s, (p0, p1) in enumerate(SL):
                    n_s = 128 * (p1 - p0)
                    cc0 = ccs[l][s]
                    for hf in range(2):
                        base = hf * NPAD + p0 * 128
                        e = nc.sync.dma_start(
                            out=tab[base:base + n_s, 0:H].rearrange(
                                "(t p) f -> p t f", p=128),
                            in_=hs1pairs[s][hf * n_s:(hf + 1) * n_s,
                                            :].rearrange(
                                "(t p) f -> p t f", p=128))
                        add_dep_helper(e.ins, cc0.ins,
                                       reason="RAW: expand after allgather")
                        if l > 0:
                            for gi in gathers[l - 1]:
                                add_dep_helper(
                                    e.ins, gi.ins,
                                    reason="WAR: tab rewrite after gathers")
                        expands[l].append(e)

            def update_chunk(l, c):
                lo = c * CH
                sz = min(CH, NPAD - lo)
                u_ps = ps.tile([H, CH], f32, tag="big", space="PSUM")
                nc.tensor.matmul(u_ps[:, :sz], lhsT=updw1[:, l, :],
                                 rhs=hT[:H, lo:lo + sz], start=True, stop=False)
                nc.tensor.matmul(u_ps[:, :sz], lhsT=updw2[:, l, :],
                                 rhs=aggT[:, lo:lo + sz], start=False,
                                 stop=True)
                un = mp.tile([H, CH], bf16, tag="un")
                nc.scalar.activation(out=un[:, :sz], in_=u_ps[:, :sz],
                                     func=AF.Relu, bias=updb[:, l:l + 1])
                nc.vector.tensor_tensor(out=hT[:H, lo:lo + sz],
                                        in0=hT[:H, lo:lo + sz],
                                        in1=un[:, :sz], op=OP.add)

            def att_chunk(c):
                lo = c * CH
                sz = min(CH, NPAD - lo)
                nt = sz // 128
                kT_ps = ps.tile([H, CH], f32, tag="big", space="PSUM")
                nc.tensor.matmul(kT_ps[:, :sz], lhsT=wk16[:],
                                 rhs=hT[:H, lo:lo + sz], start=True, stop=True)
                kT_sb = mp.tile([H, CH], bf16, tag="kT")
                nc.scalar.activation(out=kT_sb[:, :sz], in_=kT_ps[:, :sz],
                                     func=AF.Copy)
                s_ps = ps.tile([HEADS, CH], f32, tag="sm1", space="PSUM")
                nc.tensor.matmul(s_ps[:, :sz], lhsT=qblk[:], rhs=kT_sb[:, :sz],
                                 start=True, stop=True)
                nc.scalar.activation(out=scores[:, lo:lo + sz],
                                     in_=s_ps[:, :sz], func=AF.Exp)
                if lo + sz > N_HALF:
                    nc.gpsimd.memset(scores[:, N_HALF:], 0.0)
                nc.vector.tensor_reduce(out=sm[:, c:c + 1],
                                        in_=scores[:, lo:lo + sz],
                                        axis=mybir.AxisListType.X, op=OP.add)
                v_ps = ps.tile([128, 4 * H], f32, tag="tab8", space="PSUM")
                for j in range(nt):
                    nc.tensor.matmul(
                        v_ps[:, j * H:(j + 1) * H],
                        lhsT=hT[:H, lo + j * 128:lo + (j + 1) * 128],
                        rhs=wv16[:], start=True, stop=True)
                nc.scalar.activation(
                    out=vall[:, lo // 128:lo // 128 + nt, :],
                    in_=v_ps[:, :nt * H].rearrange("p (t f) -> p t f", f=H),
                    func=AF.Copy)

            def post_update(l, c):
                if l + 1 < L:
                    if c >= 1:
                        hs2_batch(l + 1, 4 * (c - 1))
                        hs1_batch(l + 1, 4 * (c - 1))
                    if c == 13:
                        issue_cc(l + 1, 0)
                    elif c == 19:
                        issue_cc(l + 1, 1)
                else:
                    att_chunk(c)

            def gdeps(l, gi, writes, after=()):
                add_dep_helper(gi.ins, w_sent.ins, reason="RAW: sentinel")
                for w in writes:
                    add_dep_helper(gi.ins, w.ins, reason="RAW: tab write")
                for cx in after:
                    add_dep_helper(gi.ins, cx.ins, reason="RAW: tab ready")
                gathers[l].append(gi)

            def ovf_dst_stream(l):
                for (c0, og) in _ovf_groups():
                    goutO = mp.tile([128, OG, 128], bf16, tag="goutO")
                    nidx = og * 128
                    gi = nc.gpsimd.dma_gather(
                        out_ap=goutO[:, :og, :], in_ap=tab[:],
                        idxs_ap=cmbO[:, (2 * c0 + og) * 8:2 * (c0 + og) * 8],
                        num_idxs=nidx, num_idxs_reg=nidx, elem_size=128,
                        queue_num=0, single_packet=False)
                    gdeps(l, gi, hs2_writes[l])
                    nc.vector.tensor_copy(out=ovfmsg[:, c0:c0 + og, :],
                                          in_=goutO[:, 0:og, 0:H])

            def main_phase(l):
                # overflow src rows + messages
                for (c0, og) in _ovf_groups():
                    goutO = mp.tile([128, OG, 128], bf16, tag="goutO")
                    nidx = og * 128
                    gi = nc.gpsimd.dma_gather(
                        out_ap=goutO[:, :og, :], in_ap=tab[:],
                        idxs_ap=cmbO[:, 2 * c0 * 8:(2 * c0 + og) * 8],
                        num_idxs=nidx, num_idxs_reg=nidx, elem_size=128,
                        queue_num=0, single_packet=False)
                    gdeps(l, gi, expands[l])
                    nc.vector.tensor_tensor(
                        out=ovfmsg[:, c0:c0 + og, :],
                        in0=ovfmsg[:, c0:c0 + og, :],
                        in1=goutO[:, 0:og, 0:H], op=OP.add)
                    nc.scalar.activation(out=ovfmsg[:, c0:c0 + og, :],
                                         in_=ovfmsg[:, c0:c0 + og, :],
                                         func=AF.Relu)

                next_up = 0

                def issue_gather(gidx, b0, gp):
                    cols = gp * K
                    gout = g3.tile([128, PPG * K, 128], bf16, tag="gout")
                    gi = nc.gpsimd.dma_gather(
                        out_ap=gout[:, :cols, :], in_ap=tab[:],
                        idxs_ap=cmbM[:, b0 * K * 8:(b0 + gp) * K * 8],
                        num_idxs=cols * 128, num_idxs_reg=cols * 128,
                        elem_size=128, queue_num=gidx % 2,
                        single_packet=False)
                    gdeps(l, gi, (), after=expands[l])
                    return gout

                def consume_group(b0, gp, gout):
                    nonlocal next_up
                    msg = mp.tile([128, PPG * K, H], bf16, tag="msg")
                    agg = mp.tile([128, PPG, H], f32, tag="agg")
                    p_sc = ps.tile([H, PPG * 128], f32, tag="scat",
                                   space="PSUM")
                    ohpg = mp.tile([128, PPG * OVFCH, 128], bf16, tag="ohpg")
                    dl4 = dlocO[:, 2 * b0 * OVFCH:2 * (b0 + gp) * OVFCH
                                ].rearrange("p (g two) -> p g two", two=2)[
                        :, :, None, :].to_broadcast([128, gp * OVFCH, 64, 2])
                    io4 = iota[:].rearrange("p (s two) -> p s two", two=2)[
                        :, None, :, :].to_broadcast([128, gp * OVFCH, 64, 2])
                    oh4 = ohpg[:, :gp * OVFCH, :].rearrange(
                        "p g (s two) -> p g s two", two=2)
                    nc.vector.tensor_tensor(out=oh4, in0=dl4, in1=io4,
                                            op=OP.is_equal)
                    for j in range(gp):
                        nc.vector.tensor_tensor(
                            out=msg[:, j * K:(j + 1) * K, :],
                            in0=gout[:, j * K:(j + 1) * K, 0:H],
                            in1=hs2t[l][:, b0 + j:b0 + j + 1, :].to_broadcast(
                                [128, K, H]), op=OP.add)
                        nc.vector.tensor_scalar(
                            out=msg[:, j * K:(j + 1) * K, :],
                            in0=msg[:, j * K:(j + 1) * K, :],
                            scalar1=0.0, scalar2=None, op0=OP.max)
                    for j in range(gp):
                        pg = b0 + j
                        nc.vector.tensor_reduce(
                            out=agg[:, j, :],
                            in_=msg[:, j * K:(j + 1) * K, :].rearrange(
                                "p j f -> p f j"),
                            axis=mybir.AxisListType.X, op=OP.add)
                        sl = p_sc[:, j * 128:(j + 1) * 128]
                        nc.tensor.matmul(sl, lhsT=agg[:, j, :], rhs=ident[:],
                                         is_transpose=True, start=True,
                                         stop=False)
                        for k in range(OVFCH):
                            cc_i = pg * OVFCH + k
                            nc.tensor.matmul(
                                sl, lhsT=ovfmsg[:, cc_i, :],
                                rhs=ohpg[:, j * OVFCH + k, :],
                                start=False, stop=(k == OVFCH - 1))
                    nc.scalar.activation(
                        out=aggT[:, b0 * 128:(b0 + gp) * 128],
                        in_=p_sc[:, :gp * 128], func=AF.Copy)
                    while (next_up < n_ch
                           and (4 * next_up + 4) * 128 <= (b0 + gp) * 128):
                        update_chunk(l, next_up)
                        post_update(l, next_up)
                        next_up += 1

                AHEAD = 2
                pend = []
                for gidx, (b0, gp) in enumerate(_main_groups()):
                    pend.append((b0, gp, issue_gather(gidx, b0, gp)))
                    if len(pend) > AHEAD:
                        consume_group(*pend.pop(0))
                for item in pend:
                    consume_group(*item)
                while next_up < n_ch:
                    update_chunk(l, next_up)
                    post_update(l, next_up)
                    next_up += 1

            # layer 0 tables: h0 = encoder(x) is computable locally for
            # BOTH halves from the raw features -- no collective needed
            for t0 in range(0, NBUCK, TB):
                hs2_batch(0, t0)
            for gt0 in range(0, 2 * NBUCK, TB):
                tn = min(TB, 2 * NBUCK - gt0)
                hf_ps = ps.tile([H, TB * 128], f32, tag="big", space="PSUM")
                xtf = mp.tile([F_NODE, TB * 128], bf16, tag="xtf")
                nc.sync.dma_start(out=xtf[:, :tn * 128],
                                  in_=d_xTF[:, gt0 * 128:(gt0 + tn) * 128])
                nc.tensor.matmul(hf_ps[:, :tn * 128], lhsT=wnode[:],
                                 rhs=xtf[:, :tn * 128], start=True, stop=True)
                htmp = mp.tile([H + 1, TB * 128], bf16, tag="kT")
                nc.vector.tensor_scalar(out=htmp[:H, :tn * 128],
                                        in0=hf_ps[:, :tn * 128],
                                        scalar1=bnode[:], scalar2=0.0,
                                        op0=OP.add, op1=OP.max)
                nc.gpsimd.memset(htmp[H:H + 1, :tn * 128], 1.0)
                p1g = ps.tile([128, TB * H], f32, tag="tab8", space="PSUM")
                for j in range(tn):
                    nc.tensor.matmul(
                        p1g[:, j * H:(j + 1) * H],
                        lhsT=htmp[:, j * 128:(j + 1) * 128],
                        rhs=msgw1[:, 0, :], start=True, stop=True)
                stg = mp.tile([128, TB, H], bf16, tag="st")
                nc.scalar.activation(
                    out=stg[:, :tn, :],
                    in_=p1g[:, :tn * H].rearrange("p (t f) -> p t f", f=H),
                    func=AF.Copy)
                e0 = nc.sync.dma_start(
                    out=tab[gt0 * 128:(gt0 + tn) * 128, 0:H].rearrange(
                        "(t p) f -> p t f", p=128),
                    in_=stg[:, :tn, :])
                expands[0].append(e0)
            ovf_dst_stream(0)
            main_phase(0)

            # layer 1: tables/cc mostly issued inside layer 0's stream
            hs2_batch(1, 76)
            hs1_batch(1, 76)
            issue_cc(1, 2)
            expand_tab(1)
            ovf_dst_stream(1)
            main_phase(1)

            # ---- attention tail: sums, ctx, pair merge ----
            s_loc = sp.tile([HEADS, 1], f32, tag="m3")
            nc.vector.tensor_reduce(out=s_loc[:], in_=sm[:],
                                    axis=mybir.AxisListType.X, op=OP.add)
            ctx_ps = ps.tile([H, HEADS], f32, tag="tab8", space="PSUM")
            for i in range(0, NBUCK, 4):
                nt = min(4, NBUCK - i)
                at_ps = ps.tile([128, 4 * HEADS], bf16, tag="sm1",
                                space="PSUM")
                for j in range(nt):
                    nc.tensor.transpose(
                        at_ps[:, j * HEADS:(j + 1) * HEADS],
                        in_=scores[:, (i + j) * 128:(i + j + 1) * 128],
                        identity=ident_bf4[:])
                at_sb = mp.tile([128, 4 * HEADS], bf16, tag="atsb")
                nc.scalar.activation(out=at_sb[:, :nt * HEADS],
                                     in_=at_ps[:, :nt * HEADS], func=AF.Copy)
                for j in range(nt):
                    t = i + j
                    nc.tensor.matmul(
                        ctx_ps[:], lhsT=vall[:, t, :],
                        rhs=at_sb[:, j * HEADS:(j + 1) * HEADS],
                        start=(t == 0), stop=(t == NBUCK - 1))

            ctx_sb = sp.tile([H, HEADS], f32, tag="ctxsb")
            nc.scalar.activation(out=ctx_sb[:], in_=ctx_ps[:], func=AF.Copy)
            ctxT_ps = ps.tile([HEADS, H], f32, tag="sm1", space="PSUM")
            nc.tensor.transpose(ctxT_ps[:], in_=ctx_sb[:],
                                identity=ident[:H, :H])
            pay = sp.tile([HEADS, H + 2], f32, tag="pay")
            nc.scalar.activation(out=pay[:, 0:H], in_=ctxT_ps[:], func=AF.Copy)
            nc.vector.tensor_copy(out=pay[:, H:H + 1], in_=s_loc[:])
            nc.vector.tensor_copy(out=pay[:, H + 1:H + 2], in_=s_loc[:])
            w_pay = nc.sync.dma_start(out=pay_own[:], in_=pay[:])
            ccp = nc.gpsimd.collective_compute(
                "AllGather", mybir.AluOpType.bypass, replica_groups=PAIRS,
                ins=[pay_own.opt()], outs=[pay_full.opt()])

            p0 = sp.tile([HEADS, H + 2], f32, tag="p0")
            p1 = sp.tile([HEADS, H + 2], f32, tag="p1")
            nc.sync.dma_start(out=p0[:], in_=pay_full[0])
            nc.sync.dma_start(out=p1[:], in_=pay_full[1])
            den = sp.tile([HEADS, 1], f32, tag="den")
            nc.vector.tensor_tensor(out=den[:], in0=p0[:, H:H + 1],
                                    in1=p1[:, H:H + 1], op=OP.add)
            rden = sp.tile([HEADS, 1], f32, tag="rden")
            nc.vector.reciprocal(out=rden[:], in_=den[:])
            ctxc = sp.tile([HEADS, H], f32, tag="ctxc")
            nc.vector.tensor_tensor(out=ctxc[:], in0=p0[:, 0:H],
                                    in1=p1[:, 0:H], op=OP.add)
            nc.vector.tensor_scalar(out=ctxc[:], in0=ctxc[:], scalar1=rden[:],
                                    scalar2=None, op0=OP.mult)
            ctxT2 = ps.tile([H, HEADS], f32, tag="sm1", space="PSUM")
            nc.tensor.transpose(ctxT2[:], in_=ctxc[:],
                                identity=ident[:HEADS, :HEADS])
            cmask = sp.tile([H, HEADS], f32, tag="cm")
            nc.sync.dma_start(out=cmask[:], in_=d_cmask[:])
            ctx_m = sp.tile([H, HEADS], f32, tag="ctxm")
            nc.vector.tensor_tensor(out=ctx_m[:], in0=ctxT2[:], in1=cmask[:],
                                    op=OP.mult)
            ctx_c = sp.tile([H, 1], f32, tag="ctxco")
            nc.vector.tensor_reduce(out=ctx_c[:], in_=ctx_m[:],
                                    axis=mybir.AxisListType.X, op=OP.add)
            bvc = sp.tile([H, 1], f32, tag="bvc")
            nc.sync.dma_start(out=bvc[:], in_=d_bv[:])
            nc.vector.tensor_tensor(out=ctx_c[:], in0=ctx_c[:], in1=bvc[:],
                                    op=OP.add)

            # g = layer_norm(a + ctx @ wo + bo)
            wo_t = sp.tile([H, H], f32, tag="w64e")
            nc.sync.dma_start(out=wo_t[:], in_=d_wo[:])
            go_ps = ps.tile([1, H], f32, tag="sm1", space="PSUM")
            nc.tensor.matmul(go_ps[:], lhsT=ctx_c[:], rhs=wo_t[:], start=True,
                             stop=True)
            bor = sp.tile([1, H], f32, tag="bor")
            nc.sync.dma_start(out=bor[:], in_=d_bo[:])
            g_row = sp.tile([1, H], f32, tag="grow")
            nc.vector.tensor_tensor(out=g_row[:], in0=go_ps[:], in1=bor[:],
                                    op=OP.add)
            nc.vector.tensor_tensor(out=g_row[:], in0=g_row[:], in1=a_row[:],
                                    op=OP.add)
            mu = sp.tile([1, 1], f32, tag="mu")
            nc.vector.tensor_reduce(out=mu[:], in_=g_row[:],
                                    axis=mybir.AxisListType.X, op=OP.add)
            nc.vector.tensor_scalar(out=mu[:], in0=mu[:], scalar1=1.0 / H,
                                    scalar2=None, op0=OP.mult)
            nc.vector.tensor_scalar(out=g_row[:], in0=g_row[:], scalar1=mu[:],
                                    scalar2=None, op0=OP.subtract)
            sq = sp.tile([1, H], f32, tag="sq")
            nc.scalar.activation(out=sq[:], in_=g_row[:], func=AF.Square)
            var = sp.tile([1, 1], f32, tag="var")
            nc.vector.tensor_reduce(out=var[:], in_=sq[:],
                                    axis=mybir.AxisListType.X, op=OP.add)
            std = sp.tile([1, 1], f32, tag="std")
            eps_t = sp.tile([1, 1], f32, tag="eps")
            nc.gpsimd.memset(eps_t[:], 1e-5)
            nc.scalar.activation(out=std[:], in_=var[:], func=AF.Sqrt,
                                 scale=1.0 / H, bias=eps_t[:])
            rstd = sp.tile([1, 1], f32, tag="rstd")
            nc.vector.reciprocal(out=rstd[:], in_=std[:])
            nc.vector.tensor_scalar(out=g_row[:], in0=g_row[:], scalar1=rstd[:],
                                    scalar2=None, op0=OP.mult)
            lng = sp.tile([1, H], f32, tag="lng")
            lnb = sp.tile([1, H], f32, tag="lnb")
            nc.sync.dma_start(out=lng[:], in_=d_lng[:])
            nc.sync.dma_start(out=lnb[:], in_=d_lnb[:])
            nc.vector.tensor_tensor(out=g_row[:], in0=g_row[:], in1=lng[:],
                                    op=OP.mult)
            nc.vector.tensor_tensor(out=g_row[:], in0=g_row[:], in1=lnb[:],
                                    op=OP.add)

            # logits = hT^T @ (g/8 + policy_w), masked (own half)
            pol = sp.tile([1, H], f32, tag="pol")
            nc.sync.dma_start(out=pol[:], in_=d_pol[:])
            nc.vector.tensor_scalar(out=g_row[:], in0=g_row[:], scalar1=1.0 / 8.0,
                                    scalar2=None, op0=OP.mult)
            nc.vector.tensor_tensor(out=g_row[:], in0=g_row[:], in1=pol[:],
                                    op=OP.add)
            wT_ps = ps.tile([H, 1], f32, tag="sm1", space="PSUM")
            nc.tensor.transpose(wT_ps[:], in_=g_row[:], identity=ident[:1, :1])
            w_col = sp.tile([H, 1], bf16, tag="wcol")
            nc.scalar.activation(out=w_col[:], in_=wT_ps[:], func=AF.Copy)
            lg_ps = ps.tile([128, NBUCK], f32, tag="sm1", space="PSUM")
            for t in range(NBUCK):
                nc.tensor.matmul(lg_ps[:, t:t + 1],
                                 lhsT=hT[:H, t * 128:(t + 1) * 128],
                                 rhs=w_col[:], start=True, stop=True)
            maskf = sp.tile([128, NBUCK], f32, tag="mf")
            maskn = sp.tile([128, NBUCK], f32, tag="mn")
            nc.sync.dma_start(out=maskf[:], in_=d_maskf[:])
            nc.sync.dma_start(out=maskn[:], in_=d_maskn[:])
            lg = sp.tile([128, NBUCK], f32, tag="lgsb")
            nc.vector.tensor_tensor(out=lg[:], in0=lg_ps[:], in1=maskf[:],
                                    op=OP.mult)
            nc.vector.tensor_tensor(out=lg[:], in0=lg[:], in1=maskn[:],
                                    op=OP.add)
            nc.sync.dma_start(out=d_out[:], in_=lg[:])

    nc.compile()
    return nc


def _wrap16(a):
    w = a.reshape(-1, 16).T
    return np.tile(w, (8, 1)).astype(np.int16)


def _prep_core(inputs, s, half):
    gn = np.asarray(inputs["graph_nodes"])
    links = np.asarray(inputs["graph_edge_links"])
    mask = np.asarray(inputs["mask"])

    x = np.zeros((NPAD, F_NODE), np.float32)
    x[:N_HALF] = gn[s, half * N_HALF:(half + 1) * N_HALF]
    xT = np.ascontiguousarray(x.T).astype(ml_dtypes.bfloat16)
    xf = np.zeros((2, NPAD, F_NODE), np.float32)
    xf[0, :N_HALF] = gn[s, :N_HALF]
    xf[1, :N_HALF] = gn[s, N_HALF:]
    xTF = np.ascontiguousarray(
        xf.reshape(NTAB, F_NODE).T).astype(ml_dtypes.bfloat16)

    src = links[s, 0].astype(np.int64)
    dst = links[s, 1].astype(np.int64)
    sel = (dst >= half * N_HALF) & (dst < (half + 1) * N_HALF)
    src_e = src[sel]
    dst_e = dst[sel]
    dl = dst_e - half * N_HALF                    # local 0..N_HALF
    psrc = src_e + (NPAD - N_HALF) * (src_e >= N_HALF)   # row in [0, NTAB)

    order = np.argsort(dl, kind="stable")
    dls = dl[order]
    pss = psrc[order]
    counts = np.bincount(dls, minlength=N_HALF)
    starts = np.zeros(N_HALF, np.int64)
    starts[1:] = np.cumsum(counts)[:-1]
    rank = np.arange(len(dls)) - starts[dls]

    mainsel = rank < K
    mn, mr, mp_ = dls[mainsel], rank[mainsel], pss[mainsel]
    idxM = np.full(MCOLS * 128, SENT, np.int64)
    slot = ((mn >> 7) * K + mr) * 128 + (mn & 127)
    idxM[slot] = mp_

    on, op_ = dls[~mainsel], pss[~mainsel]
    ob = on >> 7
    ocounts = np.bincount(ob, minlength=NBUCK)
    if ocounts.max() > OVFCH * 128:
        raise RuntimeError(f"ovf overflow: {ocounts.max()} > {OVFCH * 128}")
    ostarts = np.zeros(NBUCK, np.int64)
    ostarts[1:] = np.cumsum(ocounts)[:-1]
    within = np.arange(len(on)) - ostarts[ob]
    oslot = ob * (OVFCH * 128) + within
    idxOs = np.full(OCOLS * 128, SENT, np.int64)
    idxOd = np.full(OCOLS * 128, SENT, np.int64)
    dlocv = np.full(OCOLS * 128, 128, np.float32)
    idxOs[oslot] = op_
    idxOd[oslot] = HS2OFF + on
    dlocv[oslot] = (on & 127)

    blocks = []
    for (c0, og) in _ovf_groups():
        blocks.append(_wrap16(idxOs[c0 * 128:(c0 + og) * 128]))
        blocks.append(_wrap16(idxOd[c0 * 128:(c0 + og) * 128]))
    cmbO = np.ascontiguousarray(np.concatenate(blocks, axis=1))
    cmbM = _wrap16(idxM)
    dl_cols = dlocv.reshape(OCOLS, 128).T
    dlocO = np.ascontiguousarray(
        np.repeat(dl_cols, 2, axis=1)).astype(ml_dtypes.bfloat16)

    m = np.zeros(NPAD, bool)
    m[:N_HALF] = mask[s, half * N_HALF:(half + 1) * N_HALF]
    maskf = np.where(m, np.float32(1.0), np.float32(0.0))
    pb = np.float32(np.asarray(inputs["policy_b"]))
    maskn = np.where(m, pb, NEG)
    maskf = np.ascontiguousarray(maskf.reshape(NBUCK, 128).T)
    maskn = np.ascontiguousarray(maskn.reshape(NBUCK, 128).T)

    return {
        "xT": xT, "xTF": xTF, "cmbM": cmbM, "cmbO": cmbO, "dlocO": dlocO,
        "maskf": maskf, "maskneg": maskn,
        "ad_col": np.asarray(inputs["current_ad"])[s].reshape(F_AD, 1)
                    .astype(np.float32),
    }


def kernel(**inputs):
    from concourse.bass_utils import run_bass_kernel_spmd

    if "nc" not in _CACHE:
        _CACHE["nc"] = _build()
    nc = _CACHE["nc"]

    f = lambda k: np.ascontiguousarray(np.asarray(inputs[k], np.float32))
    bf = lambda a: np.ascontiguousarray(a).astype(ml_dtypes.bfloat16)
    iot = np.tile(np.arange(128, dtype=np.float32), (128, 1))
    blockmask = np.zeros((H, HEADS), np.float32)
    for hh in range(HEADS):
        blockmask[hh * DH:(hh + 1) * DH, hh] = 1.0

    msg_w = f("msg_w")
    upd_w = f("upd_w")
    common = {
        "iota128": iot.astype(ml_dtypes.bfloat16),
        "w_node16": bf(f("w_node")),
        "b_node_col": f("b_node").reshape(H, 1),
        "msgw1": bf(np.concatenate(
            [msg_w[:, :H, :].transpose(1, 0, 2),
             f("msg_b").reshape(1, L, H)], axis=0)),
        "msgw2": bf(msg_w[:, H:, :].transpose(1, 0, 2)),
        "bias_rep": np.tile(f("msg_b").reshape(1, L, H), (128, 1, 1)),
        "updw1": bf(upd_w[:, :H, :].transpose(1, 0, 2)),
        "updw2": bf(upd_w[:, H:, :].transpose(1, 0, 2)),
        "upd_b_col": np.ascontiguousarray(f("upd_b").T),
        "w_ad": f("w_ad"), "b_ad_row": f("b_ad").reshape(1, H),
        "wq": f("wq"), "bq_row": f("bq").reshape(1, H),
        "wk16": bf(f("wk")), "wv16": bf(f("wv")),
        "bv_col": f("bv").reshape(H, 1),
        "wo": f("wo"), "bo_row": f("bo").reshape(1, H),
        "ln_g_row": f("ln_g").reshape(1, H), "ln_b_row": f("ln_b").reshape(1, H),
        "qmask": blockmask * np.float32(1.0 / np.sqrt(DH)),
        "cmask": blockmask,
        "pol_row": f("policy_w").reshape(1, H),
    }

    in_maps = []
    for c in range(NCORES):
        m = dict(common)
        m.update(_prep_core(inputs, c // 2, c % 2))
        in_maps.append(m)

    res = run_bass_kernel_spmd(nc, in_maps, core_ids=list(range(NCORES)))
    _CACHE["last_results"] = res

    out = np.empty((B, N), np.float32)
    for c in range(NCORES):
        s, half = c // 2, c % 2
        lg = np.asarray(res.results[c]["logits"])      # [128, NBUCK]
        flat = lg.T.reshape(NPAD)
        out[s, half * N_HALF:(half + 1) * N_HALF] = flat[:N_HALF]
    return out



# revision 6
# speedup vs baseline: 1.0042x; 1.0042x over previous
"""Trainium2 Bass kernel for BillboardAllocatorGNN.

Sharding: 8 cores; core c handles sample c//2, node-half c%2 (data parallel
over batch, dst-parallel within each sample pair).

Edge phase (per layer): node-major edge slots with fixed per-node capacity
K=18 kill both the dst-side gather and the one-hot scatter for 96% of edges:
a single SWDGE gather fetches duplicated-bf16 hs1 rows (256B descriptors)
into [dst-node-partition, slot] layout, the dst-side hs2 contribution is a
free-dim broadcast add from SBUF, and segment-sum is a strided free-axis
tensor_reduce. Overflow edges (deg>K) go through a small one-hot matmul
side path whose PSUM accumulation group also hosts the main agg transpose.
Pad slots gather a -30000 sentinel row so relu zeroes them.

Pipelining: updates, next-layer table builds, and attention score/value
chunks are interleaved into the main gather stream. Layer 0 builds its
table locally from raw features (no collective); layer 1 exchanges hs1
in slices issued as updates complete; the final attention merges across
the pair via a 2KB unnormalized-softmax stats exchange (scores are O(1),
so no max subtraction is needed).
"""
import sys
import os

sys.path.insert(0, "/opt/trn_rl_repo")

import numpy as np
import ml_dtypes

# ---- problem dims (hardcoded per spec) ----
B, N, E = 4, 20000, 320000
F_NODE, F_AD = 16, 8
H, L, HEADS = 64, 2, 4
DH = H // HEADS

NCORES = 8
N_HALF = N // 2                 # 10000 real nodes per core
NBUCK = 79                      # 128-node pages per core
NPAD = NBUCK * 128              # 10112 padded nodes per core
K = 18                          # main slots per node
MCOLS = NBUCK * K               # 1422 main gather columns
PPG = 2                         # pages per main gather group
NGRP = (NBUCK + PPG - 1) // PPG  # 27
OVFCH = 2                       # overflow chunks per page (cap 256 edges)
OCOLS = NBUCK * OVFCH           # 158 overflow columns
OG = 12                         # overflow chunks per gather group
NTAB = 2 * NPAD                 # hs1 rows (both halves)
HS2OFF = NTAB                   # hs2 rows start
SENT = NTAB + NPAD              # sentinel row id
TROWS = SENT + 1
CH = 512                        # node-chunk for encoder/update/attention
NEG = np.float32(-1e9)

_CACHE = {}


def _ovf_groups():
    out = []
    c = 0
    while c < OCOLS:
        g = min(OG, OCOLS - c)
        out.append((c, g))
        c += g
    return out


def _main_groups():
    out = []
    b = 0
    while b < NBUCK:
        g = min(PPG, NBUCK - b)
        out.append((b, g))
        b += g
    return out


def _build():
    import concourse.mybir as mybir
    import concourse.tile as tile
    import concourse.bacc as bacc
    from concourse.tile import add_dep_helper
    from concourse.masks import make_identity

    f32 = mybir.dt.float32
    bf16 = mybir.dt.bfloat16
    i16 = mybir.dt.int16
    AF = mybir.ActivationFunctionType
    OP = mybir.AluOpType

    nc = bacc.Bacc("TRN2", target_bir_lowering=False, debug=False,
                   num_swdge_queues=2)

    # ---- I/O ----
    d_xT = nc.dram_tensor("xT", [F_NODE, NPAD], bf16, kind="ExternalInput")
    d_xTF = nc.dram_tensor("xTF", [F_NODE, NTAB], bf16, kind="ExternalInput")
    d_cmbM = nc.dram_tensor("cmbM", [128, MCOLS * 8], i16, kind="ExternalInput")
    d_cmbO = nc.dram_tensor("cmbO", [128, 2 * OCOLS * 8], i16,
                            kind="ExternalInput")
    d_dlocO = nc.dram_tensor("dlocO", [128, 2 * OCOLS], bf16,
                             kind="ExternalInput")
    d_iota = nc.dram_tensor("iota128", [128, 128], bf16, kind="ExternalInput")
    d_maskf = nc.dram_tensor("maskf", [128, NBUCK], f32, kind="ExternalInput")
    d_maskn = nc.dram_tensor("maskneg", [128, NBUCK], f32, kind="ExternalInput")
    d_wnode = nc.dram_tensor("w_node16", [F_NODE, H], bf16,
                             kind="ExternalInput")
    d_bnode = nc.dram_tensor("b_node_col", [H, 1], f32, kind="ExternalInput")
    d_msgw1 = nc.dram_tensor("msgw1", [H + 1, L, H], bf16,
                             kind="ExternalInput")
    d_msgw2 = nc.dram_tensor("msgw2", [H, L, H], bf16, kind="ExternalInput")
    d_brep = nc.dram_tensor("bias_rep", [128, L, H], f32, kind="ExternalInput")
    d_updw1 = nc.dram_tensor("updw1", [H, L, H], bf16, kind="ExternalInput")
    d_updw2 = nc.dram_tensor("updw2", [H, L, H], bf16, kind="ExternalInput")
    d_updb = nc.dram_tensor("upd_b_col", [H, L], f32, kind="ExternalInput")
    d_wad = nc.dram_tensor("w_ad", [F_AD, H], f32, kind="ExternalInput")
    d_bad = nc.dram_tensor("b_ad_row", [1, H], f32, kind="ExternalInput")
    d_ad = nc.dram_tensor("ad_col", [F_AD, 1], f32, kind="ExternalInput")
    d_wq = nc.dram_tensor("wq", [H, H], f32, kind="ExternalInput")
    d_bq = nc.dram_tensor("bq_row", [1, H], f32, kind="ExternalInput")
    d_wk16 = nc.dram_tensor("wk16", [H, H], bf16, kind="ExternalInput")
    d_wv16 = nc.dram_tensor("wv16", [H, H], bf16, kind="ExternalInput")
    d_bv = nc.dram_tensor("bv_col", [H, 1], f32, kind="ExternalInput")
    d_wo = nc.dram_tensor("wo", [H, H], f32, kind="ExternalInput")
    d_bo = nc.dram_tensor("bo_row", [1, H], f32, kind="ExternalInput")
    d_lng = nc.dram_tensor("ln_g_row", [1, H], f32, kind="ExternalInput")
    d_lnb = nc.dram_tensor("ln_b_row", [1, H], f32, kind="ExternalInput")
    d_qmask = nc.dram_tensor("qmask", [H, HEADS], f32, kind="ExternalInput")
    d_cmask = nc.dram_tensor("cmask", [H, HEADS], f32, kind="ExternalInput")
    d_pol = nc.dram_tensor("pol_row", [1, H], f32, kind="ExternalInput")
    d_out = nc.dram_tensor("logits", [128, NBUCK], f32, kind="ExternalOutput")

    PAIRS = [[0, 1], [2, 3], [4, 5], [6, 7]]
    n_ch = (NPAD + CH - 1) // CH
    TB = 4                      # node pages per table-export batch

    with tile.TileContext(nc) as tc:
        with (
            tc.tile_pool(name="persist", bufs=1) as pp,
            tc.tile_pool(name="mp", bufs=2) as mp,
            tc.tile_pool(name="g3", bufs=5) as g3,
            tc.tile_pool(name="single", bufs=1) as sp,
            tc.tile_pool(name="psum", bufs=2, space="PSUM") as ps,
            tc.tile_pool(name="dram", bufs=1, space="DRAM") as dp,
        ):
            # ---- persistent state / constants ----
            hT = pp.tile([H + 1, NPAD], bf16)      # node states + ones row
            aggT = pp.tile([H, NPAD], bf16)        # per-layer aggregate
            hs2 = pp.tile([128, NBUCK, H], bf16)   # dst table, node-major
            ovfmsg = pp.tile([128, OCOLS, H], bf16)
            cmbO = pp.tile([128, 2 * OCOLS * 8], i16)
            dlocO = pp.tile([128, 2 * OCOLS], bf16)
            iota = pp.tile([128, 128], bf16)
            ident = pp.tile([128, 128], f32)
            wnode = pp.tile([F_NODE, H], bf16)
            bnode = pp.tile([H, 1], f32)
            msgw1 = pp.tile([H + 1, L, H], bf16)
            msgw2 = pp.tile([H, L, H], bf16)
            updw1 = pp.tile([H, L, H], bf16)
            updw2 = pp.tile([H, L, H], bf16)
            updb = pp.tile([H, L], f32)

            make_identity(nc, ident[:])
            nc.gpsimd.memset(hT[H:H + 1, :], 1.0)
            ident_bf4 = pp.tile([HEADS, HEADS], bf16)
            nc.vector.tensor_copy(out=ident_bf4[:], in_=ident[:HEADS, :HEADS])

            for dst_t, src_t in [
                (cmbO, d_cmbO), (dlocO, d_dlocO),
                (iota, d_iota), (wnode, d_wnode), (bnode, d_bnode),
                (msgw1, d_msgw1), (msgw2, d_msgw2),
                (updw1, d_updw1), (updw2, d_updw2), (updb, d_updb),
            ]:
                nc.sync.dma_start(out=dst_t[:], in_=src_t[:])

            # ---- DRAM: gather table, hs1 exchange, attention payload ----
            tab = dp.tile([TROWS, 128], bf16)
            hs1own = dp.tile([NPAD, H], bf16)
            hs1pair = dp.tile([NTAB, H], bf16)
            pay_own = dp.tile([HEADS, H + 2], f32)
            pay_full = dp.tile([2, HEADS, H + 2], f32)

            # sentinel row (once; hs1/hs2 rewrites never touch it)
            sentc = sp.tile([1, 128], bf16, tag="sent")
            nc.gpsimd.memset(sentc[:], -30000.0)
            w_sent = nc.sync.dma_start(out=tab[SENT:SENT + 1, :], in_=sentc[:])

            # ---- node encoder: hT = relu(w_node^T @ xT + b) ----
            for i in range(n_ch):
                lo = i * CH
                sz = min(CH, NPAD - lo)
                xt = mp.tile([F_NODE, CH], bf16, tag="xtf")
                nc.sync.dma_start(out=xt[:, :sz], in_=d_xT[:, lo:lo + sz])
                h_ps = ps.tile([H, CH], f32, tag="big", space="PSUM")
                nc.tensor.matmul(h_ps[:, :sz], lhsT=wnode[:], rhs=xt[:, :sz],
                                 start=True, stop=True)
                nc.vector.tensor_scalar(out=hT[:H, lo:lo + sz],
                                        in0=h_ps[:, :sz], scalar1=bnode[:],
                                        scalar2=0.0, op0=OP.add, op1=OP.max)

            # ---- attention prelude: q from ad (independent of layers) ----
            wad = sp.tile([F_AD, H], f32, tag="w64")
            adc = sp.tile([F_AD, 1], f32, tag="col")
            nc.sync.dma_start(out=wad[:], in_=d_wad[:])
            nc.sync.dma_start(out=adc[:], in_=d_ad[:])
            a_ps = ps.tile([1, H], f32, tag="sm1", space="PSUM")
            nc.tensor.matmul(a_ps[:], lhsT=adc[:], rhs=wad[:], start=True,
                             stop=True)
            bad = sp.tile([1, H], f32, tag="row1")
            nc.sync.dma_start(out=bad[:], in_=d_bad[:])
            a_row = sp.tile([1, H], f32, tag="arow")
            nc.vector.tensor_tensor(out=a_row[:], in0=a_ps[:], in1=bad[:],
                                    op=OP.add)
            nc.vector.tensor_scalar(out=a_row[:], in0=a_row[:], scalar1=0.0,
                                    scalar2=None, op0=OP.max)
            aT_ps = ps.tile([H, 1], f32, tag="sm1", space="PSUM")
            nc.tensor.transpose(aT_ps[:], in_=a_row[:], identity=ident[:1, :1])
            a_col = sp.tile([H, 1], f32, tag="acol")
            nc.scalar.activation(out=a_col[:], in_=aT_ps[:], func=AF.Copy)
            wq_t = sp.tile([H, H], f32, tag="w64b")
            nc.sync.dma_start(out=wq_t[:], in_=d_wq[:])
            q_ps = ps.tile([1, H], f32, tag="sm1", space="PSUM")
            nc.tensor.matmul(q_ps[:], lhsT=a_col[:], rhs=wq_t[:], start=True,
                             stop=True)
            bqr = sp.tile([1, H], f32, tag="row2")
            nc.sync.dma_start(out=bqr[:], in_=d_bq[:])
            q_row = sp.tile([1, H], f32, tag="qrow")
            nc.vector.tensor_tensor(out=q_row[:], in0=q_ps[:], in1=bqr[:],
                                    op=OP.add)
            qT_ps = ps.tile([H, 1], f32, tag="sm1", space="PSUM")
            nc.tensor.transpose(qT_ps[:], in_=q_row[:], identity=ident[:1, :1])
            qmask = sp.tile([H, HEADS], f32, tag="qm")
            nc.sync.dma_start(out=qmask[:], in_=d_qmask[:])
            q_col = sp.tile([H, 1], f32, tag="qcol")
            nc.scalar.activation(out=q_col[:], in_=qT_ps[:], func=AF.Copy)
            qblk = sp.tile([H, HEADS], bf16, tag="qblk")
            nc.vector.tensor_tensor(out=qblk[:],
                                    in0=q_col[:].to_broadcast([H, HEADS]),
                                    in1=qmask[:], op=OP.mult)
            wk16 = sp.tile([H, H], bf16, tag="w64c")
            wv16 = sp.tile([H, H], bf16, tag="w64d")
            nc.sync.dma_start(out=wk16[:], in_=d_wk16[:])
            nc.sync.dma_start(out=wv16[:], in_=d_wv16[:])

            # attention state (filled during the last layer's main stream)
            scores = pp.tile([HEADS, NPAD], bf16)    # exp(raw), unnormalized
            vall = pp.tile([128, NBUCK, H], bf16)
            sm = sp.tile([HEADS, n_ch], f32, tag="sm")

            # ---- message-passing layers, software-pipelined ----
            SL = [(0, 48), (48, 72), (72, NBUCK)]    # cc slices (pages)
            hs1pairs = []
            for _si, (p0, p1) in enumerate(SL):
                hs1pair_s = dp.tile([2 * 128 * (p1 - p0), H], bf16,
                                    name=f"hs1pair_{_si}")
                hs1pairs.append(hs1pair_s)
            hs2t = []
            for _li in range(L):
                hs2_l = pp.tile([128, NBUCK, H], bf16, name=f"hs2_{_li}")
                hs2t.append(hs2_l)

            gathers = [[] for _ in range(L)]         # per-layer gather instrs
            hs2_writes = [[] for _ in range(L)]
            ccs = [[] for _ in range(L)]
            expands = [[] for _ in range(L)]

            def hs2_batch(l, t0):
                tn = min(TB, NBUCK - t0)
                p2 = ps.tile([128, TB * H], f32, tag="tab8", space="PSUM")
                for j in range(tn):
                    nc.tensor.matmul(
                        p2[:, j * H:(j + 1) * H],
                        lhsT=hT[:H, (t0 + j) * 128:(t0 + j + 1) * 128],
                        rhs=msgw2[:, l, :], start=True, stop=True)
                nc.scalar.activation(
                    out=hs2t[l][:, t0:t0 + tn, :],
                    in_=p2[:, :tn * H].rearrange("p (t f) -> p t f", f=H),
                    func=AF.Copy)
                dst0 = tab[HS2OFF + t0 * 128:HS2OFF + (t0 + tn) * 128,
                           0:H].rearrange("(t p) f -> p t f", p=128)
                w1 = nc.sync.dma_start(out=dst0, in_=hs2t[l][:, t0:t0 + tn, :])
                if l > 0:
                    # WAR: previous layer's overflow-dst gathers read this
                    nogrp = len(_ovf_groups())
                    for gi in gathers[l - 1][:nogrp]:
                        add_dep_helper(w1.ins, gi.ins,
                                       reason="WAR: hs2 rewrite after gathers")
                hs2_writes[l].append(w1)

            def hs1_batch(l, t0):
                tn = min(TB, NBUCK - t0)
                p1 = ps.tile([128, TB * H], f32, tag="tab8", space="PSUM")
                for j in range(tn):
                    nc.tensor.matmul(
                        p1[:, j * H:(j + 1) * H],
                        lhsT=hT[:, (t0 + j) * 128:(t0 + j + 1) * 128],
                        rhs=msgw1[:, l, :], start=True, stop=True)
                st = mp.tile([128, TB, H], bf16, tag="st")
                nc.scalar.activation(
                    out=st[:, :tn, :],
                    in_=p1[:, :tn * H].rearrange("p (t f) -> p t f", f=H),
                    func=AF.Copy)
                nc.sync.dma_start(
                    out=hs1own[t0 * 128:(t0 + tn) * 128, :].rearrange(
                        "(t p) f -> p t f", p=128),
                    in_=st[:, :tn, :])

            def issue_cc(l, s):
                p0, p1 = SL[s]
                cc0 = nc.gpsimd.collective_compute(
                    "AllGather", mybir.AluOpType.bypass, replica_groups=PAIRS,
                    ins=[hs1own[p0 * 128:p1 * 128, :]],
                    outs=[hs1pairs[s].opt()])
                if l > 0:
                    for e in expands[l - 1]:
                        add_dep_helper(cc0.ins, e.ins,
                                       reason="WAR: pair rewrite after expand")
                ccs[l].append(cc0)

            def expand_tab(l):
                # copy exchanged hs1 slices into tab hs1 region (cols 0:64)
                for s, (p0, p1) in enumerate(SL):
                    n_s = 128 * (p1 - p0)
                    cc0 = ccs[l][s]
                    for hf in range(2):
                        base = hf * NPAD + p0 * 128
                        e = nc.sync.dma_start(
                            out=tab[base:base + n_s, 0:H].rearrange(
                                "(t p) f -> p t f", p=128),
                            in_=hs1pairs[s][hf * n_s:(hf + 1) * n_s,
                                            :].rearrange(
                                "(t p) f -> p t f", p=128))
                        add_dep_helper(e.ins, cc0.ins,
                                       reason="RAW: expand after allgather")
                        if l > 0:
                            for gi in gathers[l - 1]:
                                add_dep_helper(
                                    e.ins, gi.ins,
                                    reason="WAR: tab rewrite after gathers")
                        expands[l].append(e)

            def update_chunk(l, c):
                lo = c * CH
                sz = min(CH, NPAD - lo)
                u_ps = ps.tile([H, CH], f32, tag="big", space="PSUM")
                nc.tensor.matmul(u_ps[:, :sz], lhsT=updw1[:, l, :],
                                 rhs=hT[:H, lo:lo + sz], start=True, stop=False)
                nc.tensor.matmul(u_ps[:, :sz], lhsT=updw2[:, l, :],
                                 rhs=aggT[:, lo:lo + sz], start=False,
                                 stop=True)
                un = mp.tile([H, CH], bf16, tag="un")
                nc.scalar.activation(out=un[:, :sz], in_=u_ps[:, :sz],
                                     func=AF.Relu, bias=updb[:, l:l + 1])
                nc.vector.tensor_tensor(out=hT[:H, lo:lo + sz],
                                        in0=hT[:H, lo:lo + sz],
                                        in1=un[:, :sz], op=OP.add)

            def att_chunk(c):
                lo = c * CH
                sz = min(CH, NPAD - lo)
                nt = sz // 128
                kT_ps = ps.tile([H, CH], f32, tag="big", space="PSUM")
                nc.tensor.matmul(kT_ps[:, :sz], lhsT=wk16[:],
                                 rhs=hT[:H, lo:lo + sz], start=True, stop=True)
                kT_sb = mp.tile([H, CH], bf16, tag="kT")
                nc.scalar.activation(out=kT_sb[:, :sz], in_=kT_ps[:, :sz],
                                     func=AF.Copy)
                s_ps = ps.tile([HEADS, CH], f32, tag="sm1", space="PSUM")
                nc.tensor.matmul(s_ps[:, :sz], lhsT=qblk[:], rhs=kT_sb[:, :sz],
                                 start=True, stop=True)
                nc.scalar.activation(out=scores[:, lo:lo + sz],
                                     in_=s_ps[:, :sz], func=AF.Exp)
                if lo + sz > N_HALF:
                    nc.gpsimd.memset(scores[:, N_HALF:], 0.0)
                nc.vector.tensor_reduce(out=sm[:, c:c + 1],
                                        in_=scores[:, lo:lo + sz],
                                        axis=mybir.AxisListType.X, op=OP.add)
                v_ps = ps.tile([128, 4 * H], f32, tag="tab8", space="PSUM")
                for j in range(nt):
                    nc.tensor.matmul(
                        v_ps[:, j * H:(j + 1) * H],
                        lhsT=hT[:H, lo + j * 128:lo + (j + 1) * 128],
                        rhs=wv16[:], start=True, stop=True)
                nc.scalar.activation(
                    out=vall[:, lo // 128:lo // 128 + nt, :],
                    in_=v_ps[:, :nt * H].rearrange("p (t f) -> p t f", f=H),
                    func=AF.Copy)

            def post_update(l, c):
                if l + 1 < L:
                    if c >= 1:
                        hs2_batch(l + 1, 4 * (c - 1))
                        hs1_batch(l + 1, 4 * (c - 1))
                    if c == 13:
                        issue_cc(l + 1, 0)
                    elif c == 19:
                        issue_cc(l + 1, 1)
                else:
                    att_chunk(c)

            def gdeps(l, gi, writes, after=()):
                add_dep_helper(gi.ins, w_sent.ins, reason="RAW: sentinel")
                for w in writes:
                    add_dep_helper(gi.ins, w.ins, reason="RAW: tab write")
                for cx in after:
                    add_dep_helper(gi.ins, cx.ins, reason="RAW: tab ready")
                gathers[l].append(gi)

            def ovf_dst_stream(l):
                for (c0, og) in _ovf_groups():
                    goutO = mp.tile([128, OG, 128], bf16, tag="goutO")
                    nidx = og * 128
                    gi = nc.gpsimd.dma_gather(
                        out_ap=goutO[:, :og, :], in_ap=tab[:],
                        idxs_ap=cmbO[:, (2 * c0 + og) * 8:2 * (c0 + og) * 8],
                        num_idxs=nidx, num_idxs_reg=nidx, elem_size=128,
                        queue_num=0, single_packet=False)
                    gdeps(l, gi, hs2_writes[l])
                    nc.vector.tensor_copy(out=ovfmsg[:, c0:c0 + og, :],
                                          in_=goutO[:, 0:og, 0:H])

            def main_phase(l):
                # overflow src rows + messages
                for (c0, og) in _ovf_groups():
                    goutO = mp.tile([128, OG, 128], bf16, tag="goutO")
                    nidx = og * 128
                    gi = nc.gpsimd.dma_gather(
                        out_ap=goutO[:, :og, :], in_ap=tab[:],
                        idxs_ap=cmbO[:, 2 * c0 * 8:(2 * c0 + og) * 8],
                        num_idxs=nidx, num_idxs_reg=nidx, elem_size=128,
                        queue_num=0, single_packet=False)
                    gdeps(l, gi, expands[l])
                    nc.vector.tensor_tensor(
                        out=ovfmsg[:, c0:c0 + og, :],
                        in0=ovfmsg[:, c0:c0 + og, :],
                        in1=goutO[:, 0:og, 0:H], op=OP.add)
                    nc.scalar.activation(out=ovfmsg[:, c0:c0 + og, :],
                                         in_=ovfmsg[:, c0:c0 + og, :],
                                         func=AF.Relu)

                next_up = 0

                def issue_gather(gidx, b0, gp):
                    cols = gp * K
                    gout = g3.tile([128, PPG * K, 128], bf16, tag="gout")
                    idxs = g3.tile([128, PPG * K * 8], i16, tag="gidx")
                    nc.sync.dma_start(
                        out=idxs[:, :cols * 8],
                        in_=d_cmbM[:, b0 * K * 8:(b0 + gp) * K * 8])
                    gi = nc.gpsimd.dma_gather(
                        out_ap=gout[:, :cols, :], in_ap=tab[:],
                        idxs_ap=idxs[:, :cols * 8],
                        num_idxs=cols * 128, num_idxs_reg=cols * 128,
                        elem_size=128, queue_num=gidx % 2,
                        single_packet=False)
                    gdeps(l, gi, (), after=expands[l])
                    return gout

                def consume_group(b0, gp, gout):
                    nonlocal next_up
                    msg = mp.tile([128, PPG * K, H], bf16, tag="msg")
                    agg = mp.tile([128, PPG, H], f32, tag="agg")
                    p_sc = ps.tile([H, PPG * 128], f32, tag="scat",
                                   space="PSUM")
                    ohpg = mp.tile([128, PPG * OVFCH, 128], bf16, tag="ohpg")
                    dl4 = dlocO[:, 2 * b0 * OVFCH:2 * (b0 + gp) * OVFCH
                                ].rearrange("p (g two) -> p g two", two=2)[
                        :, :, None, :].to_broadcast([128, gp * OVFCH, 64, 2])
                    io4 = iota[:].rearrange("p (s two) -> p s two", two=2)[
                        :, None, :, :].to_broadcast([128, gp * OVFCH, 64, 2])
                    oh4 = ohpg[:, :gp * OVFCH, :].rearrange(
                        "p g (s two) -> p g s two", two=2)
                    nc.vector.tensor_tensor(out=oh4, in0=dl4, in1=io4,
                                            op=OP.is_equal)
                    for j in range(gp):
                        nc.vector.tensor_tensor(
                            out=msg[:, j * K:(j + 1) * K, :],
                            in0=gout[:, j * K:(j + 1) * K, 0:H],
                            in1=hs2t[l][:, b0 + j:b0 + j + 1, :].to_broadcast(
                                [128, K, H]), op=OP.add)
                        nc.vector.tensor_scalar(
                            out=msg[:, j * K:(j + 1) * K, :],
                            in0=msg[:, j * K:(j + 1) * K, :],
                            scalar1=0.0, scalar2=None, op0=OP.max)
                    for j in range(gp):
                        pg = b0 + j
                        nc.vector.tensor_reduce(
                            out=agg[:, j, :],
                            in_=msg[:, j * K:(j + 1) * K, :].rearrange(
                                "p j f -> p f j"),
                            axis=mybir.AxisListType.X, op=OP.add)
                        sl = p_sc[:, j * 128:(j + 1) * 128]
                        nc.tensor.matmul(sl, lhsT=agg[:, j, :], rhs=ident[:],
                                         is_transpose=True, start=True,
                                         stop=False)
                        for k in range(OVFCH):
                            cc_i = pg * OVFCH + k
                            nc.tensor.matmul(
                                sl, lhsT=ovfmsg[:, cc_i, :],
                                rhs=ohpg[:, j * OVFCH + k, :],
                                start=False, stop=(k == OVFCH - 1))
                    nc.scalar.activation(
                        out=aggT[:, b0 * 128:(b0 + gp) * 128],
                        in_=p_sc[:, :gp * 128], func=AF.Copy)
                    while (next_up < n_ch
                           and (4 * next_up + 4) * 128 <= (b0 + gp) * 128):
                        update_chunk(l, next_up)
                        post_update(l, next_up)
                        next_up += 1

                AHEAD = 4
                pend = []
                for gidx, (b0, gp) in enumerate(_main_groups()):
                    pend.append((b0, gp, issue_gather(gidx, b0, gp)))
                    if len(pend) > AHEAD:
                        consume_group(*pend.pop(0))
                for item in pend:
                    consume_group(*item)
                while next_up < n_ch:
                    update_chunk(l, next_up)
                    post_update(l, next_up)
                    next_up += 1

            # layer 0 tables: h0 = encoder(x) is computable locally for
            # BOTH halves from the raw features -- no collective needed
            for t0 in range(0, NBUCK, TB):
                hs2_batch(0, t0)
            for gt0 in range(0, 2 * NBUCK, TB):
                tn = min(TB, 2 * NBUCK - gt0)
                hf_ps = ps.tile([H, TB * 128], f32, tag="big", space="PSUM")
                xtf = mp.tile([F_NODE, TB * 128], bf16, tag="xtf")
                nc.sync.dma_start(out=xtf[:, :tn * 128],
                                  in_=d_xTF[:, gt0 * 128:(gt0 + tn) * 128])
                nc.tensor.matmul(hf_ps[:, :tn * 128], lhsT=wnode[:],
                                 rhs=xtf[:, :tn * 128], start=True, stop=True)
                htmp = mp.tile([H + 1, TB * 128], bf16, tag="kT")
                nc.vector.tensor_scalar(out=htmp[:H, :tn * 128],
                                        in0=hf_ps[:, :tn * 128],
                                        scalar1=bnode[:], scalar2=0.0,
                                        op0=OP.add, op1=OP.max)
                nc.gpsimd.memset(htmp[H:H + 1, :tn * 128], 1.0)
                p1g = ps.tile([128, TB * H], f32, tag="tab8", space="PSUM")
                for j in range(tn):
                    nc.tensor.matmul(
                        p1g[:, j * H:(j + 1) * H],
                        lhsT=htmp[:, j * 128:(j + 1) * 128],
                        rhs=msgw1[:, 0, :], start=True, stop=True)
                stg = mp.tile([128, TB, H], bf16, tag="st")
                nc.scalar.activation(
                    out=stg[:, :tn, :],
                    in_=p1g[:, :tn * H].rearrange("p (t f) -> p t f", f=H),
                    func=AF.Copy)
                e0 = nc.sync.dma_start(
                    out=tab[gt0 * 128:(gt0 + tn) * 128, 0:H].rearrange(
                        "(t p) f -> p t f", p=128),
                    in_=stg[:, :tn, :])
                expands[0].append(e0)
            ovf_dst_stream(0)
            main_phase(0)

            # layer 1: tables/cc mostly issued inside layer 0's stream
            hs2_batch(1, 76)
            hs1_batch(1, 76)
            issue_cc(1, 2)
            expand_tab(1)
            ovf_dst_stream(1)
            main_phase(1)

            # ---- attention tail: sums, ctx, pair merge ----
            s_loc = sp.tile([HEADS, 1], f32, tag="m3")
            nc.vector.tensor_reduce(out=s_loc[:], in_=sm[:],
                                    axis=mybir.AxisListType.X, op=OP.add)
            ctx_ps = ps.tile([H, HEADS], f32, tag="tab8", space="PSUM")
            for i in range(0, NBUCK, 4):
                nt = min(4, NBUCK - i)
                at_ps = ps.tile([128, 4 * HEADS], bf16, tag="sm1",
                                space="PSUM")
                for j in range(nt):
                    nc.tensor.transpose(
                        at_ps[:, j * HEADS:(j + 1) * HEADS],
                        in_=scores[:, (i + j) * 128:(i + j + 1) * 128],
                        identity=ident_bf4[:])
                at_sb = mp.tile([128, 4 * HEADS], bf16, tag="atsb")
                nc.scalar.activation(out=at_sb[:, :nt * HEADS],
                                     in_=at_ps[:, :nt * HEADS], func=AF.Copy)
                for j in range(nt):
                    t = i + j
                    nc.tensor.matmul(
                        ctx_ps[:], lhsT=vall[:, t, :],
                        rhs=at_sb[:, j * HEADS:(j + 1) * HEADS],
                        start=(t == 0), stop=(t == NBUCK - 1))

            ctx_sb = sp.tile([H, HEADS], f32, tag="ctxsb")
            nc.scalar.activation(out=ctx_sb[:], in_=ctx_ps[:], func=AF.Copy)
            ctxT_ps = ps.tile([HEADS, H], f32, tag="sm1", space="PSUM")
            nc.tensor.transpose(ctxT_ps[:], in_=ctx_sb[:],
                                identity=ident[:H, :H])
            pay = sp.tile([HEADS, H + 2], f32, tag="pay")
            nc.scalar.activation(out=pay[:, 0:H], in_=ctxT_ps[:], func=AF.Copy)
            nc.vector.tensor_copy(out=pay[:, H:H + 1], in_=s_loc[:])
            nc.vector.tensor_copy(out=pay[:, H + 1:H + 2], in_=s_loc[:])
            w_pay = nc.sync.dma_start(out=pay_own[:], in_=pay[:])
            ccp = nc.gpsimd.collective_compute(
                "AllGather", mybir.AluOpType.bypass, replica_groups=PAIRS,
                ins=[pay_own.opt()], outs=[pay_full.opt()])

            p0 = sp.tile([HEADS, H + 2], f32, tag="p0")
            p1 = sp.tile([HEADS, H + 2], f32, tag="p1")
            nc.sync.dma_start(out=p0[:], in_=pay_full[0])
            nc.sync.dma_start(out=p1[:], in_=pay_full[1])
            den = sp.tile([HEADS, 1], f32, tag="den")
            nc.vector.tensor_tensor(out=den[:], in0=p0[:, H:H + 1],
                                    in1=p1[:, H:H + 1], op=OP.add)
            rden = sp.tile([HEADS, 1], f32, tag="rden")
            nc.vector.reciprocal(out=rden[:], in_=den[:])
            ctxc = sp.tile([HEADS, H], f32, tag="ctxc")
            nc.vector.tensor_tensor(out=ctxc[:], in0=p0[:, 0:H],
                                    in1=p1[:, 0:H], op=OP.add)
            nc.vector.tensor_scalar(out=ctxc[:], in0=ctxc[:], scalar1=rden[:],
                                    scalar2=None, op0=OP.mult)
            ctxT2 = ps.tile([H, HEADS], f32, tag="sm1", space="PSUM")
            nc.tensor.transpose(ctxT2[:], in_=ctxc[:],
                                identity=ident[:HEADS, :HEADS])
            cmask = sp.tile([H, HEADS], f32, tag="cm")
            nc.sync.dma_start(out=cmask[:], in_=d_cmask[:])
            ctx_m = sp.tile([H, HEADS], f32, tag="ctxm")
            nc.vector.tensor_tensor(out=ctx_m[:], in0=ctxT2[:], in1=cmask[:],
                                    op=OP.mult)
            ctx_c = sp.tile([H, 1], f32, tag="ctxco")
            nc.vector.tensor_reduce(out=ctx_c[:], in_=ctx_m[:],
                                    axis=mybir.AxisListType.X, op=OP.add)
            bvc = sp.tile([H, 1], f32, tag="bvc")
            nc.sync.dma_start(out=bvc[:], in_=d_bv[:])
            nc.vector.tensor_tensor(out=ctx_c[:], in0=ctx_c[:], in1=bvc[:],
                                    op=OP.add)

            # g = layer_norm(a + ctx @ wo + bo)
            wo_t = sp.tile([H, H], f32, tag="w64e")
            nc.sync.dma_start(out=wo_t[:], in_=d_wo[:])
            go_ps = ps.tile([1, H], f32, tag="sm1", space="PSUM")
            nc.tensor.matmul(go_ps[:], lhsT=ctx_c[:], rhs=wo_t[:], start=True,
                             stop=True)
            bor = sp.tile([1, H], f32, tag="bor")
            nc.sync.dma_start(out=bor[:], in_=d_bo[:])
            g_row = sp.tile([1, H], f32, tag="grow")
            nc.vector.tensor_tensor(out=g_row[:], in0=go_ps[:], in1=bor[:],
                                    op=OP.add)
            nc.vector.tensor_tensor(out=g_row[:], in0=g_row[:], in1=a_row[:],
                                    op=OP.add)
            mu = sp.tile([1, 1], f32, tag="mu")
            nc.vector.tensor_reduce(out=mu[:], in_=g_row[:],
                                    axis=mybir.AxisListType.X, op=OP.add)
            nc.vector.tensor_scalar(out=mu[:], in0=mu[:], scalar1=1.0 / H,
                                    scalar2=None, op0=OP.mult)
            nc.vector.tensor_scalar(out=g_row[:], in0=g_row[:], scalar1=mu[:],
                                    scalar2=None, op0=OP.subtract)
            sq = sp.tile([1, H], f32, tag="sq")
            nc.scalar.activation(out=sq[:], in_=g_row[:], func=AF.Square)
            var = sp.tile([1, 1], f32, tag="var")
            nc.vector.tensor_reduce(out=var[:], in_=sq[:],
                                    axis=mybir.AxisListType.X, op=OP.add)
            std = sp.tile([1, 1], f32, tag="std")
            eps_t = sp.tile([1, 1], f32, tag="eps")
            nc.gpsimd.memset(eps_t[:], 1e-5)
            nc.scalar.activation(out=std[:], in_=var[:], func=AF.Sqrt,
                                 scale=1.0 / H, bias=eps_t[:])
            rstd = sp.tile([1, 1], f32, tag="rstd")
            nc.vector.reciprocal(out=rstd[:], in_=std[:])
            nc.vector.tensor_scalar(out=g_row[:], in0=g_row[:], scalar1=rstd[:],
                                    scalar2=None, op0=OP.mult)
            lng = sp.tile([1, H], f32, tag="lng")
            lnb = sp.tile([1, H], f32, tag="lnb")
            nc.sync.dma_start(out=lng[:], in_=d_lng[:])
            nc.sync.dma_start(out=lnb[:], in_=d_lnb[:])
            nc.vector.tensor_tensor(out=g_row[:], in0=g_row[:], in1=lng[:],
                                    op=OP.mult)
            nc.vector.tensor_tensor(out=g_row[:], in0=g_row[:], in1=lnb[:],
                                    op=OP.add)

            # logits = hT^T @ (g/8 + policy_w), masked (own half)
            pol = sp.tile([1, H], f32, tag="pol")
            nc.sync.dma_start(out=pol[:], in_=d_pol[:])
            nc.vector.tensor_scalar(out=g_row[:], in0=g_row[:], scalar1=1.0 / 8.0,
                                    scalar2=None, op0=OP.mult)
            nc.vector.tensor_tensor(out=g_row[:], in0=g_row[:], in1=pol[:],
                                    op=OP.add)
            wT_ps = ps.tile([H, 1], f32, tag="sm1", space="PSUM")
            nc.tensor.transpose(wT_ps[:], in_=g_row[:], identity=ident[:1, :1])
            w_col = sp.tile([H, 1], bf16, tag="wcol")
            nc.scalar.activation(out=w_col[:], in_=wT_ps[:], func=AF.Copy)
            lg_ps = ps.tile([128, NBUCK], f32, tag="sm1", space="PSUM")
            for t in range(NBUCK):
                nc.tensor.matmul(lg_ps[:, t:t + 1],
                                 lhsT=hT[:H, t * 128:(t + 1) * 128],
                                 rhs=w_col[:], start=True, stop=True)
            maskf = sp.tile([128, NBUCK], f32, tag="mf")
            maskn = sp.tile([128, NBUCK], f32, tag="mn")
            nc.sync.dma_start(out=maskf[:], in_=d_maskf[:])
            nc.sync.dma_start(out=maskn[:], in_=d_maskn[:])
            lg = sp.tile([128, NBUCK], f32, tag="lgsb")
            nc.vector.tensor_tensor(out=lg[:], in0=lg_ps[:], in1=maskf[:],
                                    op=OP.mult)
            nc.vector.tensor_tensor(out=lg[:], in0=lg[:], in1=maskn[:],
                                    op=OP.add)
            nc.sync.dma_start(out=d_out[:], in_=lg[:])

    nc.compile()
    return nc


def _wrap16(a):
    w = a.reshape(-1, 16).T
    return np.tile(w, (8, 1)).astype(np.int16)


def _prep_core(inputs, s, half):
    gn = np.asarray(inputs["graph_nodes"])
    links = np.asarray(inputs["graph_edge_links"])
    mask = np.asarray(inputs["mask"])

    x = np.zeros((NPAD, F_NODE), np.float32)
    x[:N_HALF] = gn[s, half * N_HALF:(half + 1) * N_HALF]
    xT = np.ascontiguousarray(x.T).astype(ml_dtypes.bfloat16)
    xf = np.zeros((2, NPAD, F_NODE), np.float32)
    xf[0, :N_HALF] = gn[s, :N_HALF]
    xf[1, :N_HALF] = gn[s, N_HALF:]
    xTF = np.ascontiguousarray(
        xf.reshape(NTAB, F_NODE).T).astype(ml_dtypes.bfloat16)

    src = links[s, 0].astype(np.int64)
    dst = links[s, 1].astype(np.int64)
    sel = (dst >= half * N_HALF) & (dst < (half + 1) * N_HALF)
    src_e = src[sel]
    dst_e = dst[sel]
    dl = dst_e - half * N_HALF                    # local 0..N_HALF
    psrc = src_e + (NPAD - N_HALF) * (src_e >= N_HALF)   # row in [0, NTAB)

    order = np.argsort(dl, kind="stable")
    dls = dl[order]
    pss = psrc[order]
    counts = np.bincount(dls, minlength=N_HALF)
    starts = np.zeros(N_HALF, np.int64)
    starts[1:] = np.cumsum(counts)[:-1]
    rank = np.arange(len(dls)) - starts[dls]

    mainsel = rank < K
    mn, mr, mp_ = dls[mainsel], rank[mainsel], pss[mainsel]
    idxM = np.full(MCOLS * 128, SENT, np.int64)
    slot = ((mn >> 7) * K + mr) * 128 + (mn & 127)
    idxM[slot] = mp_

    on, op_ = dls[~mainsel], pss[~mainsel]
    ob = on >> 7
    ocounts = np.bincount(ob, minlength=NBUCK)
    if ocounts.max() > OVFCH * 128:
        raise RuntimeError(f"ovf overflow: {ocounts.max()} > {OVFCH * 128}")
    ostarts = np.zeros(NBUCK, np.int64)
    ostarts[1:] = np.cumsum(ocounts)[:-1]
    within = np.arange(len(on)) - ostarts[ob]
    oslot = ob * (OVFCH * 128) + within
    idxOs = np.full(OCOLS * 128, SENT, np.int64)
    idxOd = np.full(OCOLS * 128, SENT, np.int64)
    dlocv = np.full(OCOLS * 128, 128, np.float32)
    idxOs[oslot] = op_
    idxOd[oslot] = HS2OFF + on
    dlocv[oslot] = (on & 127)

    blocks = []
    for (c0, og) in _ovf_groups():
        blocks.append(_wrap16(idxOs[c0 * 128:(c0 + og) * 128]))
        blocks.append(_wrap16(idxOd[c0 * 128:(c0 + og) * 128]))
    cmbO = np.ascontiguousarray(np.concatenate(blocks, axis=1))
    cmbM = _wrap16(idxM)
    dl_cols = dlocv.reshape(OCOLS, 128).T
    dlocO = np.ascontiguousarray(
        np.repeat(dl_cols, 2, axis=1)).astype(ml_dtypes.bfloat16)

    m = np.zeros(NPAD, bool)
    m[:N_HALF] = mask[s, half * N_HALF:(half + 1) * N_HALF]
    maskf = np.where(m, np.float32(1.0), np.float32(0.0))
    pb = np.float32(np.asarray(inputs["policy_b"]))
    maskn = np.where(m, pb, NEG)
    maskf = np.ascontiguousarray(maskf.reshape(NBUCK, 128).T)
    maskn = np.ascontiguousarray(maskn.reshape(NBUCK, 128).T)

    return {
        "xT": xT, "xTF": xTF, "cmbM": cmbM, "cmbO": cmbO, "dlocO": dlocO,
        "maskf": maskf, "maskneg": maskn,
        "ad_col": np.asarray(inputs["current_ad"])[s].reshape(F_AD, 1)
                    .astype(np.float32),
    }


def kernel(**inputs):
    from concourse.bass_utils import run_bass_kernel_spmd

    if "nc" not in _CACHE:
        _CACHE["nc"] = _build()
    nc = _CACHE["nc"]

    f = lambda k: np.ascontiguousarray(np.asarray(inputs[k], np.float32))
    bf = lambda a: np.ascontiguousarray(a).astype(ml_dtypes.bfloat16)
    iot = np.tile(np.arange(128, dtype=np.float32), (128, 1))
    blockmask = np.zeros((H, HEADS), np.float32)
    for hh in range(HEADS):
        blockmask[hh * DH:(hh + 1) * DH, hh] = 1.0

    msg_w = f("msg_w")
    upd_w = f("upd_w")
    common = {
        "iota128": iot.astype(ml_dtypes.bfloat16),
        "w_node16": bf(f("w_node")),
        "b_node_col": f("b_node").reshape(H, 1),
        "msgw1": bf(np.concatenate(
            [msg_w[:, :H, :].transpose(1, 0, 2),
             f("msg_b").reshape(1, L, H)], axis=0)),
        "msgw2": bf(msg_w[:, H:, :].transpose(1, 0, 2)),
        "bias_rep": np.tile(f("msg_b").reshape(1, L, H), (128, 1, 1)),
        "updw1": bf(upd_w[:, :H, :].transpose(1, 0, 2)),
        "updw2": bf(upd_w[:, H:, :].transpose(1, 0, 2)),
        "upd_b_col": np.ascontiguousarray(f("upd_b").T),
        "w_ad": f("w_ad"), "b_ad_row": f("b_ad").reshape(1, H),
        "wq": f("wq"), "bq_row": f("bq").reshape(1, H),
        "wk16": bf(f("wk")), "wv16": bf(f("wv")),
        "bv_col": f("bv").reshape(H, 1),
        "wo": f("wo"), "bo_row": f("bo").reshape(1, H),
        "ln_g_row": f("ln_g").reshape(1, H), "ln_b_row": f("ln_b").reshape(1, H),
        "qmask": blockmask * np.float32(1.0 / np.sqrt(DH)),
        "cmask": blockmask,
        "pol_row": f("policy_w").reshape(1, H),
    }

    in_maps = []
    for c in range(NCORES):
        m = dict(common)
        m.update(_prep_core(inputs, c // 2, c % 2))
        in_maps.append(m)

    res = run_bass_kernel_spmd(nc, in_maps, core_ids=list(range(NCORES)))
    _CACHE["last_results"] = res

    out = np.empty((B, N), np.float32)
    for c in range(NCORES):
        s, half = c // 2, c % 2
        lg = np.asarray(res.results[c]["logits"])      # [128, NBUCK]
        flat = lg.T.reshape(NPAD)
        out[s, half * N_HALF:(half + 1) * N_HALF] = flat[:N_HALF]
    return out



# revision 13
# speedup vs baseline: 1.1350x; 1.1302x over previous
"""Trainium2 Bass kernel for BillboardAllocatorGNN.

Sharding: 8 cores; core c handles sample c//2, node-half c%2 (data parallel
over batch, dst-parallel within each sample pair).

Edge phase (per layer): node-major edge slots with fixed per-node capacity
K=18 kill both the dst-side gather and the one-hot scatter for 96% of edges:
a single SWDGE gather fetches duplicated-bf16 hs1 rows (256B descriptors)
into [dst-node-partition, slot] layout, the dst-side hs2 contribution is a
free-dim broadcast add from SBUF, and segment-sum is a strided free-axis
tensor_reduce. Overflow edges (deg>K) go through a small one-hot matmul
side path whose PSUM accumulation group also hosts the main agg transpose.
Pad slots gather a -30000 sentinel row so relu zeroes them.

Pipelining: updates, next-layer table builds, and attention score/value
chunks are interleaved into the main gather stream. Layer 0 builds its
table locally from raw features (no collective); layer 1 exchanges hs1
in slices issued as updates complete; the final attention merges across
the pair via a 2KB unnormalized-softmax stats exchange (scores are O(1),
so no max subtraction is needed).
"""
import sys
import os

sys.path.insert(0, "/opt/trn_rl_repo")

import numpy as np
import ml_dtypes

# ---- problem dims (hardcoded per spec) ----
B, N, E = 4, 20000, 320000
F_NODE, F_AD = 16, 8
H, L, HEADS = 64, 2, 4
DH = H // HEADS

NCORES = 8
N_HALF = N // 2                 # 10000 real nodes per core
NBUCK = 79                      # 128-node pages per core
NPAD = NBUCK * 128              # 10112 padded nodes per core
K = 18                          # main slots per node
MCOLS = NBUCK * K               # 1422 main gather columns
PPG = 2                         # pages per main gather group
NGRP = (NBUCK + PPG - 1) // PPG  # 27
OVFCH = 2                       # overflow chunks per page (cap 256 edges)
OCOLS = NBUCK * OVFCH           # 158 overflow columns
OG = 12                         # overflow chunks per gather group
NTAB = 2 * NPAD                 # hs1 rows (both halves)
SENT1 = NTAB                    # sentinel row in hs1 table
SENT2 = NPAD                    # sentinel row in hs2 table (local ids)
CH = 512                        # node-chunk for encoder/update/attention
NEG = np.float32(-1e9)

_CACHE = {}


def _ovf_groups():
    out = []
    c = 0
    while c < OCOLS:
        g = min(OG, OCOLS - c)
        out.append((c, g))
        c += g
    return out


def _main_groups():
    out = []
    b = 0
    while b < NBUCK:
        g = min(PPG, NBUCK - b)
        out.append((b, g))
        b += g
    return out


def _build():
    import concourse.mybir as mybir
    import concourse.tile as tile
    import concourse.bacc as bacc
    from concourse.tile import add_dep_helper
    from concourse.masks import make_identity

    f32 = mybir.dt.float32
    bf16 = mybir.dt.bfloat16
    i16 = mybir.dt.int16
    AF = mybir.ActivationFunctionType
    OP = mybir.AluOpType

    nc = bacc.Bacc("TRN2", target_bir_lowering=False, debug=False,
                   num_swdge_queues=2)

    # ---- I/O ----
    d_xT = nc.dram_tensor("xT", [F_NODE, NPAD], bf16, kind="ExternalInput")
    d_xTF = nc.dram_tensor("xTF", [F_NODE, NTAB], bf16, kind="ExternalInput")
    d_cmbM = nc.dram_tensor("cmbM", [128, MCOLS * 8], i16, kind="ExternalInput")
    d_cmbO = nc.dram_tensor("cmbO", [128, 2 * OCOLS * 8], i16,
                            kind="ExternalInput")
    d_dlocO = nc.dram_tensor("dlocO", [128, 2 * OCOLS], bf16,
                             kind="ExternalInput")
    d_iota = nc.dram_tensor("iota128", [128, 128], bf16, kind="ExternalInput")
    d_maskf = nc.dram_tensor("maskf", [128, NBUCK], f32, kind="ExternalInput")
    d_maskn = nc.dram_tensor("maskneg", [128, NBUCK], f32, kind="ExternalInput")
    d_wnode = nc.dram_tensor("w_node16", [F_NODE, H], bf16,
                             kind="ExternalInput")
    d_bnode = nc.dram_tensor("b_node_col", [H, 1], f32, kind="ExternalInput")
    d_msgw1 = nc.dram_tensor("msgw1", [H + 1, L, H], bf16,
                             kind="ExternalInput")
    d_msgw2 = nc.dram_tensor("msgw2", [H, L, H], bf16, kind="ExternalInput")
    d_brep = nc.dram_tensor("bias_rep", [128, L, H], f32, kind="ExternalInput")
    d_updw1 = nc.dram_tensor("updw1", [H, L, H], bf16, kind="ExternalInput")
    d_updw2 = nc.dram_tensor("updw2", [H, L, H], bf16, kind="ExternalInput")
    d_updb = nc.dram_tensor("upd_b_col", [H, L], f32, kind="ExternalInput")
    d_wad = nc.dram_tensor("w_ad", [F_AD, H], f32, kind="ExternalInput")
    d_bad = nc.dram_tensor("b_ad_row", [1, H], f32, kind="ExternalInput")
    d_ad = nc.dram_tensor("ad_col", [F_AD, 1], f32, kind="ExternalInput")
    d_wq = nc.dram_tensor("wq", [H, H], f32, kind="ExternalInput")
    d_bq = nc.dram_tensor("bq_row", [1, H], f32, kind="ExternalInput")
    d_wk16 = nc.dram_tensor("wk16", [H, H], bf16, kind="ExternalInput")
    d_wv16 = nc.dram_tensor("wv16", [H, H], bf16, kind="ExternalInput")
    d_bv = nc.dram_tensor("bv_col", [H, 1], f32, kind="ExternalInput")
    d_wo = nc.dram_tensor("wo", [H, H], f32, kind="ExternalInput")
    d_bo = nc.dram_tensor("bo_row", [1, H], f32, kind="ExternalInput")
    d_lng = nc.dram_tensor("ln_g_row", [1, H], f32, kind="ExternalInput")
    d_lnb = nc.dram_tensor("ln_b_row", [1, H], f32, kind="ExternalInput")
    d_qmask = nc.dram_tensor("qmask", [H, HEADS], f32, kind="ExternalInput")
    d_cmask = nc.dram_tensor("cmask", [H, HEADS], f32, kind="ExternalInput")
    d_pol = nc.dram_tensor("pol_row", [1, H], f32, kind="ExternalInput")
    d_out = nc.dram_tensor("logits", [128, NBUCK], f32, kind="ExternalOutput")

    PAIRS = [[0, 1], [2, 3], [4, 5], [6, 7]]
    n_ch = (NPAD + CH - 1) // CH
    TB = 4                      # node pages per table-export batch

    with tile.TileContext(nc) as tc:
        with (
            tc.tile_pool(name="persist", bufs=1) as pp,
            tc.tile_pool(name="mp", bufs=2) as mp,
            tc.tile_pool(name="g3", bufs=5) as g3,
            tc.tile_pool(name="single", bufs=1) as sp,
            tc.tile_pool(name="psum", bufs=2, space="PSUM") as ps,
            tc.tile_pool(name="dram", bufs=1, space="DRAM") as dp,
        ):
            # ---- persistent state / constants ----
            hT = pp.tile([H + 1, NPAD], bf16)      # node states + ones row
            aggT = pp.tile([H, NPAD], bf16)        # per-layer aggregate
            hs2 = pp.tile([128, NBUCK, H], bf16)   # dst table, node-major
            ovfmsg = pp.tile([128, OCOLS, H], bf16)
            cmbO = pp.tile([128, 2 * OCOLS * 8], i16)
            dlocO = pp.tile([128, 2 * OCOLS], bf16)
            iota = pp.tile([128, 128], bf16)
            ident = pp.tile([128, 128], f32)
            wnode = pp.tile([F_NODE, H], bf16)
            bnode = pp.tile([H, 1], f32)
            msgw1 = pp.tile([H + 1, L, H], bf16)
            msgw2 = pp.tile([H, L, H], bf16)
            updw1 = pp.tile([H, L, H], bf16)
            updw2 = pp.tile([H, L, H], bf16)
            updb = pp.tile([H, L], f32)

            make_identity(nc, ident[:])
            nc.gpsimd.memset(hT[H:H + 1, :], 1.0)
            ident_bf4 = pp.tile([HEADS, HEADS], bf16)
            nc.vector.tensor_copy(out=ident_bf4[:], in_=ident[:HEADS, :HEADS])

            for dst_t, src_t in [
                (cmbO, d_cmbO), (dlocO, d_dlocO),
                (iota, d_iota), (wnode, d_wnode), (bnode, d_bnode),
                (msgw1, d_msgw1), (msgw2, d_msgw2),
                (updw1, d_updw1), (updw2, d_updw2), (updb, d_updb),
            ]:
                nc.sync.dma_start(out=dst_t[:], in_=src_t[:])

            # ---- DRAM: gather tables, hs1 exchange, attention payload ----
            # Separate hs1/hs2 tables so the tile framework's DRAM dep
            # tracking doesn't serialize layer-l hs1 gathers behind the
            # layer-(l+1) hs2 rewrites issued mid-stream.
            tab = dp.tile([NTAB + 1, 128], bf16)     # hs1 rows + sentinel
            tab2 = dp.tile([NPAD + 1, 128], bf16)    # hs2 rows + sentinel
            hs1own = dp.tile([NPAD, H], bf16)
            hs1pair = dp.tile([NTAB, H], bf16)
            pay_own = dp.tile([HEADS, H + 2], f32)
            pay_full = dp.tile([2, HEADS, H + 2], f32)

            # sentinel rows (once; hs1/hs2 rewrites never touch them)
            sentc = sp.tile([1, 128], bf16, tag="sent")
            nc.gpsimd.memset(sentc[:], -30000.0)
            w_sent = nc.sync.dma_start(out=tab[SENT1:SENT1 + 1, :],
                                       in_=sentc[:])
            w_sent2 = nc.sync.dma_start(out=tab2[SENT2:SENT2 + 1, :],
                                        in_=sentc[:])

            # ---- node encoder: hT = relu(w_node^T @ xT + b) ----
            for i in range(n_ch):
                lo = i * CH
                sz = min(CH, NPAD - lo)
                xt = mp.tile([F_NODE, CH], bf16, tag="xtf")
                nc.sync.dma_start(out=xt[:, :sz], in_=d_xT[:, lo:lo + sz])
                h_ps = ps.tile([H, CH], f32, tag="big", space="PSUM")
                nc.tensor.matmul(h_ps[:, :sz], lhsT=wnode[:], rhs=xt[:, :sz],
                                 start=True, stop=True)
                nc.vector.tensor_scalar(out=hT[:H, lo:lo + sz],
                                        in0=h_ps[:, :sz], scalar1=bnode[:],
                                        scalar2=0.0, op0=OP.add, op1=OP.max)

            # ---- attention prelude: q from ad (independent of layers) ----
            wad = sp.tile([F_AD, H], f32, tag="w64")
            adc = sp.tile([F_AD, 1], f32, tag="col")
            nc.sync.dma_start(out=wad[:], in_=d_wad[:])
            nc.sync.dma_start(out=adc[:], in_=d_ad[:])
            a_ps = ps.tile([1, H], f32, tag="sm1", space="PSUM")
            nc.tensor.matmul(a_ps[:], lhsT=adc[:], rhs=wad[:], start=True,
                             stop=True)
            bad = sp.tile([1, H], f32, tag="row1")
            nc.sync.dma_start(out=bad[:], in_=d_bad[:])
            a_row = sp.tile([1, H], f32, tag="arow")
            nc.vector.tensor_tensor(out=a_row[:], in0=a_ps[:], in1=bad[:],
                                    op=OP.add)
            nc.vector.tensor_scalar(out=a_row[:], in0=a_row[:], scalar1=0.0,
                                    scalar2=None, op0=OP.max)
            aT_ps = ps.tile([H, 1], f32, tag="sm1", space="PSUM")
            nc.tensor.transpose(aT_ps[:], in_=a_row[:], identity=ident[:1, :1])
            a_col = sp.tile([H, 1], f32, tag="acol")
            nc.scalar.activation(out=a_col[:], in_=aT_ps[:], func=AF.Copy)
            wq_t = sp.tile([H, H], f32, tag="w64b")
            nc.sync.dma_start(out=wq_t[:], in_=d_wq[:])
            q_ps = ps.tile([1, H], f32, tag="sm1", space="PSUM")
            nc.tensor.matmul(q_ps[:], lhsT=a_col[:], rhs=wq_t[:], start=True,
                             stop=True)
            bqr = sp.tile([1, H], f32, tag="row2")
            nc.sync.dma_start(out=bqr[:], in_=d_bq[:])
            q_row = sp.tile([1, H], f32, tag="qrow")
            nc.vector.tensor_tensor(out=q_row[:], in0=q_ps[:], in1=bqr[:],
                                    op=OP.add)
            qT_ps = ps.tile([H, 1], f32, tag="sm1", space="PSUM")
            nc.tensor.transpose(qT_ps[:], in_=q_row[:], identity=ident[:1, :1])
            qmask = sp.tile([H, HEADS], f32, tag="qm")
            nc.sync.dma_start(out=qmask[:], in_=d_qmask[:])
            q_col = sp.tile([H, 1], f32, tag="qcol")
            nc.scalar.activation(out=q_col[:], in_=qT_ps[:], func=AF.Copy)
            qblk = sp.tile([H, HEADS], bf16, tag="qblk")
            nc.vector.tensor_tensor(out=qblk[:],
                                    in0=q_col[:].to_broadcast([H, HEADS]),
                                    in1=qmask[:], op=OP.mult)
            wk16 = sp.tile([H, H], bf16, tag="w64c")
            wv16 = sp.tile([H, H], bf16, tag="w64d")
            nc.sync.dma_start(out=wk16[:], in_=d_wk16[:])
            nc.sync.dma_start(out=wv16[:], in_=d_wv16[:])

            # attention state (filled during the last layer's main stream)
            scores = pp.tile([HEADS, NPAD], bf16)    # exp(raw), unnormalized
            vall = pp.tile([128, NBUCK, H], bf16)
            sm = sp.tile([HEADS, n_ch], f32, tag="sm")

            # ---- message-passing layers, software-pipelined ----
            SL = [(0, 48), (48, 72), (72, NBUCK)]    # cc slices (pages)
            hs1pairs = []
            for _si, (p0, p1) in enumerate(SL):
                hs1pair_s = dp.tile([2 * 128 * (p1 - p0), H], bf16,
                                    name=f"hs1pair_{_si}")
                hs1pairs.append(hs1pair_s)
            hs2t = []
            for _li in range(L):
                hs2_l = pp.tile([128, NBUCK, H], bf16, name=f"hs2_{_li}")
                hs2t.append(hs2_l)

            gathers = [[] for _ in range(L)]         # per-layer gather instrs
            hs2_writes = [[] for _ in range(L)]
            ccs = [[] for _ in range(L)]
            expands = [[] for _ in range(L)]

            def hs2_batch(l, t0):
                tn = min(TB, NBUCK - t0)
                p2 = ps.tile([128, TB * H], f32, tag="tab8", space="PSUM")
                for j in range(tn):
                    nc.tensor.matmul(
                        p2[:, j * H:(j + 1) * H],
                        lhsT=hT[:H, (t0 + j) * 128:(t0 + j + 1) * 128],
                        rhs=msgw2[:, l, :], start=True, stop=True)
                nc.scalar.activation(
                    out=hs2t[l][:, t0:t0 + tn, :],
                    in_=p2[:, :tn * H].rearrange("p (t f) -> p t f", f=H),
                    func=AF.Copy)
                dst0 = tab2[t0 * 128:(t0 + tn) * 128,
                            0:H].rearrange("(t p) f -> p t f", p=128)
                w1 = nc.sync.dma_start(out=dst0, in_=hs2t[l][:, t0:t0 + tn, :])
                if l > 0:
                    # WAR: previous layer's overflow-dst gathers read this
                    nogrp = len(_ovf_groups())
                    for gi in gathers[l - 1][:nogrp]:
                        add_dep_helper(w1.ins, gi.ins,
                                       reason="WAR: hs2 rewrite after gathers")
                hs2_writes[l].append(w1)

            def hs1_batch(l, t0):
                tn = min(TB, NBUCK - t0)
                p1 = ps.tile([128, TB * H], f32, tag="tab8", space="PSUM")
                for j in range(tn):
                    nc.tensor.matmul(
                        p1[:, j * H:(j + 1) * H],
                        lhsT=hT[:, (t0 + j) * 128:(t0 + j + 1) * 128],
                        rhs=msgw1[:, l, :], start=True, stop=True)
                st = mp.tile([128, TB, H], bf16, tag="st")
                nc.scalar.activation(
                    out=st[:, :tn, :],
                    in_=p1[:, :tn * H].rearrange("p (t f) -> p t f", f=H),
                    func=AF.Copy)
                nc.sync.dma_start(
                    out=hs1own[t0 * 128:(t0 + tn) * 128, :].rearrange(
                        "(t p) f -> p t f", p=128),
                    in_=st[:, :tn, :])

            def issue_cc(l, s):
                p0, p1 = SL[s]
                cc0 = nc.gpsimd.collective_compute(
                    "AllGather", mybir.AluOpType.bypass, replica_groups=PAIRS,
                    ins=[hs1own[p0 * 128:p1 * 128, :]],
                    outs=[hs1pairs[s].opt()])
                if l > 0:
                    for e in expands[l - 1]:
                        add_dep_helper(cc0.ins, e.ins,
                                       reason="WAR: pair rewrite after expand")
                ccs[l].append(cc0)

            def expand_tab(l):
                # copy exchanged hs1 slices into tab hs1 region (cols 0:64)
                for s, (p0, p1) in enumerate(SL):
                    n_s = 128 * (p1 - p0)
                    cc0 = ccs[l][s]
                    for hf in range(2):
                        base = hf * NPAD + p0 * 128
                        e = nc.sync.dma_start(
                            out=tab[base:base + n_s, 0:H].rearrange(
                                "(t p) f -> p t f", p=128),
                            in_=hs1pairs[s][hf * n_s:(hf + 1) * n_s,
                                            :].rearrange(
                                "(t p) f -> p t f", p=128))
                        add_dep_helper(e.ins, cc0.ins,
                                       reason="RAW: expand after allgather")
                        if l > 0:
                            for gi in gathers[l - 1]:
                                add_dep_helper(
                                    e.ins, gi.ins,
                                    reason="WAR: tab rewrite after gathers")
                        expands[l].append(e)

            def update_chunk(l, c):
                lo = c * CH
                sz = min(CH, NPAD - lo)
                u_ps = ps.tile([H, CH], f32, tag="big", space="PSUM")
                nc.tensor.matmul(u_ps[:, :sz], lhsT=updw1[:, l, :],
                                 rhs=hT[:H, lo:lo + sz], start=True, stop=False)
                nc.tensor.matmul(u_ps[:, :sz], lhsT=updw2[:, l, :],
                                 rhs=aggT[:, lo:lo + sz], start=False,
                                 stop=True)
                un = mp.tile([H, CH], bf16, tag="un")
                nc.scalar.activation(out=un[:, :sz], in_=u_ps[:, :sz],
                                     func=AF.Relu, bias=updb[:, l:l + 1])
                nc.vector.tensor_tensor(out=hT[:H, lo:lo + sz],
                                        in0=hT[:H, lo:lo + sz],
                                        in1=un[:, :sz], op=OP.add)

            def att_chunk(c):
                lo = c * CH
                sz = min(CH, NPAD - lo)
                nt = sz // 128
                kT_ps = ps.tile([H, CH], f32, tag="big", space="PSUM")
                nc.tensor.matmul(kT_ps[:, :sz], lhsT=wk16[:],
                                 rhs=hT[:H, lo:lo + sz], start=True, stop=True)
                kT_sb = mp.tile([H, CH], bf16, tag="kT")
                nc.scalar.activation(out=kT_sb[:, :sz], in_=kT_ps[:, :sz],
                                     func=AF.Copy)
                s_ps = ps.tile([HEADS, CH], f32, tag="sm1", space="PSUM")
                nc.tensor.matmul(s_ps[:, :sz], lhsT=qblk[:], rhs=kT_sb[:, :sz],
                                 start=True, stop=True)
                nc.scalar.activation(out=scores[:, lo:lo + sz],
                                     in_=s_ps[:, :sz], func=AF.Exp)
                if lo + sz > N_HALF:
                    nc.gpsimd.memset(scores[:, N_HALF:], 0.0)
                nc.vector.tensor_reduce(out=sm[:, c:c + 1],
                                        in_=scores[:, lo:lo + sz],
                                        axis=mybir.AxisListType.X, op=OP.add)
                v_ps = ps.tile([128, 4 * H], f32, tag="tab8", space="PSUM")
                for j in range(nt):
                    nc.tensor.matmul(
                        v_ps[:, j * H:(j + 1) * H],
                        lhsT=hT[:H, lo + j * 128:lo + (j + 1) * 128],
                        rhs=wv16[:], start=True, stop=True)
                nc.scalar.activation(
                    out=vall[:, lo // 128:lo // 128 + nt, :],
                    in_=v_ps[:, :nt * H].rearrange("p (t f) -> p t f", f=H),
                    func=AF.Copy)

            def post_update(l, c):
                if l + 1 < L:
                    if c >= 1:
                        hs2_batch(l + 1, 4 * (c - 1))
                        hs1_batch(l + 1, 4 * (c - 1))
                    if c == 13:
                        issue_cc(l + 1, 0)
                    elif c == 19:
                        issue_cc(l + 1, 1)
                else:
                    att_chunk(c)

            def gdeps(l, gi, writes, after=()):
                add_dep_helper(gi.ins, w_sent.ins, reason="RAW: sentinel")
                add_dep_helper(gi.ins, w_sent2.ins, reason="RAW: sentinel2")
                for w in writes:
                    add_dep_helper(gi.ins, w.ins, reason="RAW: tab write")
                for cx in after:
                    add_dep_helper(gi.ins, cx.ins, reason="RAW: tab ready")
                gathers[l].append(gi)

            def ovf_dst_stream(l):
                for (c0, og) in _ovf_groups():
                    goutO = mp.tile([128, OG, 128], bf16, tag="goutO")
                    nidx = og * 128
                    gi = nc.gpsimd.dma_gather(
                        out_ap=goutO[:, :og, :], in_ap=tab2[:],
                        idxs_ap=cmbO[:, (2 * c0 + og) * 8:2 * (c0 + og) * 8],
                        num_idxs=nidx, num_idxs_reg=nidx, elem_size=128,
                        queue_num=0, single_packet=False)
                    gdeps(l, gi, hs2_writes[l])
                    nc.vector.tensor_copy(out=ovfmsg[:, c0:c0 + og, :],
                                          in_=goutO[:, 0:og, 0:H])

            def main_phase(l):
                # overflow src rows + messages
                for (c0, og) in _ovf_groups():
                    goutO = mp.tile([128, OG, 128], bf16, tag="goutO")
                    nidx = og * 128
                    gi = nc.gpsimd.dma_gather(
                        out_ap=goutO[:, :og, :], in_ap=tab[:],
                        idxs_ap=cmbO[:, 2 * c0 * 8:(2 * c0 + og) * 8],
                        num_idxs=nidx, num_idxs_reg=nidx, elem_size=128,
                        queue_num=0, single_packet=False)
                    gdeps(l, gi, expands[l])
                    nc.vector.tensor_tensor(
                        out=ovfmsg[:, c0:c0 + og, :],
                        in0=ovfmsg[:, c0:c0 + og, :],
                        in1=goutO[:, 0:og, 0:H], op=OP.add)
                    nc.scalar.activation(out=ovfmsg[:, c0:c0 + og, :],
                                         in_=ovfmsg[:, c0:c0 + og, :],
                                         func=AF.Relu)

                next_up = 0

                def issue_gather(gidx, b0, gp):
                    cols = gp * K
                    gout = g3.tile([128, PPG * K, 128], bf16, tag="gout")
                    idxs = g3.tile([128, PPG * K * 8], i16, tag="gidx")
                    nc.sync.dma_start(
                        out=idxs[:, :cols * 8],
                        in_=d_cmbM[:, b0 * K * 8:(b0 + gp) * K * 8])
                    gi = nc.gpsimd.dma_gather(
                        out_ap=gout[:, :cols, :], in_ap=tab[:],
                        idxs_ap=idxs[:, :cols * 8],
                        num_idxs=cols * 128, num_idxs_reg=cols * 128,
                        elem_size=128, queue_num=gidx % 2,
                        single_packet=False)
                    gdeps(l, gi, (), after=expands[l])
                    return gout

                def consume_group(b0, gp, gout):
                    nonlocal next_up
                    msg = mp.tile([128, PPG * K, H], bf16, tag="msg")
                    agg = mp.tile([128, PPG, H], f32, tag="agg")
                    p_sc = ps.tile([H, PPG * 128], f32, tag="scat",
                                   space="PSUM")
                    ohpg = mp.tile([128, PPG * OVFCH, 128], bf16, tag="ohpg")
                    dl4 = dlocO[:, 2 * b0 * OVFCH:2 * (b0 + gp) * OVFCH
                                ].rearrange("p (g two) -> p g two", two=2)[
                        :, :, None, :].to_broadcast([128, gp * OVFCH, 64, 2])
                    io4 = iota[:].rearrange("p (s two) -> p s two", two=2)[
                        :, None, :, :].to_broadcast([128, gp * OVFCH, 64, 2])
                    oh4 = ohpg[:, :gp * OVFCH, :].rearrange(
                        "p g (s two) -> p g s two", two=2)
                    nc.vector.tensor_tensor(out=oh4, in0=dl4, in1=io4,
                                            op=OP.is_equal)
                    for j in range(gp):
                        nc.vector.tensor_tensor(
                            out=msg[:, j * K:(j + 1) * K, :],
                            in0=gout[:, j * K:(j + 1) * K, 0:H],
                            in1=hs2t[l][:, b0 + j:b0 + j + 1, :].to_broadcast(
                                [128, K, H]), op=OP.add)
                        nc.vector.tensor_scalar(
                            out=msg[:, j * K:(j + 1) * K, :],
                            in0=msg[:, j * K:(j + 1) * K, :],
                            scalar1=0.0, scalar2=None, op0=OP.max)
                    for j in range(gp):
                        pg = b0 + j
                        nc.vector.tensor_reduce(
                            out=agg[:, j, :],
                            in_=msg[:, j * K:(j + 1) * K, :].rearrange(
                                "p j f -> p f j"),
                            axis=mybir.AxisListType.X, op=OP.add)
                        sl = p_sc[:, j * 128:(j + 1) * 128]
                        nc.tensor.matmul(sl, lhsT=agg[:, j, :], rhs=ident[:],
                                         is_transpose=True, start=True,
                                         stop=False)
                        for k in range(OVFCH):
                            cc_i = pg * OVFCH + k
                            nc.tensor.matmul(
                                sl, lhsT=ovfmsg[:, cc_i, :],
                                rhs=ohpg[:, j * OVFCH + k, :],
                                start=False, stop=(k == OVFCH - 1))
                    nc.scalar.activation(
                        out=aggT[:, b0 * 128:(b0 + gp) * 128],
                        in_=p_sc[:, :gp * 128], func=AF.Copy)
                    while (next_up < n_ch
                           and (4 * next_up + 4) * 128 <= (b0 + gp) * 128):
                        update_chunk(l, next_up)
                        post_update(l, next_up)
                        next_up += 1

                AHEAD = 4
                pend = []
                for gidx, (b0, gp) in enumerate(_main_groups()):
                    pend.append((b0, gp, issue_gather(gidx, b0, gp)))
                    if len(pend) > AHEAD:
                        consume_group(*pend.pop(0))
                for item in pend:
                    consume_group(*item)
                while next_up < n_ch:
                    update_chunk(l, next_up)
                    post_update(l, next_up)
                    next_up += 1

            # layer 0 tables: h0 = encoder(x) is computable locally for
            # BOTH halves from the raw features -- no collective needed
            for t0 in range(0, NBUCK, TB):
                hs2_batch(0, t0)
            for gt0 in range(0, 2 * NBUCK, TB):
                tn = min(TB, 2 * NBUCK - gt0)
                hf_ps = ps.tile([H, TB * 128], f32, tag="big", space="PSUM")
                xtf = mp.tile([F_NODE, TB * 128], bf16, tag="xtf")
                nc.sync.dma_start(out=xtf[:, :tn * 128],
                                  in_=d_xTF[:, gt0 * 128:(gt0 + tn) * 128])
                nc.tensor.matmul(hf_ps[:, :tn * 128], lhsT=wnode[:],
                                 rhs=xtf[:, :tn * 128], start=True, stop=True)
                htmp = mp.tile([H + 1, TB * 128], bf16, tag="kT")
                nc.vector.tensor_scalar(out=htmp[:H, :tn * 128],
                                        in0=hf_ps[:, :tn * 128],
                                        scalar1=bnode[:], scalar2=0.0,
                                        op0=OP.add, op1=OP.max)
                nc.gpsimd.memset(htmp[H:H + 1, :tn * 128], 1.0)
                p1g = ps.tile([128, TB * H], f32, tag="tab8", space="PSUM")
                for j in range(tn):
                    nc.tensor.matmul(
                        p1g[:, j * H:(j + 1) * H],
                        lhsT=htmp[:, j * 128:(j + 1) * 128],
                        rhs=msgw1[:, 0, :], start=True, stop=True)
                stg = mp.tile([128, TB, H], bf16, tag="st")
                nc.scalar.activation(
                    out=stg[:, :tn, :],
                    in_=p1g[:, :tn * H].rearrange("p (t f) -> p t f", f=H),
                    func=AF.Copy)
                e0 = nc.sync.dma_start(
                    out=tab[gt0 * 128:(gt0 + tn) * 128, 0:H].rearrange(
                        "(t p) f -> p t f", p=128),
                    in_=stg[:, :tn, :])
                expands[0].append(e0)
            ovf_dst_stream(0)
            main_phase(0)

            # layer 1: tables/cc mostly issued inside layer 0's stream
            hs2_batch(1, 76)
            hs1_batch(1, 76)
            issue_cc(1, 2)
            expand_tab(1)
            ovf_dst_stream(1)
            main_phase(1)

            # ---- attention tail: sums, ctx, pair merge ----
            s_loc = sp.tile([HEADS, 1], f32, tag="m3")
            nc.vector.tensor_reduce(out=s_loc[:], in_=sm[:],
                                    axis=mybir.AxisListType.X, op=OP.add)
            ctx_ps = ps.tile([H, HEADS], f32, tag="tab8", space="PSUM")
            for i in range(0, NBUCK, 4):
                nt = min(4, NBUCK - i)
                at_ps = ps.tile([128, 4 * HEADS], bf16, tag="sm1",
                                space="PSUM")
                for j in range(nt):
                    nc.tensor.transpose(
                        at_ps[:, j * HEADS:(j + 1) * HEADS],
                        in_=scores[:, (i + j) * 128:(i + j + 1) * 128],
                        identity=ident_bf4[:])
                at_sb = mp.tile([128, 4 * HEADS], bf16, tag="atsb")
                nc.scalar.activation(out=at_sb[:, :nt * HEADS],
                                     in_=at_ps[:, :nt * HEADS], func=AF.Copy)
                for j in range(nt):
                    t = i + j
                    nc.tensor.matmul(
                        ctx_ps[:], lhsT=vall[:, t, :],
                        rhs=at_sb[:, j * HEADS:(j + 1) * HEADS],
                        start=(t == 0), stop=(t == NBUCK - 1))

            ctx_sb = sp.tile([H, HEADS], f32, tag="ctxsb")
            nc.scalar.activation(out=ctx_sb[:], in_=ctx_ps[:], func=AF.Copy)
            ctxT_ps = ps.tile([HEADS, H], f32, tag="sm1", space="PSUM")
            nc.tensor.transpose(ctxT_ps[:], in_=ctx_sb[:],
                                identity=ident[:H, :H])
            pay = sp.tile([HEADS, H + 2], f32, tag="pay")
            nc.scalar.activation(out=pay[:, 0:H], in_=ctxT_ps[:], func=AF.Copy)
            nc.vector.tensor_copy(out=pay[:, H:H + 1], in_=s_loc[:])
            nc.vector.tensor_copy(out=pay[:, H + 1:H + 2], in_=s_loc[:])
            w_pay = nc.sync.dma_start(out=pay_own[:], in_=pay[:])
            ccp = nc.gpsimd.collective_compute(
                "AllGather", mybir.AluOpType.bypass, replica_groups=PAIRS,
                ins=[pay_own.opt()], outs=[pay_full.opt()])

            p0 = sp.tile([HEADS, H + 2], f32, tag="p0")
            p1 = sp.tile([HEADS, H + 2], f32, tag="p1")
            nc.sync.dma_start(out=p0[:], in_=pay_full[0])
            nc.sync.dma_start(out=p1[:], in_=pay_full[1])
            den = sp.tile([HEADS, 1], f32, tag="den")
            nc.vector.tensor_tensor(out=den[:], in0=p0[:, H:H + 1],
                                    in1=p1[:, H:H + 1], op=OP.add)
            rden = sp.tile([HEADS, 1], f32, tag="rden")
            nc.vector.reciprocal(out=rden[:], in_=den[:])
            ctxc = sp.tile([HEADS, H], f32, tag="ctxc")
            nc.vector.tensor_tensor(out=ctxc[:], in0=p0[:, 0:H],
                                    in1=p1[:, 0:H], op=OP.add)
            nc.vector.tensor_scalar(out=ctxc[:], in0=ctxc[:], scalar1=rden[:],
                                    scalar2=None, op0=OP.mult)
            ctxT2 = ps.tile([H, HEADS], f32, tag="sm1", space="PSUM")
            nc.tensor.transpose(ctxT2[:], in_=ctxc[:],
                                identity=ident[:HEADS, :HEADS])
            cmask = sp.tile([H, HEADS], f32, tag="cm")
            nc.sync.dma_start(out=cmask[:], in_=d_cmask[:])
            ctx_m = sp.tile([H, HEADS], f32, tag="ctxm")
            nc.vector.tensor_tensor(out=ctx_m[:], in0=ctxT2[:], in1=cmask[:],
                                    op=OP.mult)
            ctx_c = sp.tile([H, 1], f32, tag="ctxco")
            nc.vector.tensor_reduce(out=ctx_c[:], in_=ctx_m[:],
                                    axis=mybir.AxisListType.X, op=OP.add)
            bvc = sp.tile([H, 1], f32, tag="bvc")
            nc.sync.dma_start(out=bvc[:], in_=d_bv[:])
            nc.vector.tensor_tensor(out=ctx_c[:], in0=ctx_c[:], in1=bvc[:],
                                    op=OP.add)

            # g = layer_norm(a + ctx @ wo + bo)
            wo_t = sp.tile([H, H], f32, tag="w64e")
            nc.sync.dma_start(out=wo_t[:], in_=d_wo[:])
            go_ps = ps.tile([1, H], f32, tag="sm1", space="PSUM")
            nc.tensor.matmul(go_ps[:], lhsT=ctx_c[:], rhs=wo_t[:], start=True,
                             stop=True)
            bor = sp.tile([1, H], f32, tag="bor")
            nc.sync.dma_start(out=bor[:], in_=d_bo[:])
            g_row = sp.tile([1, H], f32, tag="grow")
            nc.vector.tensor_tensor(out=g_row[:], in0=go_ps[:], in1=bor[:],
                                    op=OP.add)
            nc.vector.tensor_tensor(out=g_row[:], in0=g_row[:], in1=a_row[:],
                                    op=OP.add)
            mu = sp.tile([1, 1], f32, tag="mu")
            nc.vector.tensor_reduce(out=mu[:], in_=g_row[:],
                                    axis=mybir.AxisListType.X, op=OP.add)
            nc.vector.tensor_scalar(out=mu[:], in0=mu[:], scalar1=1.0 / H,
                                    scalar2=None, op0=OP.mult)
            nc.vector.tensor_scalar(out=g_row[:], in0=g_row[:], scalar1=mu[:],
                                    scalar2=None, op0=OP.subtract)
            sq = sp.tile([1, H], f32, tag="sq")
            nc.scalar.activation(out=sq[:], in_=g_row[:], func=AF.Square)
            var = sp.tile([1, 1], f32, tag="var")
            nc.vector.tensor_reduce(out=var[:], in_=sq[:],
                                    axis=mybir.AxisListType.X, op=OP.add)
            std = sp.tile([1, 1], f32, tag="std")
            eps_t = sp.tile([1, 1], f32, tag="eps")
            nc.gpsimd.memset(eps_t[:], 1e-5)
            nc.scalar.activation(out=std[:], in_=var[:], func=AF.Sqrt,
                                 scale=1.0 / H, bias=eps_t[:])
            rstd = sp.tile([1, 1], f32, tag="rstd")
            nc.vector.reciprocal(out=rstd[:], in_=std[:])
            nc.vector.tensor_scalar(out=g_row[:], in0=g_row[:], scalar1=rstd[:],
                                    scalar2=None, op0=OP.mult)
            lng = sp.tile([1, H], f32, tag="lng")
            lnb = sp.tile([1, H], f32, tag="lnb")
            nc.sync.dma_start(out=lng[:], in_=d_lng[:])
            nc.sync.dma_start(out=lnb[:], in_=d_lnb[:])
            nc.vector.tensor_tensor(out=g_row[:], in0=g_row[:], in1=lng[:],
                                    op=OP.mult)
            nc.vector.tensor_tensor(out=g_row[:], in0=g_row[:], in1=lnb[:],
                                    op=OP.add)

            # logits = hT^T @ (g/8 + policy_w), masked (own half)
            pol = sp.tile([1, H], f32, tag="pol")
            nc.sync.dma_start(out=pol[:], in_=d_pol[:])
            nc.vector.tensor_scalar(out=g_row[:], in0=g_row[:], scalar1=1.0 / 8.0,
                                    scalar2=None, op0=OP.mult)
            nc.vector.tensor_tensor(out=g_row[:], in0=g_row[:], in1=pol[:],
                                    op=OP.add)
            wT_ps = ps.tile([H, 1], f32, tag="sm1", space="PSUM")
            nc.tensor.transpose(wT_ps[:], in_=g_row[:], identity=ident[:1, :1])
            w_col = sp.tile([H, 1], bf16, tag="wcol")
            nc.scalar.activation(out=w_col[:], in_=wT_ps[:], func=AF.Copy)
            lg_ps = ps.tile([128, NBUCK], f32, tag="sm1", space="PSUM")
            for t in range(NBUCK):
                nc.tensor.matmul(lg_ps[:, t:t + 1],
                                 lhsT=hT[:H, t * 128:(t + 1) * 128],
                                 rhs=w_col[:], start=True, stop=True)
            maskf = sp.tile([128, NBUCK], f32, tag="mf")
            maskn = sp.tile([128, NBUCK], f32, tag="mn")
            nc.sync.dma_start(out=maskf[:], in_=d_maskf[:])
            nc.sync.dma_start(out=maskn[:], in_=d_maskn[:])
            lg = sp.tile([128, NBUCK], f32, tag="lgsb")
            nc.vector.tensor_tensor(out=lg[:], in0=lg_ps[:], in1=maskf[:],
                                    op=OP.mult)
            nc.vector.tensor_tensor(out=lg[:], in0=lg[:], in1=maskn[:],
                                    op=OP.add)
            nc.sync.dma_start(out=d_out[:], in_=lg[:])

    nc.compile()
    return nc


def _wrap16(a):
    w = a.reshape(-1, 16).T
    return np.tile(w, (8, 1)).astype(np.int16)


def _prep_core(inputs, s, half):
    gn = np.asarray(inputs["graph_nodes"])
    links = np.asarray(inputs["graph_edge_links"])
    mask = np.asarray(inputs["mask"])

    x = np.zeros((NPAD, F_NODE), np.float32)
    x[:N_HALF] = gn[s, half * N_HALF:(half + 1) * N_HALF]
    xT = np.ascontiguousarray(x.T).astype(ml_dtypes.bfloat16)
    xf = np.zeros((2, NPAD, F_NODE), np.float32)
    xf[0, :N_HALF] = gn[s, :N_HALF]
    xf[1, :N_HALF] = gn[s, N_HALF:]
    xTF = np.ascontiguousarray(
        xf.reshape(NTAB, F_NODE).T).astype(ml_dtypes.bfloat16)

    src = links[s, 0].astype(np.int64)
    dst = links[s, 1].astype(np.int64)
    sel = (dst >= half * N_HALF) & (dst < (half + 1) * N_HALF)
    src_e = src[sel]
    dst_e = dst[sel]
    dl = dst_e - half * N_HALF                    # local 0..N_HALF
    psrc = src_e + (NPAD - N_HALF) * (src_e >= N_HALF)   # row in [0, NTAB)

    order = np.argsort(dl, kind="stable")
    dls = dl[order]
    pss = psrc[order]
    counts = np.bincount(dls, minlength=N_HALF)
    starts = np.zeros(N_HALF, np.int64)
    starts[1:] = np.cumsum(counts)[:-1]
    rank = np.arange(len(dls)) - starts[dls]

    mainsel = rank < K
    mn, mr, mp_ = dls[mainsel], rank[mainsel], pss[mainsel]
    idxM = np.full(MCOLS * 128, SENT1, np.int64)
    slot = ((mn >> 7) * K + mr) * 128 + (mn & 127)
    idxM[slot] = mp_

    on, op_ = dls[~mainsel], pss[~mainsel]
    ob = on >> 7
    ocounts = np.bincount(ob, minlength=NBUCK)
    if ocounts.max() > OVFCH * 128:
        raise RuntimeError(f"ovf overflow: {ocounts.max()} > {OVFCH * 128}")
    ostarts = np.zeros(NBUCK, np.int64)
    ostarts[1:] = np.cumsum(ocounts)[:-1]
    within = np.arange(len(on)) - ostarts[ob]
    oslot = ob * (OVFCH * 128) + within
    idxOs = np.full(OCOLS * 128, SENT1, np.int64)
    idxOd = np.full(OCOLS * 128, SENT2, np.int64)
    dlocv = np.full(OCOLS * 128, 128, np.float32)
    idxOs[oslot] = op_
    idxOd[oslot] = on
    dlocv[oslot] = (on & 127)

    blocks = []
    for (c0, og) in _ovf_groups():
        blocks.append(_wrap16(idxOs[c0 * 128:(c0 + og) * 128]))
        blocks.append(_wrap16(idxOd[c0 * 128:(c0 + og) * 128]))
    cmbO = np.ascontiguousarray(np.concatenate(blocks, axis=1))
    cmbM = _wrap16(idxM)
    dl_cols = dlocv.reshape(OCOLS, 128).T
    dlocO = np.ascontiguousarray(
        np.repeat(dl_cols, 2, axis=1)).astype(ml_dtypes.bfloat16)

    m = np.zeros(NPAD, bool)
    m[:N_HALF] = mask[s, half * N_HALF:(half + 1) * N_HALF]
    maskf = np.where(m, np.float32(1.0), np.float32(0.0))
    pb = np.float32(np.asarray(inputs["policy_b"]))
    maskn = np.where(m, pb, NEG)
    maskf = np.ascontiguousarray(maskf.reshape(NBUCK, 128).T)
    maskn = np.ascontiguousarray(maskn.reshape(NBUCK, 128).T)

    return {
        "xT": xT, "xTF": xTF, "cmbM": cmbM, "cmbO": cmbO, "dlocO": dlocO,
        "maskf": maskf, "maskneg": maskn,
        "ad_col": np.asarray(inputs["current_ad"])[s].reshape(F_AD, 1)
                    .astype(np.float32),
    }


def kernel(**inputs):
    from concourse.bass_utils import run_bass_kernel_spmd

    if "nc" not in _CACHE:
        _CACHE["nc"] = _build()
    nc = _CACHE["nc"]

    f = lambda k: np.ascontiguousarray(np.asarray(inputs[k], np.float32))
    bf = lambda a: np.ascontiguousarray(a).astype(ml_dtypes.bfloat16)
    iot = np.tile(np.arange(128, dtype=np.float32), (128, 1))
    blockmask = np.zeros((H, HEADS), np.float32)
    for hh in range(HEADS):
        blockmask[hh * DH:(hh + 1) * DH, hh] = 1.0

    msg_w = f("msg_w")
    upd_w = f("upd_w")
    common = {
        "iota128": iot.astype(ml_dtypes.bfloat16),
        "w_node16": bf(f("w_node")),
        "b_node_col": f("b_node").reshape(H, 1),
        "msgw1": bf(np.concatenate(
            [msg_w[:, :H, :].transpose(1, 0, 2),
             f("msg_b").reshape(1, L, H)], axis=0)),
        "msgw2": bf(msg_w[:, H:, :].transpose(1, 0, 2)),
        "bias_rep": np.tile(f("msg_b").reshape(1, L, H), (128, 1, 1)),
        "updw1": bf(upd_w[:, :H, :].transpose(1, 0, 2)),
        "updw2": bf(upd_w[:, H:, :].transpose(1, 0, 2)),
        "upd_b_col": np.ascontiguousarray(f("upd_b").T),
        "w_ad": f("w_ad"), "b_ad_row": f("b_ad").reshape(1, H),
        "wq": f("wq"), "bq_row": f("bq").reshape(1, H),
        "wk16": bf(f("wk")), "wv16": bf(f("wv")),
        "bv_col": f("bv").reshape(H, 1),
        "wo": f("wo"), "bo_row": f("bo").reshape(1, H),
        "ln_g_row": f("ln_g").reshape(1, H), "ln_b_row": f("ln_b").reshape(1, H),
        "qmask": blockmask * np.float32(1.0 / np.sqrt(DH)),
        "cmask": blockmask,
        "pol_row": f("policy_w").reshape(1, H),
    }

    in_maps = []
    for c in range(NCORES):
        m = dict(common)
        m.update(_prep_core(inputs, c // 2, c % 2))
        in_maps.append(m)

    res = run_bass_kernel_spmd(nc, in_maps, core_ids=list(range(NCORES)))
    _CACHE["last_results"] = res

    out = np.empty((B, N), np.float32)
    for c in range(NCORES):
        s, half = c // 2, c % 2
        lg = np.asarray(res.results[c]["logits"])      # [128, NBUCK]
        flat = lg.T.reshape(NPAD)
        out[s, half * N_HALF:(half + 1) * N_HALF] = flat[:N_HALF]
    return out



# revision 24
# speedup vs baseline: 1.1731x; 1.0336x over previous
"""Trainium2 Bass kernel for BillboardAllocatorGNN.

Sharding: 8 cores; core c handles sample c//2, node-half c%2 (data parallel
over batch, dst-parallel within each sample pair).

Edge phase (per layer): node-major edge slots with fixed per-node capacity
K=18 kill both the dst-side gather and the one-hot scatter for 96% of edges:
a single SWDGE gather fetches duplicated-bf16 hs1 rows (256B descriptors)
into [dst-node-partition, slot] layout, the dst-side hs2 contribution is a
free-dim broadcast add from SBUF, and segment-sum is a strided free-axis
tensor_reduce. Overflow edges (deg>K) go through a small one-hot matmul
side path whose PSUM accumulation group also hosts the main agg transpose.
Pad slots gather a -30000 sentinel row so relu zeroes them.

Pipelining: updates, next-layer table builds, and attention score/value
chunks are interleaved into the main gather stream. Layer 0 builds its
table locally from raw features (no collective); layer 1 exchanges hs1
in slices issued as updates complete; the final attention merges across
the pair via a 2KB unnormalized-softmax stats exchange (scores are O(1),
so no max subtraction is needed).
"""
import sys
import os

sys.path.insert(0, "/opt/trn_rl_repo")

import numpy as np
import ml_dtypes

# ---- problem dims (hardcoded per spec) ----
B, N, E = 4, 20000, 320000
F_NODE, F_AD = 16, 8
H, L, HEADS = 64, 2, 4
DH = H // HEADS

NCORES = 8
N_HALF = N // 2                 # 10000 real nodes per core
NBUCK = 79                      # 128-node pages per core
NPAD = NBUCK * 128              # 10112 padded nodes per core
K = 18                          # main slots per node
MCOLS = NBUCK * K               # 1422 main gather columns
PPG = 2                         # pages per main gather group
NGRP = (NBUCK + PPG - 1) // PPG  # 27
OVFCH = 2                       # overflow chunks per page (cap 256 edges)
OCOLS = NBUCK * OVFCH           # 158 overflow columns
OG = 12                         # overflow chunks per gather group
NTAB = 2 * NPAD                 # hs1 rows (both halves)
SENT1 = NTAB                    # sentinel row in hs1 table
SENT2 = NPAD                    # sentinel row in hs2 table (local ids)
CH = 512                        # node-chunk for encoder/update/attention
NEG = np.float32(-1e9)

_CACHE = {}


def _ovf_groups():
    out = []
    c = 0
    while c < OCOLS:
        g = min(OG, OCOLS - c)
        out.append((c, g))
        c += g
    return out


def _main_groups():
    out = []
    b = 0
    while b < NBUCK:
        g = min(PPG, NBUCK - b)
        out.append((b, g))
        b += g
    return out


def _build():
    import concourse.mybir as mybir
    import concourse.tile as tile
    import concourse.bacc as bacc
    from concourse.tile import add_dep_helper
    from concourse.masks import make_identity

    f32 = mybir.dt.float32
    bf16 = mybir.dt.bfloat16
    i16 = mybir.dt.int16
    AF = mybir.ActivationFunctionType
    OP = mybir.AluOpType

    nc = bacc.Bacc("TRN2", target_bir_lowering=False, debug=False,
                   num_swdge_queues=2)

    # ---- I/O ----
    d_xT = nc.dram_tensor("xT", [F_NODE, NPAD], bf16, kind="ExternalInput")
    d_xTF = nc.dram_tensor("xTF", [F_NODE, NTAB], bf16, kind="ExternalInput")
    d_cmbM = nc.dram_tensor("cmbM", [128, MCOLS * 8], i16, kind="ExternalInput")
    d_cmbO = nc.dram_tensor("cmbO", [128, 2 * OCOLS * 8], i16,
                            kind="ExternalInput")
    d_dlocO = nc.dram_tensor("dlocO", [128, 2 * OCOLS], bf16,
                             kind="ExternalInput")
    d_iota = nc.dram_tensor("iota128", [128, 128], bf16, kind="ExternalInput")
    d_ones = nc.dram_tensor("ones_row", [1, NPAD], bf16, kind="ExternalInput")
    d_maskf = nc.dram_tensor("maskf", [128, NBUCK], f32, kind="ExternalInput")
    d_maskn = nc.dram_tensor("maskneg", [128, NBUCK], f32, kind="ExternalInput")
    d_wnode = nc.dram_tensor("w_node16", [F_NODE, H], bf16,
                             kind="ExternalInput")
    d_bnode = nc.dram_tensor("b_node_col", [H, 1], f32, kind="ExternalInput")
    d_msgw1 = nc.dram_tensor("msgw1", [H + 1, L, H], bf16,
                             kind="ExternalInput")
    d_msgw2 = nc.dram_tensor("msgw2", [H, L, H], bf16, kind="ExternalInput")
    d_brep = nc.dram_tensor("bias_rep", [128, L, H], f32, kind="ExternalInput")
    d_updw1 = nc.dram_tensor("updw1", [H, L, H], bf16, kind="ExternalInput")
    d_updw2 = nc.dram_tensor("updw2", [H, L, H], bf16, kind="ExternalInput")
    d_updb = nc.dram_tensor("upd_b_col", [H, L], f32, kind="ExternalInput")
    d_wad = nc.dram_tensor("w_ad", [F_AD, H], f32, kind="ExternalInput")
    d_bad = nc.dram_tensor("b_ad_row", [1, H], f32, kind="ExternalInput")
    d_ad = nc.dram_tensor("ad_col", [F_AD, 1], f32, kind="ExternalInput")
    d_wq = nc.dram_tensor("wq", [H, H], f32, kind="ExternalInput")
    d_bq = nc.dram_tensor("bq_row", [1, H], f32, kind="ExternalInput")
    d_wk16 = nc.dram_tensor("wk16", [H, H], bf16, kind="ExternalInput")
    d_wv16 = nc.dram_tensor("wv16", [H, H], bf16, kind="ExternalInput")
    d_bv = nc.dram_tensor("bv_col", [H, 1], f32, kind="ExternalInput")
    d_wo = nc.dram_tensor("wo", [H, H], f32, kind="ExternalInput")
    d_bo = nc.dram_tensor("bo_row", [1, H], f32, kind="ExternalInput")
    d_lng = nc.dram_tensor("ln_g_row", [1, H], f32, kind="ExternalInput")
    d_lnb = nc.dram_tensor("ln_b_row", [1, H], f32, kind="ExternalInput")
    d_qmask = nc.dram_tensor("qmask", [H, HEADS], f32, kind="ExternalInput")
    d_cmask = nc.dram_tensor("cmask", [H, HEADS], f32, kind="ExternalInput")
    d_pol = nc.dram_tensor("pol_row", [1, H], f32, kind="ExternalInput")
    d_out = nc.dram_tensor("logits", [128, NBUCK], f32, kind="ExternalOutput")

    PAIRS = [[0, 1], [2, 3], [4, 5], [6, 7]]
    n_ch = (NPAD + CH - 1) // CH
    TB = 8                      # node pages per table-export batch

    with tile.TileContext(nc) as tc:
        with (
            tc.tile_pool(name="persist", bufs=1) as pp,
            tc.tile_pool(name="mp", bufs=2) as mp,
            tc.tile_pool(name="g3", bufs=4) as g3,
            tc.tile_pool(name="gx", bufs=2) as gx,
            tc.tile_pool(name="single", bufs=1) as sp,
            tc.tile_pool(name="psum", bufs=2, space="PSUM") as ps,
            tc.tile_pool(name="dram", bufs=1, space="DRAM") as dp,
        ):
            # ---- persistent state / constants ----
            hT = pp.tile([H + 1, NPAD], bf16)      # node states + ones row
            aggT = pp.tile([H, NPAD], bf16)        # per-layer aggregate
            hs2 = pp.tile([128, NBUCK, H], bf16)   # dst table, node-major
            ovfmsg = pp.tile([128, OCOLS, H], bf16)
            cmbO = pp.tile([128, 2 * OCOLS * 8], i16)
            dlocO = pp.tile([128, 2 * OCOLS], bf16)
            iota = pp.tile([128, 128], bf16)
            ident = pp.tile([128, 128], f32)
            wnode = pp.tile([F_NODE, H], bf16)
            bnode = pp.tile([H, 1], f32)
            msgw1 = pp.tile([H + 1, L, H], bf16)
            msgw2 = pp.tile([H, L, H], bf16)
            updw1 = pp.tile([H, L, H], bf16)
            updw2 = pp.tile([H, L, H], bf16)
            updb = pp.tile([H, L], f32)

            make_identity(nc, ident[:])
            nc.sync.dma_start(out=hT[H:H + 1, :], in_=d_ones[:])
            ident_bf4 = pp.tile([HEADS, HEADS], bf16)
            nc.vector.tensor_copy(out=ident_bf4[:], in_=ident[:HEADS, :HEADS])

            for dst_t, src_t in [
                (cmbO, d_cmbO), (dlocO, d_dlocO),
                (iota, d_iota), (wnode, d_wnode), (bnode, d_bnode),
                (msgw1, d_msgw1), (msgw2, d_msgw2),
                (updw1, d_updw1), (updw2, d_updw2), (updb, d_updb),
            ]:
                nc.sync.dma_start(out=dst_t[:], in_=src_t[:])

            # ---- DRAM: gather tables, hs1 exchange, attention payload ----
            # Separate hs1/hs2 tables so the tile framework's DRAM dep
            # tracking doesn't serialize layer-l hs1 gathers behind the
            # layer-(l+1) hs2 rewrites issued mid-stream.
            tab = dp.tile([NTAB + 1, 128], bf16)     # hs1 rows + sentinel
            tab2 = dp.tile([NPAD + 1, 128], bf16)    # hs2 rows + sentinel
            hs1own = dp.tile([NPAD, H], bf16)
            hs1pair = dp.tile([NTAB, H], bf16)
            pay_own = dp.tile([HEADS, H + 2], f32)
            pay_full = dp.tile([2, HEADS, H + 2], f32)

            # sentinel rows (once; hs1/hs2 rewrites never touch them)
            sentc = sp.tile([1, 128], bf16, tag="sent")
            nc.gpsimd.memset(sentc[:], -30000.0)
            w_sent = nc.sync.dma_start(out=tab[SENT1:SENT1 + 1, :],
                                       in_=sentc[:])
            w_sent2 = nc.sync.dma_start(out=tab2[SENT2:SENT2 + 1, :],
                                        in_=sentc[:])

            # ---- node encoder: hT = relu(w_node^T @ xT + b) ----
            for i in range(n_ch):
                lo = i * CH
                sz = min(CH, NPAD - lo)
                xt = mp.tile([F_NODE, CH], bf16, tag="xtf")
                nc.sync.dma_start(out=xt[:, :sz], in_=d_xT[:, lo:lo + sz])
                h_ps = ps.tile([H, CH], f32, tag="big", space="PSUM")
                nc.tensor.matmul(h_ps[:, :sz], lhsT=wnode[:], rhs=xt[:, :sz],
                                 start=True, stop=True)
                nc.vector.tensor_scalar(out=hT[:H, lo:lo + sz],
                                        in0=h_ps[:, :sz], scalar1=bnode[:],
                                        scalar2=0.0, op0=OP.add, op1=OP.max)

            # ---- attention prelude: q from ad (independent of layers) ----
            wad = sp.tile([F_AD, H], f32, tag="w64")
            adc = sp.tile([F_AD, 1], f32, tag="col")
            nc.sync.dma_start(out=wad[:], in_=d_wad[:])
            nc.sync.dma_start(out=adc[:], in_=d_ad[:])
            a_ps = ps.tile([1, H], f32, tag="sm1", space="PSUM")
            nc.tensor.matmul(a_ps[:], lhsT=adc[:], rhs=wad[:], start=True,
                             stop=True)
            bad = sp.tile([1, H], f32, tag="row1")
            nc.sync.dma_start(out=bad[:], in_=d_bad[:])
            a_row = sp.tile([1, H], f32, tag="arow")
            nc.vector.tensor_tensor(out=a_row[:], in0=a_ps[:], in1=bad[:],
                                    op=OP.add)
            nc.vector.tensor_scalar(out=a_row[:], in0=a_row[:], scalar1=0.0,
                                    scalar2=None, op0=OP.max)
            aT_ps = ps.tile([H, 1], f32, tag="sm1", space="PSUM")
            nc.tensor.transpose(aT_ps[:], in_=a_row[:], identity=ident[:1, :1])
            a_col = sp.tile([H, 1], f32, tag="acol")
            nc.scalar.activation(out=a_col[:], in_=aT_ps[:], func=AF.Copy)
            wq_t = sp.tile([H, H], f32, tag="w64b")
            nc.sync.dma_start(out=wq_t[:], in_=d_wq[:])
            q_ps = ps.tile([1, H], f32, tag="sm1", space="PSUM")
            nc.tensor.matmul(q_ps[:], lhsT=a_col[:], rhs=wq_t[:], start=True,
                             stop=True)
            bqr = sp.tile([1, H], f32, tag="row2")
            nc.sync.dma_start(out=bqr[:], in_=d_bq[:])
            q_row = sp.tile([1, H], f32, tag="qrow")
            nc.vector.tensor_tensor(out=q_row[:], in0=q_ps[:], in1=bqr[:],
                                    op=OP.add)
            qT_ps = ps.tile([H, 1], f32, tag="sm1", space="PSUM")
            nc.tensor.transpose(qT_ps[:], in_=q_row[:], identity=ident[:1, :1])
            qmask = sp.tile([H, HEADS], f32, tag="qm")
            nc.sync.dma_start(out=qmask[:], in_=d_qmask[:])
            q_col = sp.tile([H, 1], f32, tag="qcol")
            nc.scalar.activation(out=q_col[:], in_=qT_ps[:], func=AF.Copy)
            qblk = sp.tile([H, HEADS], bf16, tag="qblk")
            nc.vector.tensor_tensor(out=qblk[:],
                                    in0=q_col[:].to_broadcast([H, HEADS]),
                                    in1=qmask[:], op=OP.mult)
            wk16 = sp.tile([H, H], bf16, tag="w64c")
            wv16 = sp.tile([H, H], bf16, tag="w64d")
            nc.sync.dma_start(out=wk16[:], in_=d_wk16[:])
            nc.sync.dma_start(out=wv16[:], in_=d_wv16[:])

            # attention state (filled during the last layer's main stream)
            scores = pp.tile([HEADS, NPAD], bf16)    # exp(raw), unnormalized
            vall = pp.tile([128, NBUCK, H], bf16)
            sm = sp.tile([HEADS, n_ch], f32, tag="sm")

            # ---- message-passing layers, software-pipelined ----
            SL = [(0, 48), (48, 72), (72, NBUCK)]    # cc slices (pages)
            hs1pairs = []
            for _si, (p0, p1) in enumerate(SL):
                hs1pair_s = dp.tile([2 * 128 * (p1 - p0), H], bf16,
                                    name=f"hs1pair_{_si}")
                hs1pairs.append(hs1pair_s)
            hs2t = []
            for _li in range(L):
                hs2_l = pp.tile([128, NBUCK, H], bf16, name=f"hs2_{_li}")
                hs2t.append(hs2_l)

            gathers = [[] for _ in range(L)]         # per-layer gather instrs
            hs2_writes = [[] for _ in range(L)]
            ccs = [[] for _ in range(L)]
            expands = [[] for _ in range(L)]

            def hs2_batch(l, t0):
                tn = min(TB, NBUCK - t0)
                p2 = ps.tile([128, TB * H], f32, tag="tab8", space="PSUM")
                for j in range(tn):
                    nc.tensor.matmul(
                        p2[:, j * H:(j + 1) * H],
                        lhsT=hT[:H, (t0 + j) * 128:(t0 + j + 1) * 128],
                        rhs=msgw2[:, l, :], start=True, stop=True)
                nc.scalar.activation(
                    out=hs2t[l][:, t0:t0 + tn, :],
                    in_=p2[:, :tn * H].rearrange("p (t f) -> p t f", f=H),
                    func=AF.Copy)
                dst0 = tab2[t0 * 128:(t0 + tn) * 128,
                            0:H].rearrange("(t p) f -> p t f", p=128)
                w1 = nc.sync.dma_start(out=dst0, in_=hs2t[l][:, t0:t0 + tn, :])
                if l > 0:
                    # WAR: previous layer's overflow-dst gathers read this
                    nogrp = len(_ovf_groups())
                    for gi in gathers[l - 1][:nogrp]:
                        add_dep_helper(w1.ins, gi.ins,
                                       reason="WAR: hs2 rewrite after gathers")
                hs2_writes[l].append(w1)

            def hs1_batch(l, t0):
                tn = min(TB, NBUCK - t0)
                p1 = ps.tile([128, TB * H], f32, tag="tab8", space="PSUM")
                for j in range(tn):
                    nc.tensor.matmul(
                        p1[:, j * H:(j + 1) * H],
                        lhsT=hT[:, (t0 + j) * 128:(t0 + j + 1) * 128],
                        rhs=msgw1[:, l, :], start=True, stop=True)
                st = mp.tile([128, TB, H], bf16, tag="st")
                nc.scalar.activation(
                    out=st[:, :tn, :],
                    in_=p1[:, :tn * H].rearrange("p (t f) -> p t f", f=H),
                    func=AF.Copy)
                nc.sync.dma_start(
                    out=hs1own[t0 * 128:(t0 + tn) * 128, :].rearrange(
                        "(t p) f -> p t f", p=128),
                    in_=st[:, :tn, :])

            def issue_cc(l, s):
                p0, p1 = SL[s]
                cc0 = nc.gpsimd.collective_compute(
                    "AllGather", mybir.AluOpType.bypass, replica_groups=PAIRS,
                    ins=[hs1own[p0 * 128:p1 * 128, :]],
                    outs=[hs1pairs[s].opt()])
                if l > 0:
                    for e in expands[l - 1]:
                        add_dep_helper(cc0.ins, e.ins,
                                       reason="WAR: pair rewrite after expand")
                ccs[l].append(cc0)

            def expand_tab(l):
                # copy exchanged hs1 slices into tab hs1 region (cols 0:64)
                for s, (p0, p1) in enumerate(SL):
                    n_s = 128 * (p1 - p0)
                    cc0 = ccs[l][s]
                    for hf in range(2):
                        base = hf * NPAD + p0 * 128
                        e = nc.sync.dma_start(
                            out=tab[base:base + n_s, 0:H].rearrange(
                                "(t p) f -> p t f", p=128),
                            in_=hs1pairs[s][hf * n_s:(hf + 1) * n_s,
                                            :].rearrange(
                                "(t p) f -> p t f", p=128))
                        add_dep_helper(e.ins, cc0.ins,
                                       reason="RAW: expand after allgather")
                        if l > 0:
                            for gi in gathers[l - 1]:
                                add_dep_helper(
                                    e.ins, gi.ins,
                                    reason="WAR: tab rewrite after gathers")
                        expands[l].append(e)

            def update_chunk(l, c):
                lo = c * CH
                sz = min(CH, NPAD - lo)
                u_ps = ps.tile([H, CH], f32, tag="big", space="PSUM")
                nc.tensor.matmul(u_ps[:, :sz], lhsT=updw1[:, l, :],
                                 rhs=hT[:H, lo:lo + sz], start=True, stop=False)
                nc.tensor.matmul(u_ps[:, :sz], lhsT=updw2[:, l, :],
                                 rhs=aggT[:, lo:lo + sz], start=False,
                                 stop=True)
                un = mp.tile([H, CH], bf16, tag="un")
                nc.scalar.activation(out=un[:, :sz], in_=u_ps[:, :sz],
                                     func=AF.Relu, bias=updb[:, l:l + 1])
                nc.vector.tensor_tensor(out=hT[:H, lo:lo + sz],
                                        in0=hT[:H, lo:lo + sz],
                                        in1=un[:, :sz], op=OP.add)

            def att_chunk(c):
                lo = c * CH
                sz = min(CH, NPAD - lo)
                nt = sz // 128
                kT_ps = ps.tile([H, CH], f32, tag="big", space="PSUM")
                nc.tensor.matmul(kT_ps[:, :sz], lhsT=wk16[:],
                                 rhs=hT[:H, lo:lo + sz], start=True, stop=True)
                kT_sb = mp.tile([H, CH], bf16, tag="kT")
                nc.scalar.activation(out=kT_sb[:, :sz], in_=kT_ps[:, :sz],
                                     func=AF.Copy)
                s_ps = ps.tile([HEADS, CH], f32, tag="sm1", space="PSUM")
                nc.tensor.matmul(s_ps[:, :sz], lhsT=qblk[:], rhs=kT_sb[:, :sz],
                                 start=True, stop=True)
                nc.scalar.activation(out=scores[:, lo:lo + sz],
                                     in_=s_ps[:, :sz], func=AF.Exp)
                if lo + sz > N_HALF:
                    nc.gpsimd.memset(scores[:, N_HALF:], 0.0)
                nc.vector.tensor_reduce(out=sm[:, c:c + 1],
                                        in_=scores[:, lo:lo + sz],
                                        axis=mybir.AxisListType.X, op=OP.add)
                v_ps = ps.tile([128, 4 * H], f32, tag="tab8", space="PSUM")
                for j in range(nt):
                    nc.tensor.matmul(
                        v_ps[:, j * H:(j + 1) * H],
                        lhsT=hT[:H, lo + j * 128:lo + (j + 1) * 128],
                        rhs=wv16[:], start=True, stop=True)
                nc.scalar.activation(
                    out=vall[:, lo // 128:lo // 128 + nt, :],
                    in_=v_ps[:, :nt * H].rearrange("p (t f) -> p t f", f=H),
                    func=AF.Copy)

            def post_update(l, c):
                if l + 1 < L:
                    if c >= 2 and c % 2 == 0:
                        hs2_batch(l + 1, 4 * (c - 2))
                        hs1_batch(l + 1, 4 * (c - 2))
                    if c == 13:
                        issue_cc(l + 1, 0)
                    elif c == 19:
                        issue_cc(l + 1, 1)
                else:
                    att_chunk(c)

            def gdeps(l, gi, writes, after=()):
                add_dep_helper(gi.ins, w_sent.ins, reason="RAW: sentinel")
                add_dep_helper(gi.ins, w_sent2.ins, reason="RAW: sentinel2")
                for w in writes:
                    add_dep_helper(gi.ins, w.ins, reason="RAW: tab write")
                for cx in after:
                    add_dep_helper(gi.ins, cx.ins, reason="RAW: tab ready")
                gathers[l].append(gi)

            def ovf_dst_stream(l):
                for (c0, og) in _ovf_groups():
                    goutO = mp.tile([128, OG, 128], bf16, tag="goutO")
                    nidx = og * 128
                    gi = nc.gpsimd.dma_gather(
                        out_ap=goutO[:, :og, :], in_ap=tab2[:],
                        idxs_ap=cmbO[:, (2 * c0 + og) * 8:2 * (c0 + og) * 8],
                        num_idxs=nidx, num_idxs_reg=nidx, elem_size=128,
                        queue_num=0, single_packet=False)
                    gdeps(l, gi, hs2_writes[l])
                    nc.vector.tensor_copy(out=ovfmsg[:, c0:c0 + og, :],
                                          in_=goutO[:, 0:og, 0:H])

            def main_phase(l):
                # overflow src rows + messages
                for (c0, og) in _ovf_groups():
                    goutO = mp.tile([128, OG, 128], bf16, tag="goutO")
                    nidx = og * 128
                    gi = nc.gpsimd.dma_gather(
                        out_ap=goutO[:, :og, :], in_ap=tab[:],
                        idxs_ap=cmbO[:, 2 * c0 * 8:(2 * c0 + og) * 8],
                        num_idxs=nidx, num_idxs_reg=nidx, elem_size=128,
                        queue_num=0, single_packet=False)
                    gdeps(l, gi, expands[l])
                    nc.vector.tensor_tensor(
                        out=ovfmsg[:, c0:c0 + og, :],
                        in0=ovfmsg[:, c0:c0 + og, :],
                        in1=goutO[:, 0:og, 0:H], op=OP.add)
                    nc.scalar.activation(out=ovfmsg[:, c0:c0 + og, :],
                                         in_=ovfmsg[:, c0:c0 + og, :],
                                         func=AF.Relu)

                next_up = 0

                PFB = 4             # gather groups per idx-prefetch copy
                pf = {}

                def issue_gather(gidx, b0, gp):
                    cols = gp * K
                    gout = g3.tile([128, PPG * K, 128], bf16, tag="gout")
                    if gidx % PFB == 0:
                        npg = min(PFB * PPG, NBUCK - b0)
                        t = gx.tile([128, PFB * PPG * K * 8], i16, tag="gidx")
                        nc.sync.dma_start(
                            out=t[:, :npg * K * 8],
                            in_=d_cmbM[:, b0 * K * 8:(b0 + npg) * K * 8])
                        pf[gidx // PFB] = (t, b0)
                    t, pb0 = pf[gidx // PFB]
                    off = (b0 - pb0) * K * 8
                    gi = nc.gpsimd.dma_gather(
                        out_ap=gout[:, :cols, :], in_ap=tab[:],
                        idxs_ap=t[:, off:off + cols * 8],
                        num_idxs=cols * 128, num_idxs_reg=cols * 128,
                        elem_size=128, queue_num=gidx % 2,
                        single_packet=False)
                    gdeps(l, gi, (), after=expands[l])
                    return gout

                def consume_group(b0, gp, gout):
                    nonlocal next_up
                    msg = mp.tile([128, PPG * K, H], bf16, tag="msg")
                    agg = mp.tile([128, PPG, H], f32, tag="agg")
                    p_sc = ps.tile([H, PPG * 128], f32, tag="scat",
                                   space="PSUM")
                    ohpg = mp.tile([128, PPG * OVFCH, 128], bf16, tag="ohpg")
                    dl4 = dlocO[:, 2 * b0 * OVFCH:2 * (b0 + gp) * OVFCH
                                ].rearrange("p (g two) -> p g two", two=2)[
                        :, :, None, :].to_broadcast([128, gp * OVFCH, 64, 2])
                    io4 = iota[:].rearrange("p (s two) -> p s two", two=2)[
                        :, None, :, :].to_broadcast([128, gp * OVFCH, 64, 2])
                    oh4 = ohpg[:, :gp * OVFCH, :].rearrange(
                        "p g (s two) -> p g s two", two=2)
                    nc.vector.tensor_tensor(out=oh4, in0=dl4, in1=io4,
                                            op=OP.is_equal)
                    for j in range(gp):
                        nc.vector.tensor_tensor(
                            out=msg[:, j * K:(j + 1) * K, :],
                            in0=gout[:, j * K:(j + 1) * K, 0:H],
                            in1=hs2t[l][:, b0 + j:b0 + j + 1, :].to_broadcast(
                                [128, K, H]), op=OP.add)
                        nc.vector.tensor_scalar(
                            out=msg[:, j * K:(j + 1) * K, :],
                            in0=msg[:, j * K:(j + 1) * K, :],
                            scalar1=0.0, scalar2=None, op0=OP.max)
                    for j in range(gp):
                        pg = b0 + j
                        nc.vector.tensor_reduce(
                            out=agg[:, j, :],
                            in_=msg[:, j * K:(j + 1) * K, :].rearrange(
                                "p j f -> p f j"),
                            axis=mybir.AxisListType.X, op=OP.add)
                        sl = p_sc[:, j * 128:(j + 1) * 128]
                        nc.tensor.matmul(sl, lhsT=agg[:, j, :], rhs=ident[:],
                                         is_transpose=True, start=True,
                                         stop=False)
                        for k in range(OVFCH):
                            cc_i = pg * OVFCH + k
                            nc.tensor.matmul(
                                sl, lhsT=ovfmsg[:, cc_i, :],
                                rhs=ohpg[:, j * OVFCH + k, :],
                                start=False, stop=(k == OVFCH - 1))
                    nc.scalar.activation(
                        out=aggT[:, b0 * 128:(b0 + gp) * 128],
                        in_=p_sc[:, :gp * 128], func=AF.Copy)
                    while (next_up < n_ch
                           and (4 * next_up + 4) * 128 <= (b0 + gp) * 128):
                        update_chunk(l, next_up)
                        post_update(l, next_up)
                        next_up += 1

                AHEAD = 4
                pend = []
                for gidx, (b0, gp) in enumerate(_main_groups()):
                    pend.append((b0, gp, issue_gather(gidx, b0, gp)))
                    if len(pend) > AHEAD:
                        consume_group(*pend.pop(0))
                for item in pend:
                    consume_group(*item)
                while next_up < n_ch:
                    update_chunk(l, next_up)
                    post_update(l, next_up)
                    next_up += 1

            # layer 0 tables: h0 = encoder(x) is computable locally for
            # BOTH halves from the raw features -- no collective needed
            for t0 in range(0, NBUCK, TB):
                hs2_batch(0, t0)
            for gt0 in range(0, 2 * NBUCK, TB):
                tn = min(TB, 2 * NBUCK - gt0)
                xtf = mp.tile([F_NODE, TB * 128], bf16, tag="xtf")
                nc.sync.dma_start(out=xtf[:, :tn * 128],
                                  in_=d_xTF[:, gt0 * 128:(gt0 + tn) * 128])
                htmp = mp.tile([H + 1, TB * 128], bf16, tag="htmp")
                if gt0 < 2 * TB:
                    # ones row survives in the 2 rotating bufs afterwards
                    nc.gpsimd.memset(htmp[H:H + 1, :], 1.0)
                for h0 in range(0, tn * 128, CH):
                    hsz = min(CH, tn * 128 - h0)
                    hf_ps = ps.tile([H, CH], f32, tag="big", space="PSUM")
                    nc.tensor.matmul(hf_ps[:, :hsz], lhsT=wnode[:],
                                     rhs=xtf[:, h0:h0 + hsz], start=True,
                                     stop=True)
                    nc.vector.tensor_scalar(out=htmp[:H, h0:h0 + hsz],
                                            in0=hf_ps[:, :hsz],
                                            scalar1=bnode[:], scalar2=0.0,
                                            op0=OP.add, op1=OP.max)
                p1g = ps.tile([128, TB * H], f32, tag="tab8", space="PSUM")
                for j in range(tn):
                    nc.tensor.matmul(
                        p1g[:, j * H:(j + 1) * H],
                        lhsT=htmp[:, j * 128:(j + 1) * 128],
                        rhs=msgw1[:, 0, :], start=True, stop=True)
                stg = mp.tile([128, TB, H], bf16, tag="st")
                nc.scalar.activation(
                    out=stg[:, :tn, :],
                    in_=p1g[:, :tn * H].rearrange("p (t f) -> p t f", f=H),
                    func=AF.Copy)
                e0 = nc.sync.dma_start(
                    out=tab[gt0 * 128:(gt0 + tn) * 128, 0:H].rearrange(
                        "(t p) f -> p t f", p=128),
                    in_=stg[:, :tn, :])
                expands[0].append(e0)
            ovf_dst_stream(0)
            main_phase(0)

            # layer 1: tables/cc mostly issued inside layer 0's stream
            hs2_batch(1, 72)
            hs1_batch(1, 72)
            issue_cc(1, 2)
            expand_tab(1)
            ovf_dst_stream(1)
            main_phase(1)

            # ---- attention tail: sums, ctx, pair merge ----
            s_loc = sp.tile([HEADS, 1], f32, tag="m3")
            nc.vector.tensor_reduce(out=s_loc[:], in_=sm[:],
                                    axis=mybir.AxisListType.X, op=OP.add)
            ctx_ps = ps.tile([H, HEADS], f32, tag="tab8", space="PSUM")
            for i in range(0, NBUCK, 4):
                nt = min(4, NBUCK - i)
                at_ps = ps.tile([128, 4 * HEADS], bf16, tag="sm1",
                                space="PSUM")
                for j in range(nt):
                    nc.tensor.transpose(
                        at_ps[:, j * HEADS:(j + 1) * HEADS],
                        in_=scores[:, (i + j) * 128:(i + j + 1) * 128],
                        identity=ident_bf4[:])
                at_sb = mp.tile([128, 4 * HEADS], bf16, tag="atsb")
                nc.scalar.activation(out=at_sb[:, :nt * HEADS],
                                     in_=at_ps[:, :nt * HEADS], func=AF.Copy)
                for j in range(nt):
                    t = i + j
                    nc.tensor.matmul(
                        ctx_ps[:], lhsT=vall[:, t, :],
                        rhs=at_sb[:, j * HEADS:(j + 1) * HEADS],
                        start=(t == 0), stop=(t == NBUCK - 1))

            ctx_sb = sp.tile([H, HEADS], f32, tag="ctxsb")
            nc.scalar.activation(out=ctx_sb[:], in_=ctx_ps[:], func=AF.Copy)
            ctxT_ps = ps.tile([HEADS, H], f32, tag="sm1", space="PSUM")
            nc.tensor.transpose(ctxT_ps[:], in_=ctx_sb[:],
                                identity=ident[:H, :H])
            pay = sp.tile([HEADS, H + 2], f32, tag="pay")
            nc.scalar.activation(out=pay[:, 0:H], in_=ctxT_ps[:], func=AF.Copy)
            nc.vector.tensor_copy(out=pay[:, H:H + 1], in_=s_loc[:])
            nc.vector.tensor_copy(out=pay[:, H + 1:H + 2], in_=s_loc[:])
            w_pay = nc.sync.dma_start(out=pay_own[:], in_=pay[:])
            ccp = nc.gpsimd.collective_compute(
                "AllGather", mybir.AluOpType.bypass, replica_groups=PAIRS,
                ins=[pay_own.opt()], outs=[pay_full.opt()])

            p0 = sp.tile([HEADS, H + 2], f32, tag="p0")
            p1 = sp.tile([HEADS, H + 2], f32, tag="p1")
            nc.sync.dma_start(out=p0[:], in_=pay_full[0])
            nc.sync.dma_start(out=p1[:], in_=pay_full[1])
            den = sp.tile([HEADS, 1], f32, tag="den")
            nc.vector.tensor_tensor(out=den[:], in0=p0[:, H:H + 1],
                                    in1=p1[:, H:H + 1], op=OP.add)
            rden = sp.tile([HEADS, 1], f32, tag="rden")
            nc.vector.reciprocal(out=rden[:], in_=den[:])
            ctxc = sp.tile([HEADS, H], f32, tag="ctxc")
            nc.vector.tensor_tensor(out=ctxc[:], in0=p0[:, 0:H],
                                    in1=p1[:, 0:H], op=OP.add)
            nc.vector.tensor_scalar(out=ctxc[:], in0=ctxc[:], scalar1=rden[:],
                                    scalar2=None, op0=OP.mult)
            ctxT2 = ps.tile([H, HEADS], f32, tag="sm1", space="PSUM")
            nc.tensor.transpose(ctxT2[:], in_=ctxc[:],
                                identity=ident[:HEADS, :HEADS])
            cmask = sp.tile([H, HEADS], f32, tag="cm")
            nc.sync.dma_start(out=cmask[:], in_=d_cmask[:])
            ctx_m = sp.tile([H, HEADS], f32, tag="ctxm")
            nc.vector.tensor_tensor(out=ctx_m[:], in0=ctxT2[:], in1=cmask[:],
                                    op=OP.mult)
            ctx_c = sp.tile([H, 1], f32, tag="ctxco")
            nc.vector.tensor_reduce(out=ctx_c[:], in_=ctx_m[:],
                                    axis=mybir.AxisListType.X, op=OP.add)
            bvc = sp.tile([H, 1], f32, tag="bvc")
            nc.sync.dma_start(out=bvc[:], in_=d_bv[:])
            nc.vector.tensor_tensor(out=ctx_c[:], in0=ctx_c[:], in1=bvc[:],
                                    op=OP.add)

            # g = layer_norm(a + ctx @ wo + bo)
            wo_t = sp.tile([H, H], f32, tag="w64e")
            nc.sync.dma_start(out=wo_t[:], in_=d_wo[:])
            go_ps = ps.tile([1, H], f32, tag="sm1", space="PSUM")
            nc.tensor.matmul(go_ps[:], lhsT=ctx_c[:], rhs=wo_t[:], start=True,
                             stop=True)
            bor = sp.tile([1, H], f32, tag="bor")
            nc.sync.dma_start(out=bor[:], in_=d_bo[:])
            g_row = sp.tile([1, H], f32, tag="grow")
            nc.vector.tensor_tensor(out=g_row[:], in0=go_ps[:], in1=bor[:],
                                    op=OP.add)
            nc.vector.tensor_tensor(out=g_row[:], in0=g_row[:], in1=a_row[:],
                                    op=OP.add)
            mu = sp.tile([1, 1], f32, tag="mu")
            nc.vector.tensor_reduce(out=mu[:], in_=g_row[:],
                                    axis=mybir.AxisListType.X, op=OP.add)
            nc.vector.tensor_scalar(out=mu[:], in0=mu[:], scalar1=1.0 / H,
                                    scalar2=None, op0=OP.mult)
            nc.vector.tensor_scalar(out=g_row[:], in0=g_row[:], scalar1=mu[:],
                                    scalar2=None, op0=OP.subtract)
            sq = sp.tile([1, H], f32, tag="sq")
            nc.scalar.activation(out=sq[:], in_=g_row[:], func=AF.Square)
            var = sp.tile([1, 1], f32, tag="var")
            nc.vector.tensor_reduce(out=var[:], in_=sq[:],
                                    axis=mybir.AxisListType.X, op=OP.add)
            std = sp.tile([1, 1], f32, tag="std")
            eps_t = sp.tile([1, 1], f32, tag="eps")
            nc.gpsimd.memset(eps_t[:], 1e-5)
            nc.scalar.activation(out=std[:], in_=var[:], func=AF.Sqrt,
                                 scale=1.0 / H, bias=eps_t[:])
            rstd = sp.tile([1, 1], f32, tag="rstd")
            nc.vector.reciprocal(out=rstd[:], in_=std[:])
            nc.vector.tensor_scalar(out=g_row[:], in0=g_row[:], scalar1=rstd[:],
                                    scalar2=None, op0=OP.mult)
            lng = sp.tile([1, H], f32, tag="lng")
            lnb = sp.tile([1, H], f32, tag="lnb")
            nc.sync.dma_start(out=lng[:], in_=d_lng[:])
            nc.sync.dma_start(out=lnb[:], in_=d_lnb[:])
            nc.vector.tensor_tensor(out=g_row[:], in0=g_row[:], in1=lng[:],
                                    op=OP.mult)
            nc.vector.tensor_tensor(out=g_row[:], in0=g_row[:], in1=lnb[:],
                                    op=OP.add)

            # logits = hT^T @ (g/8 + policy_w), masked (own half)
            pol = sp.tile([1, H], f32, tag="pol")
            nc.sync.dma_start(out=pol[:], in_=d_pol[:])
            nc.vector.tensor_scalar(out=g_row[:], in0=g_row[:], scalar1=1.0 / 8.0,
                                    scalar2=None, op0=OP.mult)
            nc.vector.tensor_tensor(out=g_row[:], in0=g_row[:], in1=pol[:],
                                    op=OP.add)
            wT_ps = ps.tile([H, 1], f32, tag="sm1", space="PSUM")
            nc.tensor.transpose(wT_ps[:], in_=g_row[:], identity=ident[:1, :1])
            w_col = sp.tile([H, 1], bf16, tag="wcol")
            nc.scalar.activation(out=w_col[:], in_=wT_ps[:], func=AF.Copy)
            lg_ps = ps.tile([128, NBUCK], f32, tag="sm1", space="PSUM")
            for t in range(NBUCK):
                nc.tensor.matmul(lg_ps[:, t:t + 1],
                                 lhsT=hT[:H, t * 128:(t + 1) * 128],
                                 rhs=w_col[:], start=True, stop=True)
            maskf = sp.tile([128, NBUCK], f32, tag="mf")
            maskn = sp.tile([128, NBUCK], f32, tag="mn")
            nc.sync.dma_start(out=maskf[:], in_=d_maskf[:])
            nc.sync.dma_start(out=maskn[:], in_=d_maskn[:])
            lg = sp.tile([128, NBUCK], f32, tag="lgsb")
            nc.vector.tensor_tensor(out=lg[:], in0=lg_ps[:], in1=maskf[:],
                                    op=OP.mult)
            nc.vector.tensor_tensor(out=lg[:], in0=lg[:], in1=maskn[:],
                                    op=OP.add)
            nc.sync.dma_start(out=d_out[:], in_=lg[:])

    nc.compile()
    return nc


def _wrap16(a):
    w = a.reshape(-1, 16).T
    return np.tile(w, (8, 1)).astype(np.int16)


def _prep_core(inputs, s, half):
    gn = np.asarray(inputs["graph_nodes"])
    links = np.asarray(inputs["graph_edge_links"])
    mask = np.asarray(inputs["mask"])

    x = np.zeros((NPAD, F_NODE), np.float32)
    x[:N_HALF] = gn[s, half * N_HALF:(half + 1) * N_HALF]
    xT = np.ascontiguousarray(x.T).astype(ml_dtypes.bfloat16)
    xf = np.zeros((2, NPAD, F_NODE), np.float32)
    xf[0, :N_HALF] = gn[s, :N_HALF]
    xf[1, :N_HALF] = gn[s, N_HALF:]
    xTF = np.ascontiguousarray(
        xf.reshape(NTAB, F_NODE).T).astype(ml_dtypes.bfloat16)

    src = links[s, 0].astype(np.int64)
    dst = links[s, 1].astype(np.int64)
    sel = (dst >= half * N_HALF) & (dst < (half + 1) * N_HALF)
    src_e = src[sel]
    dst_e = dst[sel]
    dl = dst_e - half * N_HALF                    # local 0..N_HALF
    psrc = src_e + (NPAD - N_HALF) * (src_e >= N_HALF)   # row in [0, NTAB)

    order = np.argsort(dl, kind="stable")
    dls = dl[order]
    pss = psrc[order]
    counts = np.bincount(dls, minlength=N_HALF)
    starts = np.zeros(N_HALF, np.int64)
    starts[1:] = np.cumsum(counts)[:-1]
    rank = np.arange(len(dls)) - starts[dls]

    mainsel = rank < K
    mn, mr, mp_ = dls[mainsel], rank[mainsel], pss[mainsel]
    idxM = np.full(MCOLS * 128, SENT1, np.int64)
    slot = ((mn >> 7) * K + mr) * 128 + (mn & 127)
    idxM[slot] = mp_

    on, op_ = dls[~mainsel], pss[~mainsel]
    ob = on >> 7
    ocounts = np.bincount(ob, minlength=NBUCK)
    if ocounts.max() > OVFCH * 128:
        raise RuntimeError(f"ovf overflow: {ocounts.max()} > {OVFCH * 128}")
    ostarts = np.zeros(NBUCK, np.int64)
    ostarts[1:] = np.cumsum(ocounts)[:-1]
    within = np.arange(len(on)) - ostarts[ob]
    oslot = ob * (OVFCH * 128) + within
    idxOs = np.full(OCOLS * 128, SENT1, np.int64)
    idxOd = np.full(OCOLS * 128, SENT2, np.int64)
    dlocv = np.full(OCOLS * 128, 128, np.float32)
    idxOs[oslot] = op_
    idxOd[oslot] = on
    dlocv[oslot] = (on & 127)

    blocks = []
    for (c0, og) in _ovf_groups():
        blocks.append(_wrap16(idxOs[c0 * 128:(c0 + og) * 128]))
        blocks.append(_wrap16(idxOd[c0 * 128:(c0 + og) * 128]))
    cmbO = np.ascontiguousarray(np.concatenate(blocks, axis=1))
    cmbM = _wrap16(idxM)
    dl_cols = dlocv.reshape(OCOLS, 128).T
    dlocO = np.ascontiguousarray(
        np.repeat(dl_cols, 2, axis=1)).astype(ml_dtypes.bfloat16)

    m = np.zeros(NPAD, bool)
    m[:N_HALF] = mask[s, half * N_HALF:(half + 1) * N_HALF]
    maskf = np.where(m, np.float32(1.0), np.float32(0.0))
    pb = np.float32(np.asarray(inputs["policy_b"]))
    maskn = np.where(m, pb, NEG)
    maskf = np.ascontiguousarray(maskf.reshape(NBUCK, 128).T)
    maskn = np.ascontiguousarray(maskn.reshape(NBUCK, 128).T)

    return {
        "xT": xT, "xTF": xTF, "cmbM": cmbM, "cmbO": cmbO, "dlocO": dlocO,
        "maskf": maskf, "maskneg": maskn,
        "ad_col": np.asarray(inputs["current_ad"])[s].reshape(F_AD, 1)
                    .astype(np.float32),
    }


def kernel(**inputs):
    from concourse.bass_utils import run_bass_kernel_spmd

    if "nc" not in _CACHE:
        _CACHE["nc"] = _build()
    nc = _CACHE["nc"]

    f = lambda k: np.ascontiguousarray(np.asarray(inputs[k], np.float32))
    bf = lambda a: np.ascontiguousarray(a).astype(ml_dtypes.bfloat16)
    iot = np.tile(np.arange(128, dtype=np.float32), (128, 1))
    blockmask = np.zeros((H, HEADS), np.float32)
    for hh in range(HEADS):
        blockmask[hh * DH:(hh + 1) * DH, hh] = 1.0

    msg_w = f("msg_w")
    upd_w = f("upd_w")
    common = {
        "iota128": iot.astype(ml_dtypes.bfloat16),
        "ones_row": np.ones((1, NPAD), ml_dtypes.bfloat16),
        "w_node16": bf(f("w_node")),
        "b_node_col": f("b_node").reshape(H, 1),
        "msgw1": bf(np.concatenate(
            [msg_w[:, :H, :].transpose(1, 0, 2),
             f("msg_b").reshape(1, L, H)], axis=0)),
        "msgw2": bf(msg_w[:, H:, :].transpose(1, 0, 2)),
        "bias_rep": np.tile(f("msg_b").reshape(1, L, H), (128, 1, 1)),
        "updw1": bf(upd_w[:, :H, :].transpose(1, 0, 2)),
        "updw2": bf(upd_w[:, H:, :].transpose(1, 0, 2)),
        "upd_b_col": np.ascontiguousarray(f("upd_b").T),
        "w_ad": f("w_ad"), "b_ad_row": f("b_ad").reshape(1, H),
        "wq": f("wq"), "bq_row": f("bq").reshape(1, H),
        "wk16": bf(f("wk")), "wv16": bf(f("wv")),
        "bv_col": f("bv").reshape(H, 1),
        "wo": f("wo"), "bo_row": f("bo").reshape(1, H),
        "ln_g_row": f("ln_g").reshape(1, H), "ln_b_row": f("ln_b").reshape(1, H),
        "qmask": blockmask * np.float32(1.0 / np.sqrt(DH)),
        "cmask": blockmask,
        "pol_row": f("policy_w").reshape(1, H),
    }

    in_maps = []
    for c in range(NCORES):
        m = dict(common)
        m.update(_prep_core(inputs, c // 2, c % 2))
        in_maps.append(m)

    res = run_bass_kernel_spmd(nc, in_maps, core_ids=list(range(NCORES)))
    _CACHE["last_results"] = res

    out = np.empty((B, N), np.float32)
    for c in range(NCORES):
        s, half = c // 2, c % 2
        lg = np.asarray(res.results[c]["logits"])      # [128, NBUCK]
        flat = lg.T.reshape(NPAD)
        out[s, half * N_HALF:(half + 1) * N_HALF] = flat[:N_HALF]
    return out



# revision 35
# speedup vs baseline: 1.2729x; 1.0850x over previous
"""Trainium2 Bass kernel for BillboardAllocatorGNN.

Sharding: 8 cores; core c handles sample c//2, node-half c%2 (data parallel
over batch, dst-parallel within each sample pair).

Edge phase (per layer): node-major edge slots with fixed per-node capacity
K=18 kill both the dst-side gather and the one-hot scatter for 96% of edges:
a single SWDGE gather fetches duplicated-bf16 hs1 rows (256B descriptors)
into [dst-node-partition, slot] layout, the dst-side hs2 contribution is a
free-dim broadcast add from SBUF, and segment-sum is a strided free-axis
tensor_reduce. Overflow edges (deg>K) go through a small one-hot matmul
side path whose PSUM accumulation group also hosts the main agg transpose.
Pad slots gather a -30000 sentinel row so relu zeroes them.

Pipelining: updates, next-layer table builds, and attention score/value
chunks are interleaved into the main gather stream. Layer 0 builds its
table locally from raw features (no collective); layer 1 exchanges hs1
in slices issued as updates complete; the final attention merges across
the pair via a 2KB unnormalized-softmax stats exchange (scores are O(1),
so no max subtraction is needed).
"""
import sys
import os

sys.path.insert(0, "/opt/trn_rl_repo")

import numpy as np
import ml_dtypes

# ---- problem dims (hardcoded per spec) ----
B, N, E = 4, 20000, 320000
F_NODE, F_AD = 16, 8
H, L, HEADS = 64, 2, 4
DH = H // HEADS

NCORES = 8
N_HALF = N // 2                 # 10000 real nodes per core
NBUCK = 79                      # 128-node pages per core
NPAD = NBUCK * 128              # 10112 padded nodes per core
K = 16                          # main slots per node
MCOLS = NBUCK * K               # main gather columns
PPG = 2                         # pages per main gather group
NGRP = (NBUCK + PPG - 1) // PPG
OVB = 16                        # overflow chunks per gather batch
NTAB = 2 * NPAD                 # hs1 rows (both halves)
SENT1 = NTAB                    # sentinel row in hs1 table
SENT2 = NPAD                    # sentinel row in hs2 table (local ids)
CH = 512                        # node-chunk for encoder/update/attention
NEG = np.float32(-1e9)

_CACHE = {}


def _main_groups():
    out = []
    b = 0
    while b < NBUCK:
        g = min(PPG, NBUCK - b)
        out.append((b, g))
        b += g
    return out


def _build(nch, chunk_pages):
    # nch[g]: overflow chunks for main group g (shared static layout,
    # max over cores). chunk_pages[ci]: pages-within-group (0..PPG-1)
    # that chunk ci can scatter into (union over cores).
    TCH = sum(nch)
    choff = [0]
    for v in nch:
        choff.append(choff[-1] + v)
    import concourse.mybir as mybir
    import concourse.tile as tile
    import concourse.bacc as bacc
    from concourse.tile import add_dep_helper
    from concourse.masks import make_identity

    f32 = mybir.dt.float32
    bf16 = mybir.dt.bfloat16
    i16 = mybir.dt.int16
    AF = mybir.ActivationFunctionType
    OP = mybir.AluOpType

    nc = bacc.Bacc("TRN2", target_bir_lowering=False, debug=False,
                   num_swdge_queues=2)

    # ---- I/O ----
    d_xT = nc.dram_tensor("xT", [F_NODE, NPAD], bf16, kind="ExternalInput")
    d_xTF = nc.dram_tensor("xTF", [F_NODE, NTAB], bf16, kind="ExternalInput")
    d_cmbM = nc.dram_tensor("cmbM", [128, MCOLS * 8], i16, kind="ExternalInput")
    d_cmbO = nc.dram_tensor("cmbO", [128, 2 * TCH * 8], i16,
                            kind="ExternalInput")
    d_dlocO = nc.dram_tensor("dlocO", [128, max(TCH, 1)], bf16,
                             kind="ExternalInput")
    d_iota = nc.dram_tensor("iota256", [128, 256], bf16, kind="ExternalInput")
    d_ones = nc.dram_tensor("ones_row", [1, NPAD], bf16, kind="ExternalInput")
    d_maskf = nc.dram_tensor("maskf", [128, NBUCK], f32, kind="ExternalInput")
    d_maskn = nc.dram_tensor("maskneg", [128, NBUCK], f32, kind="ExternalInput")
    d_wnode = nc.dram_tensor("w_node16", [F_NODE, H], bf16,
                             kind="ExternalInput")
    d_bnode = nc.dram_tensor("b_node_col", [H, 1], f32, kind="ExternalInput")
    d_msgw1 = nc.dram_tensor("msgw1", [H + 1, L, H], bf16,
                             kind="ExternalInput")
    d_msgw2 = nc.dram_tensor("msgw2", [H, L, H], bf16, kind="ExternalInput")
    d_brep = nc.dram_tensor("bias_rep", [128, L, H], f32, kind="ExternalInput")
    d_updw1 = nc.dram_tensor("updw1", [H, L, H], bf16, kind="ExternalInput")
    d_updw2 = nc.dram_tensor("updw2", [H, L, H], bf16, kind="ExternalInput")
    d_updb = nc.dram_tensor("upd_b_col", [H, L], f32, kind="ExternalInput")
    d_wad = nc.dram_tensor("w_ad", [F_AD, H], f32, kind="ExternalInput")
    d_bad = nc.dram_tensor("b_ad_row", [1, H], f32, kind="ExternalInput")
    d_ad = nc.dram_tensor("ad_col", [F_AD, 1], f32, kind="ExternalInput")
    d_wq = nc.dram_tensor("wq", [H, H], f32, kind="ExternalInput")
    d_bq = nc.dram_tensor("bq_row", [1, H], f32, kind="ExternalInput")
    d_wk16 = nc.dram_tensor("wk16", [H, H], bf16, kind="ExternalInput")
    d_wv16 = nc.dram_tensor("wv16", [H, H], bf16, kind="ExternalInput")
    d_bv = nc.dram_tensor("bv_col", [H, 1], f32, kind="ExternalInput")
    d_wo = nc.dram_tensor("wo", [H, H], f32, kind="ExternalInput")
    d_bo = nc.dram_tensor("bo_row", [1, H], f32, kind="ExternalInput")
    d_lng = nc.dram_tensor("ln_g_row", [1, H], f32, kind="ExternalInput")
    d_lnb = nc.dram_tensor("ln_b_row", [1, H], f32, kind="ExternalInput")
    d_qmask = nc.dram_tensor("qmask", [H, HEADS], f32, kind="ExternalInput")
    d_cmask = nc.dram_tensor("cmask", [H, HEADS], f32, kind="ExternalInput")
    d_pol = nc.dram_tensor("pol_row", [1, H], f32, kind="ExternalInput")
    d_out = nc.dram_tensor("logits", [128, NBUCK], f32, kind="ExternalOutput")

    PAIRS = [[0, 1], [2, 3], [4, 5], [6, 7]]
    n_ch = (NPAD + CH - 1) // CH
    TB = 8                      # node pages per table-export batch

    with tile.TileContext(nc) as tc:
        with (
            tc.tile_pool(name="persist", bufs=1) as pp,
            tc.tile_pool(name="mp", bufs=2) as mp,
            tc.tile_pool(name="g3", bufs=4) as g3,
            tc.tile_pool(name="gx", bufs=2) as gx,
            tc.tile_pool(name="single", bufs=1) as sp,
            tc.tile_pool(name="psum", bufs=2, space="PSUM") as ps,
            tc.tile_pool(name="dram", bufs=1, space="DRAM") as dp,
        ):
            # ---- persistent state / constants ----
            hT = pp.tile([H + 1, NPAD], bf16)      # node states + ones row
            aggT = pp.tile([H, NPAD], bf16)        # per-layer aggregate
            hs2 = pp.tile([128, NBUCK, H], bf16)   # dst table, node-major
            ovfmsg = pp.tile([128, max(TCH, 1), H], bf16)
            cmbO = pp.tile([128, 2 * TCH * 8], i16)
            dlocO = pp.tile([128, max(TCH, 1)], bf16)
            iota = pp.tile([128, 256], bf16)
            ident = pp.tile([128, 128], f32)
            wnode = pp.tile([F_NODE, H], bf16)
            bnode = pp.tile([H, 1], f32)
            msgw1 = pp.tile([H + 1, L, H], bf16)
            msgw2 = pp.tile([H, L, H], bf16)
            updw1 = pp.tile([H, L, H], bf16)
            updw2 = pp.tile([H, L, H], bf16)
            updb = pp.tile([H, L], f32)

            make_identity(nc, ident[:])
            nc.sync.dma_start(out=hT[H:H + 1, :], in_=d_ones[:])
            ident_bf4 = pp.tile([HEADS, HEADS], bf16)
            nc.vector.tensor_copy(out=ident_bf4[:], in_=ident[:HEADS, :HEADS])

            for dst_t, src_t in [
                (cmbO, d_cmbO), (dlocO, d_dlocO),
                (iota, d_iota), (wnode, d_wnode), (bnode, d_bnode),
                (msgw1, d_msgw1), (msgw2, d_msgw2),
                (updw1, d_updw1), (updw2, d_updw2), (updb, d_updb),
            ]:
                nc.sync.dma_start(out=dst_t[:], in_=src_t[:])

            # ---- DRAM: gather tables, hs1 exchange, attention payload ----
            # Separate hs1/hs2 tables so the tile framework's DRAM dep
            # tracking doesn't serialize layer-l hs1 gathers behind the
            # layer-(l+1) hs2 rewrites issued mid-stream.
            tab = dp.tile([NTAB + 1, 128], bf16)     # hs1 rows + sentinel
            tab2 = dp.tile([NPAD + 1, 128], bf16)    # hs2 rows + sentinel
            hs1own = dp.tile([NPAD, H], bf16)
            hs1pair = dp.tile([NTAB, H], bf16)
            pay_own = dp.tile([HEADS, H + 2], f32)
            pay_full = dp.tile([2, HEADS, H + 2], f32)

            # sentinel rows (once; hs1/hs2 rewrites never touch them)
            sentc = sp.tile([1, 128], bf16, tag="sent")
            nc.gpsimd.memset(sentc[:], -30000.0)
            w_sent = nc.sync.dma_start(out=tab[SENT1:SENT1 + 1, :],
                                       in_=sentc[:])
            w_sent2 = nc.sync.dma_start(out=tab2[SENT2:SENT2 + 1, :],
                                        in_=sentc[:])

            # ---- node encoder: hT = relu(w_node^T @ xT + b) ----
            for i in range(n_ch):
                lo = i * CH
                sz = min(CH, NPAD - lo)
                xt = mp.tile([F_NODE, CH], bf16, tag="xtf")
                nc.sync.dma_start(out=xt[:, :sz], in_=d_xT[:, lo:lo + sz])
                h_ps = ps.tile([H, CH], f32, tag="big", space="PSUM")
                nc.tensor.matmul(h_ps[:, :sz], lhsT=wnode[:], rhs=xt[:, :sz],
                                 start=True, stop=True)
                nc.vector.tensor_scalar(out=hT[:H, lo:lo + sz],
                                        in0=h_ps[:, :sz], scalar1=bnode[:],
                                        scalar2=0.0, op0=OP.add, op1=OP.max)

            # ---- attention prelude: q from ad (independent of layers) ----
            wad = sp.tile([F_AD, H], f32, tag="w64")
            adc = sp.tile([F_AD, 1], f32, tag="col")
            nc.sync.dma_start(out=wad[:], in_=d_wad[:])
            nc.sync.dma_start(out=adc[:], in_=d_ad[:])
            a_ps = ps.tile([1, H], f32, tag="sm1", space="PSUM")
            nc.tensor.matmul(a_ps[:], lhsT=adc[:], rhs=wad[:], start=True,
                             stop=True)
            bad = sp.tile([1, H], f32, tag="row1")
            nc.sync.dma_start(out=bad[:], in_=d_bad[:])
            a_row = sp.tile([1, H], f32, tag="arow")
            nc.vector.tensor_tensor(out=a_row[:], in0=a_ps[:], in1=bad[:],
                                    op=OP.add)
            nc.vector.tensor_scalar(out=a_row[:], in0=a_row[:], scalar1=0.0,
                                    scalar2=None, op0=OP.max)
            aT_ps = ps.tile([H, 1], f32, tag="sm1", space="PSUM")
            nc.tensor.transpose(aT_ps[:], in_=a_row[:], identity=ident[:1, :1])
            a_col = sp.tile([H, 1], f32, tag="acol")
            nc.scalar.activation(out=a_col[:], in_=aT_ps[:], func=AF.Copy)
            wq_t = sp.tile([H, H], f32, tag="w64b")
            nc.sync.dma_start(out=wq_t[:], in_=d_wq[:])
            q_ps = ps.tile([1, H], f32, tag="sm1", space="PSUM")
            nc.tensor.matmul(q_ps[:], lhsT=a_col[:], rhs=wq_t[:], start=True,
                             stop=True)
            bqr = sp.tile([1, H], f32, tag="row2")
            nc.sync.dma_start(out=bqr[:], in_=d_bq[:])
            q_row = sp.tile([1, H], f32, tag="qrow")
            nc.vector.tensor_tensor(out=q_row[:], in0=q_ps[:], in1=bqr[:],
                                    op=OP.add)
            qT_ps = ps.tile([H, 1], f32, tag="sm1", space="PSUM")
            nc.tensor.transpose(qT_ps[:], in_=q_row[:], identity=ident[:1, :1])
            qmask = sp.tile([H, HEADS], f32, tag="qm")
            nc.sync.dma_start(out=qmask[:], in_=d_qmask[:])
            q_col = sp.tile([H, 1], f32, tag="qcol")
            nc.scalar.activation(out=q_col[:], in_=qT_ps[:], func=AF.Copy)
            qblk = sp.tile([H, HEADS], bf16, tag="qblk")
            nc.vector.tensor_tensor(out=qblk[:],
                                    in0=q_col[:].to_broadcast([H, HEADS]),
                                    in1=qmask[:], op=OP.mult)
            wk16 = sp.tile([H, H], bf16, tag="w64c")
            wv16 = sp.tile([H, H], bf16, tag="w64d")
            nc.sync.dma_start(out=wk16[:], in_=d_wk16[:])
            nc.sync.dma_start(out=wv16[:], in_=d_wv16[:])

            # attention state (filled during the last layer's main stream)
            scores = pp.tile([HEADS, NPAD], bf16)    # exp(raw), unnormalized
            vall = pp.tile([128, NBUCK, H], bf16)
            sm = sp.tile([HEADS, n_ch], f32, tag="sm")

            # ---- message-passing layers, software-pipelined ----
            SL = [(0, 48), (48, 72), (72, NBUCK)]    # cc slices (pages)
            hs1pairs = []
            for _si, (p0, p1) in enumerate(SL):
                hs1pair_s = dp.tile([2 * 128 * (p1 - p0), H], bf16,
                                    name=f"hs1pair_{_si}")
                hs1pairs.append(hs1pair_s)
            hs2t = []
            for _li in range(L):
                hs2_l = pp.tile([128, NBUCK, H], bf16, name=f"hs2_{_li}")
                hs2t.append(hs2_l)

            gathers = [[] for _ in range(L)]         # per-layer gather instrs
            hs2_writes = [[] for _ in range(L)]
            ccs = [[] for _ in range(L)]
            expands = [[] for _ in range(L)]

            def hs2_batch(l, t0):
                tn = min(TB, NBUCK - t0)
                p2 = ps.tile([128, TB * H], f32, tag="tab8", space="PSUM")
                for j in range(tn):
                    nc.tensor.matmul(
                        p2[:, j * H:(j + 1) * H],
                        lhsT=hT[:H, (t0 + j) * 128:(t0 + j + 1) * 128],
                        rhs=msgw2[:, l, :], start=True, stop=True)
                nc.scalar.activation(
                    out=hs2t[l][:, t0:t0 + tn, :],
                    in_=p2[:, :tn * H].rearrange("p (t f) -> p t f", f=H),
                    func=AF.Copy)
                dst0 = tab2[t0 * 128:(t0 + tn) * 128,
                            0:H].rearrange("(t p) f -> p t f", p=128)
                w1 = nc.sync.dma_start(out=dst0, in_=hs2t[l][:, t0:t0 + tn, :])
                if l > 0:
                    # WAR: previous layer's overflow-dst gathers read this
                    nogrp = (TCH + OVB - 1) // OVB
                    for gi in gathers[l - 1][:nogrp]:
                        add_dep_helper(w1.ins, gi.ins,
                                       reason="WAR: hs2 rewrite after gathers")
                hs2_writes[l].append(w1)

            def hs1_batch(l, t0):
                tn = min(TB, NBUCK - t0)
                p1 = ps.tile([128, TB * H], f32, tag="tab8", space="PSUM")
                for j in range(tn):
                    nc.tensor.matmul(
                        p1[:, j * H:(j + 1) * H],
                        lhsT=hT[:, (t0 + j) * 128:(t0 + j + 1) * 128],
                        rhs=msgw1[:, l, :], start=True, stop=True)
                st = mp.tile([128, TB, H], bf16, tag="st")
                nc.scalar.activation(
                    out=st[:, :tn, :],
                    in_=p1[:, :tn * H].rearrange("p (t f) -> p t f", f=H),
                    func=AF.Copy)
                nc.sync.dma_start(
                    out=hs1own[t0 * 128:(t0 + tn) * 128, :].rearrange(
                        "(t p) f -> p t f", p=128),
                    in_=st[:, :tn, :])

            def issue_cc(l, s):
                p0, p1 = SL[s]
                cc0 = nc.gpsimd.collective_compute(
                    "AllGather", mybir.AluOpType.bypass, replica_groups=PAIRS,
                    ins=[hs1own[p0 * 128:p1 * 128, :]],
                    outs=[hs1pairs[s].opt()])
                if l > 0:
                    for e in expands[l - 1]:
                        add_dep_helper(cc0.ins, e.ins,
                                       reason="WAR: pair rewrite after expand")
                ccs[l].append(cc0)

            def expand_tab(l):
                # copy exchanged hs1 slices into tab hs1 region (cols 0:64)
                for s, (p0, p1) in enumerate(SL):
                    n_s = 128 * (p1 - p0)
                    cc0 = ccs[l][s]
                    for hf in range(2):
                        base = hf * NPAD + p0 * 128
                        e = nc.sync.dma_start(
                            out=tab[base:base + n_s, 0:H].rearrange(
                                "(t p) f -> p t f", p=128),
                            in_=hs1pairs[s][hf * n_s:(hf + 1) * n_s,
                                            :].rearrange(
                                "(t p) f -> p t f", p=128))
                        add_dep_helper(e.ins, cc0.ins,
                                       reason="RAW: expand after allgather")
                        if l > 0:
                            for gi in gathers[l - 1]:
                                add_dep_helper(
                                    e.ins, gi.ins,
                                    reason="WAR: tab rewrite after gathers")
                        expands[l].append(e)

            def update_chunk(l, c):
                lo = c * CH
                sz = min(CH, NPAD - lo)
                u_ps = ps.tile([H, CH], f32, tag="big", space="PSUM")
                nc.tensor.matmul(u_ps[:, :sz], lhsT=updw1[:, l, :],
                                 rhs=hT[:H, lo:lo + sz], start=True, stop=False)
                nc.tensor.matmul(u_ps[:, :sz], lhsT=updw2[:, l, :],
                                 rhs=aggT[:, lo:lo + sz], start=False,
                                 stop=True)
                un = mp.tile([H, CH], bf16, tag="un")
                nc.scalar.activation(out=un[:, :sz], in_=u_ps[:, :sz],
                                     func=AF.Relu, bias=updb[:, l:l + 1])
                nc.vector.tensor_tensor(out=hT[:H, lo:lo + sz],
                                        in0=hT[:H, lo:lo + sz],
                                        in1=un[:, :sz], op=OP.add)

            def att_chunk(c):
                lo = c * CH
                sz = min(CH, NPAD - lo)
                nt = sz // 128
                kT_ps = ps.tile([H, CH], f32, tag="big", space="PSUM")
                nc.tensor.matmul(kT_ps[:, :sz], lhsT=wk16[:],
                                 rhs=hT[:H, lo:lo + sz], start=True, stop=True)
                kT_sb = mp.tile([H, CH], bf16, tag="kT")
                nc.scalar.activation(out=kT_sb[:, :sz], in_=kT_ps[:, :sz],
                                     func=AF.Copy)
                s_ps = ps.tile([HEADS, CH], f32, tag="sm1", space="PSUM")
                nc.tensor.matmul(s_ps[:, :sz], lhsT=qblk[:], rhs=kT_sb[:, :sz],
                                 start=True, stop=True)
                nc.scalar.activation(out=scores[:, lo:lo + sz],
                                     in_=s_ps[:, :sz], func=AF.Exp)
                if lo + sz > N_HALF:
                    nc.gpsimd.memset(scores[:, N_HALF:], 0.0)
                nc.vector.tensor_reduce(out=sm[:, c:c + 1],
                                        in_=scores[:, lo:lo + sz],
                                        axis=mybir.AxisListType.X, op=OP.add)
                v_ps = ps.tile([128, 4 * H], f32, tag="tab8", space="PSUM")
                for j in range(nt):
                    nc.tensor.matmul(
                        v_ps[:, j * H:(j + 1) * H],
                        lhsT=hT[:H, lo + j * 128:lo + (j + 1) * 128],
                        rhs=wv16[:], start=True, stop=True)
                nc.scalar.activation(
                    out=vall[:, lo // 128:lo // 128 + nt, :],
                    in_=v_ps[:, :nt * H].rearrange("p (t f) -> p t f", f=H),
                    func=AF.Copy)

            def post_update(l, c):
                if l + 1 < L:
                    if c >= 2 and c % 2 == 0:
                        hs2_batch(l + 1, 4 * (c - 2))
                        hs1_batch(l + 1, 4 * (c - 2))
                    if c == 13:
                        issue_cc(l + 1, 0)
                    elif c == 19:
                        issue_cc(l + 1, 1)
                else:
                    att_chunk(c)

            def gdeps(l, gi, writes, after=()):
                add_dep_helper(gi.ins, w_sent.ins, reason="RAW: sentinel")
                add_dep_helper(gi.ins, w_sent2.ins, reason="RAW: sentinel2")
                for w in writes:
                    add_dep_helper(gi.ins, w.ins, reason="RAW: tab write")
                for cx in after:
                    add_dep_helper(gi.ins, cx.ins, reason="RAW: tab ready")
                gathers[l].append(gi)

            def ovf_dst_stream(l):
                for c0 in range(0, TCH, OVB):
                    og = min(OVB, TCH - c0)
                    goutO = mp.tile([128, OVB, 128], bf16, tag="goutO")
                    nidx = og * 128
                    gi = nc.gpsimd.dma_gather(
                        out_ap=goutO[:, :og, :], in_ap=tab2[:],
                        idxs_ap=cmbO[:, (TCH + c0) * 8:(TCH + c0 + og) * 8],
                        num_idxs=nidx, num_idxs_reg=nidx, elem_size=128,
                        queue_num=0, single_packet=False)
                    gdeps(l, gi, hs2_writes[l])
                    nc.vector.tensor_copy(out=ovfmsg[:, c0:c0 + og, :],
                                          in_=goutO[:, 0:og, 0:H])

            def main_phase(l):
                # overflow src rows + messages
                for c0 in range(0, TCH, OVB):
                    og = min(OVB, TCH - c0)
                    goutO = mp.tile([128, OVB, 128], bf16, tag="goutO")
                    nidx = og * 128
                    gi = nc.gpsimd.dma_gather(
                        out_ap=goutO[:, :og, :], in_ap=tab[:],
                        idxs_ap=cmbO[:, c0 * 8:(c0 + og) * 8],
                        num_idxs=nidx, num_idxs_reg=nidx, elem_size=128,
                        queue_num=0, single_packet=False)
                    gdeps(l, gi, expands[l])
                    nc.vector.tensor_tensor(
                        out=ovfmsg[:, c0:c0 + og, :],
                        in0=ovfmsg[:, c0:c0 + og, :],
                        in1=goutO[:, 0:og, 0:H], op=OP.add)
                    nc.scalar.activation(out=ovfmsg[:, c0:c0 + og, :],
                                         in_=ovfmsg[:, c0:c0 + og, :],
                                         func=AF.Relu)

                next_up = 0

                PFB = 4             # gather groups per idx-prefetch copy
                pf = {}

                def issue_gather(gidx, b0, gp):
                    cols = gp * K
                    gout = g3.tile([128, PPG * K, 128], bf16, tag="gout")
                    if gidx % PFB == 0:
                        npg = min(PFB * PPG, NBUCK - b0)
                        t = gx.tile([128, PFB * PPG * K * 8], i16, tag="gidx")
                        nc.sync.dma_start(
                            out=t[:, :npg * K * 8],
                            in_=d_cmbM[:, b0 * K * 8:(b0 + npg) * K * 8])
                        pf[gidx // PFB] = (t, b0)
                    t, pb0 = pf[gidx // PFB]
                    off = (b0 - pb0) * K * 8
                    gi = nc.gpsimd.dma_gather(
                        out_ap=gout[:, :cols, :], in_ap=tab[:],
                        idxs_ap=t[:, off:off + cols * 8],
                        num_idxs=cols * 128, num_idxs_reg=cols * 128,
                        elem_size=128, queue_num=gidx % 2,
                        single_packet=False)
                    gdeps(l, gi, (), after=expands[l])
                    return gout

                def consume_group(b0, gp, gout):
                    nonlocal next_up
                    g = b0 // PPG
                    chunks = list(range(choff[g], choff[g + 1]))
                    msg = mp.tile([128, PPG * K, H], bf16, tag="msg")
                    agg = mp.tile([128, PPG, H], f32, tag="agg")
                    p_sc = ps.tile([H, PPG * 128], f32, tag="scat",
                                   space="PSUM")
                    sched = [(ci, j) for ci in chunks
                             for j in sorted(chunk_pages[ci]) if j < gp]
                    ohpg = mp.tile([128, max(len(sched), 1), 128], bf16,
                                   tag="ohpg")
                    for si, (ci, j) in enumerate(sched):
                        nc.vector.tensor_tensor(
                            out=ohpg[:, si, :],
                            in0=dlocO[:, ci:ci + 1].to_broadcast([128, 128]),
                            in1=iota[:, j * 128:(j + 1) * 128],
                            op=OP.is_equal)
                    for j in range(gp):
                        nc.vector.tensor_tensor(
                            out=msg[:, j * K:(j + 1) * K, :],
                            in0=gout[:, j * K:(j + 1) * K, 0:H],
                            in1=hs2t[l][:, b0 + j:b0 + j + 1, :].to_broadcast(
                                [128, K, H]), op=OP.add)
                        nc.vector.tensor_scalar(
                            out=msg[:, j * K:(j + 1) * K, :],
                            in0=msg[:, j * K:(j + 1) * K, :],
                            scalar1=0.0, scalar2=None, op0=OP.max)
                    for j in range(gp):
                        ovf_j = [(si, ci) for si, (ci, jj) in enumerate(sched)
                                 if jj == j]
                        nc.vector.tensor_reduce(
                            out=agg[:, j, :],
                            in_=msg[:, j * K:(j + 1) * K, :].rearrange(
                                "p j f -> p f j"),
                            axis=mybir.AxisListType.X, op=OP.add)
                        sl = p_sc[:, j * 128:(j + 1) * 128]
                        nc.tensor.matmul(sl, lhsT=agg[:, j, :], rhs=ident[:],
                                         is_transpose=True, start=True,
                                         stop=(not ovf_j))
                        for oi, (si, ci) in enumerate(ovf_j):
                            nc.tensor.matmul(
                                sl, lhsT=ovfmsg[:, ci, :],
                                rhs=ohpg[:, si, :],
                                start=False, stop=(oi == len(ovf_j) - 1))
                    nc.scalar.activation(
                        out=aggT[:, b0 * 128:(b0 + gp) * 128],
                        in_=p_sc[:, :gp * 128], func=AF.Copy)
                    while (next_up < n_ch
                           and (4 * next_up + 4) * 128 <= (b0 + gp) * 128):
                        update_chunk(l, next_up)
                        post_update(l, next_up)
                        next_up += 1

                AHEAD = 4
                pend = []
                for gidx, (b0, gp) in enumerate(_main_groups()):
                    pend.append((b0, gp, issue_gather(gidx, b0, gp)))
                    if len(pend) > AHEAD:
                        consume_group(*pend.pop(0))
                for item in pend:
                    consume_group(*item)
                while next_up < n_ch:
                    update_chunk(l, next_up)
                    post_update(l, next_up)
                    next_up += 1

            # layer 0 tables: h0 = encoder(x) is computable locally for
            # BOTH halves from the raw features -- no collective needed
            for t0 in range(0, NBUCK, TB):
                hs2_batch(0, t0)
            for gt0 in range(0, 2 * NBUCK, TB):
                tn = min(TB, 2 * NBUCK - gt0)
                xtf = mp.tile([F_NODE, TB * 128], bf16, tag="xtf")
                nc.sync.dma_start(out=xtf[:, :tn * 128],
                                  in_=d_xTF[:, gt0 * 128:(gt0 + tn) * 128])
                htmp = mp.tile([H + 1, TB * 128], bf16, tag="htmp")
                if gt0 < 2 * TB:
                    # ones row survives in the 2 rotating bufs afterwards
                    nc.gpsimd.memset(htmp[H:H + 1, :], 1.0)
                for h0 in range(0, tn * 128, CH):
                    hsz = min(CH, tn * 128 - h0)
                    hf_ps = ps.tile([H, CH], f32, tag="big", space="PSUM")
                    nc.tensor.matmul(hf_ps[:, :hsz], lhsT=wnode[:],
                                     rhs=xtf[:, h0:h0 + hsz], start=True,
                                     stop=True)
                    nc.vector.tensor_scalar(out=htmp[:H, h0:h0 + hsz],
                                            in0=hf_ps[:, :hsz],
                                            scalar1=bnode[:], scalar2=0.0,
                                            op0=OP.add, op1=OP.max)
                p1g = ps.tile([128, TB * H], f32, tag="tab8", space="PSUM")
                for j in range(tn):
                    nc.tensor.matmul(
                        p1g[:, j * H:(j + 1) * H],
                        lhsT=htmp[:, j * 128:(j + 1) * 128],
                        rhs=msgw1[:, 0, :], start=True, stop=True)
                stg = mp.tile([128, TB, H], bf16, tag="st")
                nc.scalar.activation(
                    out=stg[:, :tn, :],
                    in_=p1g[:, :tn * H].rearrange("p (t f) -> p t f", f=H),
                    func=AF.Copy)
                e0 = nc.sync.dma_start(
                    out=tab[gt0 * 128:(gt0 + tn) * 128, 0:H].rearrange(
                        "(t p) f -> p t f", p=128),
                    in_=stg[:, :tn, :])
                expands[0].append(e0)
            ovf_dst_stream(0)
            main_phase(0)

            # layer 1: tables/cc mostly issued inside layer 0's stream
            hs2_batch(1, 72)
            hs1_batch(1, 72)
            issue_cc(1, 2)
            expand_tab(1)
            ovf_dst_stream(1)
            main_phase(1)

            # ---- attention tail: sums, ctx, pair merge ----
            s_loc = sp.tile([HEADS, 1], f32, tag="m3")
            nc.vector.tensor_reduce(out=s_loc[:], in_=sm[:],
                                    axis=mybir.AxisListType.X, op=OP.add)
            ctx_ps = ps.tile([H, HEADS], f32, tag="tab8", space="PSUM")
            for i in range(0, NBUCK, 4):
                nt = min(4, NBUCK - i)
                at_ps = ps.tile([128, 4 * HEADS], bf16, tag="sm1",
                                space="PSUM")
                for j in range(nt):
                    nc.tensor.transpose(
                        at_ps[:, j * HEADS:(j + 1) * HEADS],
                        in_=scores[:, (i + j) * 128:(i + j + 1) * 128],
                        identity=ident_bf4[:])
                at_sb = mp.tile([128, 4 * HEADS], bf16, tag="atsb")
                nc.scalar.activation(out=at_sb[:, :nt * HEADS],
                                     in_=at_ps[:, :nt * HEADS], func=AF.Copy)
                for j in range(nt):
                    t = i + j
                    nc.tensor.matmul(
                        ctx_ps[:], lhsT=vall[:, t, :],
                        rhs=at_sb[:, j * HEADS:(j + 1) * HEADS],
                        start=(t == 0), stop=(t == NBUCK - 1))

            ctx_sb = sp.tile([H, HEADS], f32, tag="ctxsb")
            nc.scalar.activation(out=ctx_sb[:], in_=ctx_ps[:], func=AF.Copy)
            ctxT_ps = ps.tile([HEADS, H], f32, tag="sm1", space="PSUM")
            nc.tensor.transpose(ctxT_ps[:], in_=ctx_sb[:],
                                identity=ident[:H, :H])
            pay = sp.tile([HEADS, H + 2], f32, tag="pay")
            nc.scalar.activation(out=pay[:, 0:H], in_=ctxT_ps[:], func=AF.Copy)
            nc.vector.tensor_copy(out=pay[:, H:H + 1], in_=s_loc[:])
            nc.vector.tensor_copy(out=pay[:, H + 1:H + 2], in_=s_loc[:])
            w_pay = nc.sync.dma_start(out=pay_own[:], in_=pay[:])
            ccp = nc.gpsimd.collective_compute(
                "AllGather", mybir.AluOpType.bypass, replica_groups=PAIRS,
                ins=[pay_own.opt()], outs=[pay_full.opt()])

            p0 = sp.tile([HEADS, H + 2], f32, tag="p0")
            p1 = sp.tile([HEADS, H + 2], f32, tag="p1")
            nc.sync.dma_start(out=p0[:], in_=pay_full[0])
            nc.sync.dma_start(out=p1[:], in_=pay_full[1])
            den = sp.tile([HEADS, 1], f32, tag="den")
            nc.vector.tensor_tensor(out=den[:], in0=p0[:, H:H + 1],
                                    in1=p1[:, H:H + 1], op=OP.add)
            rden = sp.tile([HEADS, 1], f32, tag="rden")
            nc.vector.reciprocal(out=rden[:], in_=den[:])
            ctxc = sp.tile([HEADS, H], f32, tag="ctxc")
            nc.vector.tensor_tensor(out=ctxc[:], in0=p0[:, 0:H],
                                    in1=p1[:, 0:H], op=OP.add)
            nc.vector.tensor_scalar(out=ctxc[:], in0=ctxc[:], scalar1=rden[:],
                                    scalar2=None, op0=OP.mult)
            ctxT2 = ps.tile([H, HEADS], f32, tag="sm1", space="PSUM")
            nc.tensor.transpose(ctxT2[:], in_=ctxc[:],
                                identity=ident[:HEADS, :HEADS])
            cmask = sp.tile([H, HEADS], f32, tag="cm")
            nc.sync.dma_start(out=cmask[:], in_=d_cmask[:])
            ctx_m = sp.tile([H, HEADS], f32, tag="ctxm")
            nc.vector.tensor_tensor(out=ctx_m[:], in0=ctxT2[:], in1=cmask[:],
                                    op=OP.mult)
            ctx_c = sp.tile([H, 1], f32, tag="ctxco")
            nc.vector.tensor_reduce(out=ctx_c[:], in_=ctx_m[:],
                                    axis=mybir.AxisListType.X, op=OP.add)
            bvc = sp.tile([H, 1], f32, tag="bvc")
            nc.sync.dma_start(out=bvc[:], in_=d_bv[:])
            nc.vector.tensor_tensor(out=ctx_c[:], in0=ctx_c[:], in1=bvc[:],
                                    op=OP.add)

            # g = layer_norm(a + ctx @ wo + bo)
            wo_t = sp.tile([H, H], f32, tag="w64e")
            nc.sync.dma_start(out=wo_t[:], in_=d_wo[:])
            go_ps = ps.tile([1, H], f32, tag="sm1", space="PSUM")
            nc.tensor.matmul(go_ps[:], lhsT=ctx_c[:], rhs=wo_t[:], start=True,
                             stop=True)
            bor = sp.tile([1, H], f32, tag="bor")
            nc.sync.dma_start(out=bor[:], in_=d_bo[:])
            g_row = sp.tile([1, H], f32, tag="grow")
            nc.vector.tensor_tensor(out=g_row[:], in0=go_ps[:], in1=bor[:],
                                    op=OP.add)
            nc.vector.tensor_tensor(out=g_row[:], in0=g_row[:], in1=a_row[:],
                                    op=OP.add)
            mu = sp.tile([1, 1], f32, tag="mu")
            nc.vector.tensor_reduce(out=mu[:], in_=g_row[:],
                                    axis=mybir.AxisListType.X, op=OP.add)
            nc.vector.tensor_scalar(out=mu[:], in0=mu[:], scalar1=1.0 / H,
                                    scalar2=None, op0=OP.mult)
            nc.vector.tensor_scalar(out=g_row[:], in0=g_row[:], scalar1=mu[:],
                                    scalar2=None, op0=OP.subtract)
            sq = sp.tile([1, H], f32, tag="sq")
            nc.scalar.activation(out=sq[:], in_=g_row[:], func=AF.Square)
            var = sp.tile([1, 1], f32, tag="var")
            nc.vector.tensor_reduce(out=var[:], in_=sq[:],
                                    axis=mybir.AxisListType.X, op=OP.add)
            std = sp.tile([1, 1], f32, tag="std")
            eps_t = sp.tile([1, 1], f32, tag="eps")
            nc.gpsimd.memset(eps_t[:], 1e-5)
            nc.scalar.activation(out=std[:], in_=var[:], func=AF.Sqrt,
                                 scale=1.0 / H, bias=eps_t[:])
            rstd = sp.tile([1, 1], f32, tag="rstd")
            nc.vector.reciprocal(out=rstd[:], in_=std[:])
            nc.vector.tensor_scalar(out=g_row[:], in0=g_row[:], scalar1=rstd[:],
                                    scalar2=None, op0=OP.mult)
            lng = sp.tile([1, H], f32, tag="lng")
            lnb = sp.tile([1, H], f32, tag="lnb")
            nc.sync.dma_start(out=lng[:], in_=d_lng[:])
            nc.sync.dma_start(out=lnb[:], in_=d_lnb[:])
            nc.vector.tensor_tensor(out=g_row[:], in0=g_row[:], in1=lng[:],
                                    op=OP.mult)
            nc.vector.tensor_tensor(out=g_row[:], in0=g_row[:], in1=lnb[:],
                                    op=OP.add)

            # logits = hT^T @ (g/8 + policy_w), masked (own half)
            pol = sp.tile([1, H], f32, tag="pol")
            nc.sync.dma_start(out=pol[:], in_=d_pol[:])
            nc.vector.tensor_scalar(out=g_row[:], in0=g_row[:], scalar1=1.0 / 8.0,
                                    scalar2=None, op0=OP.mult)
            nc.vector.tensor_tensor(out=g_row[:], in0=g_row[:], in1=pol[:],
                                    op=OP.add)
            wT_ps = ps.tile([H, 1], f32, tag="sm1", space="PSUM")
            nc.tensor.transpose(wT_ps[:], in_=g_row[:], identity=ident[:1, :1])
            w_col = sp.tile([H, 1], bf16, tag="wcol")
            nc.scalar.activation(out=w_col[:], in_=wT_ps[:], func=AF.Copy)
            lg_ps = ps.tile([128, NBUCK], f32, tag="sm1", space="PSUM")
            for t in range(NBUCK):
                nc.tensor.matmul(lg_ps[:, t:t + 1],
                                 lhsT=hT[:H, t * 128:(t + 1) * 128],
                                 rhs=w_col[:], start=True, stop=True)
            maskf = sp.tile([128, NBUCK], f32, tag="mf")
            maskn = sp.tile([128, NBUCK], f32, tag="mn")
            nc.sync.dma_start(out=maskf[:], in_=d_maskf[:])
            nc.sync.dma_start(out=maskn[:], in_=d_maskn[:])
            lg = sp.tile([128, NBUCK], f32, tag="lgsb")
            nc.vector.tensor_tensor(out=lg[:], in0=lg_ps[:], in1=maskf[:],
                                    op=OP.mult)
            nc.vector.tensor_tensor(out=lg[:], in0=lg[:], in1=maskn[:],
                                    op=OP.add)
            nc.sync.dma_start(out=d_out[:], in_=lg[:])

    nc.compile()
    return nc


def _wrap16(a):
    w = a.reshape(-1, 16).T
    return np.tile(w, (8, 1)).astype(np.int16)


def _edge_arrays(inputs, s, half):
    links = np.asarray(inputs["graph_edge_links"])
    src = links[s, 0].astype(np.int64)
    dst = links[s, 1].astype(np.int64)
    sel = (dst >= half * N_HALF) & (dst < (half + 1) * N_HALF)
    src_e = src[sel]
    dst_e = dst[sel]
    dl = dst_e - half * N_HALF                    # local 0..N_HALF
    psrc = src_e + (NPAD - N_HALF) * (src_e >= N_HALF)   # row in [0, NTAB)
    order = np.argsort(dl, kind="stable")
    dls = dl[order]
    pss = psrc[order]
    counts = np.bincount(dls, minlength=N_HALF)
    starts = np.zeros(N_HALF, np.int64)
    starts[1:] = np.cumsum(counts)[:-1]
    rank = np.arange(len(dls)) - starts[dls]
    return dls, pss, rank


def _ovf_layout(edges):
    # shared static overflow layout: per-group chunk counts are the max
    # over cores; chunk_pages is the union of pages each chunk scatters to
    cnt = np.zeros((len(edges), NGRP), np.int64)
    for c, (dls, pss, rank) in enumerate(edges):
        og = (dls[rank >= K] >> 7) // PPG
        cnt[c] = np.bincount(og, minlength=NGRP)
    nch = [int(np.ceil(cnt[:, g].max() / 128)) for g in range(NGRP)]
    choff = np.concatenate([[0], np.cumsum(nch)]).astype(np.int64)
    tch = int(choff[-1])
    chunk_pages = [set() for _ in range(tch)]
    for dls, pss, rank in edges:
        ovfm = rank >= K
        od = dls[ovfm]
        og = (od >> 7) // PPG
        for g in range(NGRP):
            m = og == g
            ne = int(m.sum())
            if ne == 0:
                continue
            ci = choff[g] + np.arange(ne) // 128
            pg = (od[m] >> 7) - g * PPG
            for c_, p_ in set(zip(ci.tolist(), pg.tolist())):
                chunk_pages[c_].add(int(p_))
    return nch, choff, tch, chunk_pages


def _prep_core(inputs, s, half, edges_c, choff, tch):
    gn = np.asarray(inputs["graph_nodes"])
    mask = np.asarray(inputs["mask"])

    x = np.zeros((NPAD, F_NODE), np.float32)
    x[:N_HALF] = gn[s, half * N_HALF:(half + 1) * N_HALF]
    xT = np.ascontiguousarray(x.T).astype(ml_dtypes.bfloat16)
    xf = np.zeros((2, NPAD, F_NODE), np.float32)
    xf[0, :N_HALF] = gn[s, :N_HALF]
    xf[1, :N_HALF] = gn[s, N_HALF:]
    xTF = np.ascontiguousarray(
        xf.reshape(NTAB, F_NODE).T).astype(ml_dtypes.bfloat16)

    dls, pss, rank = edges_c

    mainsel = rank < K
    mn, mr, mp_ = dls[mainsel], rank[mainsel], pss[mainsel]
    idxM = np.full(MCOLS * 128, SENT1, np.int64)
    slot = ((mn >> 7) * K + mr) * 128 + (mn & 127)
    idxM[slot] = mp_

    ovfm = rank >= K
    od, op_ = dls[ovfm], pss[ovfm]
    og = (od >> 7) // PPG
    idxOs = np.full(tch * 128, SENT1, np.int64)
    idxOd = np.full(tch * 128, SENT2, np.int64)
    dlocv = np.full(tch * 128, 300.0, np.float32)
    for g in range(NGRP):
        m = og == g
        ne = int(m.sum())
        if ne == 0:
            continue
        sl = choff[g] * 128 + np.arange(ne)
        idxOs[sl] = op_[m]
        idxOd[sl] = od[m]
        dlocv[sl] = od[m] - g * PPG * 128

    blocks = ([_wrap16(idxOs[ci * 128:(ci + 1) * 128]) for ci in range(tch)]
              + [_wrap16(idxOd[ci * 128:(ci + 1) * 128]) for ci in range(tch)])
    cmbO = np.ascontiguousarray(np.concatenate(blocks, axis=1)) if tch else \
        np.zeros((128, 0), np.int16)
    cmbM = _wrap16(idxM)
    dlocO = np.ascontiguousarray(
        dlocv.reshape(max(tch, 1), 128).T).astype(ml_dtypes.bfloat16)

    m = np.zeros(NPAD, bool)
    m[:N_HALF] = mask[s, half * N_HALF:(half + 1) * N_HALF]
    maskf = np.where(m, np.float32(1.0), np.float32(0.0))
    pb = np.float32(np.asarray(inputs["policy_b"]))
    maskn = np.where(m, pb, NEG)
    maskf = np.ascontiguousarray(maskf.reshape(NBUCK, 128).T)
    maskn = np.ascontiguousarray(maskn.reshape(NBUCK, 128).T)

    return {
        "xT": xT, "xTF": xTF, "cmbM": cmbM, "cmbO": cmbO, "dlocO": dlocO,
        "maskf": maskf, "maskneg": maskn,
        "ad_col": np.asarray(inputs["current_ad"])[s].reshape(F_AD, 1)
                    .astype(np.float32),
    }


def kernel(**inputs):
    from concourse.bass_utils import run_bass_kernel_spmd

    edges = [_edge_arrays(inputs, c // 2, c % 2) for c in range(NCORES)]
    nch, choff, tch, chunk_pages = _ovf_layout(edges)
    key = (tuple(nch), tuple(tuple(sorted(s)) for s in chunk_pages))
    if _CACHE.get("key") != key:
        _CACHE["nc"] = _build(nch, chunk_pages)
        _CACHE["key"] = key
    nc = _CACHE["nc"]

    f = lambda k: np.ascontiguousarray(np.asarray(inputs[k], np.float32))
    bf = lambda a: np.ascontiguousarray(a).astype(ml_dtypes.bfloat16)
    iot = np.tile(np.arange(256, dtype=np.float32), (128, 1))
    blockmask = np.zeros((H, HEADS), np.float32)
    for hh in range(HEADS):
        blockmask[hh * DH:(hh + 1) * DH, hh] = 1.0

    msg_w = f("msg_w")
    upd_w = f("upd_w")
    common = {
        "iota256": iot.astype(ml_dtypes.bfloat16),
        "ones_row": np.ones((1, NPAD), ml_dtypes.bfloat16),
        "w_node16": bf(f("w_node")),
        "b_node_col": f("b_node").reshape(H, 1),
        "msgw1": bf(np.concatenate(
            [msg_w[:, :H, :].transpose(1, 0, 2),
             f("msg_b").reshape(1, L, H)], axis=0)),
        "msgw2": bf(msg_w[:, H:, :].transpose(1, 0, 2)),
        "bias_rep": np.tile(f("msg_b").reshape(1, L, H), (128, 1, 1)),
        "updw1": bf(upd_w[:, :H, :].transpose(1, 0, 2)),
        "updw2": bf(upd_w[:, H:, :].transpose(1, 0, 2)),
        "upd_b_col": np.ascontiguousarray(f("upd_b").T),
        "w_ad": f("w_ad"), "b_ad_row": f("b_ad").reshape(1, H),
        "wq": f("wq"), "bq_row": f("bq").reshape(1, H),
        "wk16": bf(f("wk")), "wv16": bf(f("wv")),
        "bv_col": f("bv").reshape(H, 1),
        "wo": f("wo"), "bo_row": f("bo").reshape(1, H),
        "ln_g_row": f("ln_g").reshape(1, H), "ln_b_row": f("ln_b").reshape(1, H),
        "qmask": blockmask * np.float32(1.0 / np.sqrt(DH)),
        "cmask": blockmask,
        "pol_row": f("policy_w").reshape(1, H),
    }

    in_maps = []
    for c in range(NCORES):
        m = dict(common)
        m.update(_prep_core(inputs, c // 2, c % 2, edges[c], choff, tch))
        in_maps.append(m)

    res = run_bass_kernel_spmd(nc, in_maps, core_ids=list(range(NCORES)))
    _CACHE["last_results"] = res

    out = np.empty((B, N), np.float32)
    for c in range(NCORES):
        s, half = c // 2, c % 2
        lg = np.asarray(res.results[c]["logits"])      # [128, NBUCK]
        flat = lg.T.reshape(NPAD)
        out[s, half * N_HALF:(half + 1) * N_HALF] = flat[:N_HALF]
    return out

